# revision 10
# baseline (speedup 1.0000x reference)
"""Trainium2 Bass kernel for nn_AttnDecoderRNN3 (sparse_attention).

Strategy (8 NeuronCores):
- Only input_enc[:, :10, :] matters: the attention mask is a fixed 10-wide
  window at t=0 (aw0 is a one-hot at t=0 -> argmax 0) and the softmax
  max-subtraction cancels exactly, so the full-T encoder GEMM is skipped.
- LSTM h0=c0=0 -> whh*/f-gate weights unused. Transformer S=1 -> att == V.
- LSTM layer 0 sharded over the hidden dim (each core computes a 256-wide
  h1 slice for all 32 batch rows); layer 1 sharded over the contraction
  dim (each core's h1 slice x its wih1 column slice -> partial full
  gates), combined with ONE AllReduce; bias folded in via core-0-only
  bias data (other cores carry zeros).
- Everything else is computed redundantly on all cores for all 32 batch
  rows (it is tiny); final outputs are read from core 0.
- Feature-on-partition layout everywhere; weights are the stationary
  matmul operand; biases/activations fused into PSUM->SBUF evacuation.
"""

import numpy as np

import concourse.bacc as bacc
import concourse.mybir as mybir
import concourse.tile as tile
from concourse.bass_utils import run_bass_kernel_spmd

NCORES = 8
N = 32            # batch
ENC = 512
ATT = 256
DEC = 512
H = 2048          # lstm hidden
SPK = 64
OUT = 80
FF = 1024
AW = 10           # attention window (ATT_RANGE)
HS = H // NCORES  # hidden slice per core = 256
G3 = 3 * H        # i,g,o gates = 6144
GS = 3 * HS       # per-core layer0 gate rows = 768
NA = N * AW       # 320

f32 = mybir.dt.float32
AF = mybir.ActivationFunctionType
ALU = mybir.AluOpType
AX = mybir.AxisListType


def _ceil_div(a, b):
    return (a + b - 1) // b


def gemm(nc, wp, pp, name, w_dram, K, M, rhs_tiles, evac, wbufs=4,
         col_group=512, psum_bufs=4):
    """out[M, N=32] = w^T x. w_dram is (K, M) (pre-transposed host-side).

    k-outer / m-inner per column group: weight blocks stream through a
    small pool while <= col_group/128 PSUM accumulation groups stay open.
    rhs_tiles: SBUF APs [ks, 32] covering K in order.
    evac(m_idx, msize, psum_ap): consume one [msize, 32] output tile.
    """
    kt = []
    k0 = 0
    for r in rhs_tiles:
        ks = r.partition_size()
        kt.append((k0, ks))
        k0 += ks
    assert k0 == K, (name, k0, K)
    G = min(col_group, M)
    nkt = len(kt)
    m_global = 0
    for g0 in range(0, M, G):
        gw = min(G, M - g0)
        nm = _ceil_div(gw, 128)
        pss = []
        for mi in range(nm):
            msize = min(128, gw - mi * 128)
            pss.append(pp.tile([msize, N], f32, name=f"{name}_ps{g0}_{mi}",
                               tag="ps_main", bufs=psum_bufs,
                               padded_shape=[128, N]))
        for ki, (kk, ks) in enumerate(kt):
            w_sb = wp.tile([ks, gw], f32, name=f"{name}_w{ki}_{g0}",
                           tag=f"{name}_w", bufs=wbufs, padded_shape=[128, G])
            nc.sync.dma_start(out=w_sb, in_=w_dram.ap()[kk:kk + ks, g0:g0 + gw])
            for mi in range(nm):
                c0 = mi * 128
                msize = min(128, gw - c0)
                nc.tensor.matmul(pss[mi], w_sb[:, c0:c0 + msize],
                                 rhs_tiles[ki], start=(ki == 0),
                                 stop=(ki == nkt - 1), skip_group_check=True)
        for mi in range(nm):
            evac(m_global + mi, min(128, gw - mi * 128), pss[mi])
        m_global += nm


def _finish(nc):
    return None

def build_program(phase_limit=99):
    import os
    phase_limit = int(os.environ.get("KPHASE", phase_limit))
    nc = bacc.Bacc("TRN2", target_bir_lowering=False, debug=False,
                   num_devices=NCORES)

    dram = {}

    def din(name, shape):
        dram[name] = nc.dram_tensor(name, list(shape), f32,
                                    kind="ExternalInput")
        return dram[name]

    # ---- inputs ----
    din("encT", (ENC, NA))
    din("spkrT", (SPK, N))
    din("speedT", (1, N))
    din("pinT", (OUT + SPK, N))
    din("maskneg", (1, NA))
    din("convT", (ATT, AW))
    din("enc_wT", (ENC, ATT)); din("enc_b", (ATT, 1))
    din("spkr_wT", (SPK, ATT))
    din("sattT", (1, ATT))
    din("apT", (ATT, 1)); din("apb", (1, 1))
    din("spd1T", (1, DEC)); din("spd1b", (DEC, 1))
    din("spd2T", (DEC, ENC)); din("spd2b", (ENC, 1))
    din("pre1T", (OUT + SPK, 2 * DEC)); din("pre1b", (2 * DEC, 1))
    din("pre2T", (2 * DEC, DEC)); din("pre2b", (DEC, 1))
    din("w0T", (ENC + DEC + SPK, GS)); din("b0", (GS, 1))
    din("w1T", (HS, G3)); din("b1", (G3, 1))
    din("ol1T", (H + ENC + DEC + SPK, DEC)); din("ol1b", (DEC, 1))
    for l in range(2):
        din(f"wvT{l}", (DEC, DEC)); din(f"bv{l}", (DEC, 1))
        din(f"woT{l}", (DEC, DEC)); din(f"bo{l}", (DEC, 1))
        din(f"ln1g{l}", (DEC, 1)); din(f"ln1b{l}", (DEC, 1))
        din(f"wf1T{l}", (DEC, FF)); din(f"bf1{l}", (FF, 1))
        din(f"wf2T{l}", (FF, DEC)); din(f"bf2{l}", (DEC, 1))
        din(f"ln2g{l}", (DEC, 1)); din(f"ln2b{l}", (DEC, 1))
    din("ol2T", (DEC, 2 * OUT)); din("ol2b", (256, 1))
    din("ident", (128, 128))
    out_d = nc.dram_tensor("out", [N, 2 * OUT], f32, kind="ExternalOutput")
    ctx_d = nc.dram_tensor("ctx", [N, ENC], f32, kind="ExternalOutput")

    with tile.TileContext(nc) as tc:
        with tc.tile_pool(name="wp", bufs=1) as wp, \
             tc.tile_pool(name="tp", bufs=2) as tp, \
             tc.tile_pool(name="pp", bufs=1, space="PSUM") as pp, \
             tc.tile_pool(name="dr", bufs=1, space="DRAM") as dr:

            def sb_load(name, shape, tag=None, bufs=1):
                t = wp.tile(list(shape), f32, name=f"{name}_sb",
                            tag=tag or f"{name}_t", bufs=bufs)
                nc.sync.dma_start(out=t, in_=dram[name].ap())
                return t

            def load_bias(name, M):
                mt = _ceil_div(M, 128)
                assert M % 128 == 0
                t = wp.tile([128, mt], f32, name=f"{name}_sb",
                            tag=f"{name}_b", bufs=1)
                nc.sync.dma_start(
                    out=t,
                    in_=dram[name].ap().rearrange("(m p) o -> p (m o)", p=128))
                return t

            def softsign_from_psum(name, ps, bias_ap, w):
                val = tp.tile([128, w], f32, name=f"{name}_v",
                              tag=f"ss_v{w}", bufs=2)
                den = tp.tile([128, w], f32, name=f"{name}_d",
                              tag=f"ss_d{w}", bufs=2)
                if bias_ap is None:
                    nc.scalar.activation(val, ps, AF.Identity)
                    nc.scalar.activation(den, ps, AF.Abs)
                else:
                    nc.scalar.activation(val, ps, AF.Identity, bias=bias_ap)
                    nc.scalar.activation(den, ps, AF.Abs, bias=bias_ap)
                nc.vector.tensor_scalar_add(den, den, 1.0)
                nc.vector.reciprocal(den, den)
                nc.vector.tensor_mul(val, val, den)
                return val

            # ---- persistent SBUF loads ----
            encT_sb = []
            for k in range(4):
                t = wp.tile([128, NA], f32, name=f"encT{k}_sb", tag="encT",
                            bufs=4)
                nc.sync.dma_start(out=t,
                                  in_=dram["encT"].ap()[k * 128:(k + 1) * 128, :])
                encT_sb.append(t)
            spkrT_sb = sb_load("spkrT", (SPK, N))
            speedT_sb = sb_load("speedT", (1, N))
            pinT_sb0 = wp.tile([128, N], f32, name="pinT0", bufs=1)
            nc.sync.dma_start(out=pinT_sb0, in_=dram["pinT"].ap()[0:128, :])
            pinT_sb1 = wp.tile([16, N], f32, name="pinT1", bufs=1)
            nc.sync.dma_start(out=pinT_sb1, in_=dram["pinT"].ap()[128:144, :])
            maskneg_sb = sb_load("maskneg", (1, NA))
            conv_sb = []
            for k in range(2):
                t = wp.tile([128, AW], f32, name=f"conv{k}_sb", tag="convT",
                            bufs=2)
                nc.sync.dma_start(out=t,
                                  in_=dram["convT"].ap()[k * 128:(k + 1) * 128, :])
                conv_sb.append(t)
            ident_sb = sb_load("ident", (128, 128))
            apb_sb = sb_load("apb", (1, 1))

            enc_b_sb = load_bias("enc_b", ATT)
            ap_sb = load_bias("apT", ATT)
            spd1b_sb = load_bias("spd1b", DEC)
            spd2b_sb = load_bias("spd2b", ENC)
            pre1b_sb = load_bias("pre1b", 2 * DEC)
            pre2b_sb = load_bias("pre2b", DEC)
            b0_sb = load_bias("b0", GS)
            b1_sb = load_bias("b1", G3)
            ol1b_sb = load_bias("ol1b", DEC)
            lb = {}
            for l in range(2):
                for nm in (f"bv{l}", f"bo{l}", f"ln1g{l}", f"ln1b{l}",
                           f"bf1{l}", f"bf2{l}", f"ln2g{l}", f"ln2b{l}"):
                    M = FF if nm.startswith("bf1") else DEC
                    lb[nm] = load_bias(nm, M)
            ol2b_sb = load_bias("ol2b", 256)

            ones_col = wp.tile([128, 1], f32, name="ones_col", bufs=1)
            nc.vector.memset(ones_col, 1.0)
            ones_row = wp.tile([1, 128], f32, name="ones_row", bufs=1)
            nc.vector.memset(ones_row, 1.0)
            eps_sb = wp.tile([1, 1], f32, name="eps_sb", bufs=1)
            nc.vector.memset(eps_sb, 1e-5)

            # =========== attention ===========
            spkr_w_sb = sb_load("spkr_wT", (SPK, ATT))
            satt_sb = sb_load("sattT", (1, ATT))
            c1 = []
            for m in range(2):
                ps = pp.tile([128, N], f32, name=f"spk_ps{m}", tag="ps_main",
                             bufs=4, padded_shape=[128, N])
                nc.tensor.matmul(ps, spkr_w_sb[:, m * 128:(m + 1) * 128],
                                 spkrT_sb, start=True, stop=True)
                s_spkr = softsign_from_psum(f"sspkr{m}", ps, None, N)
                ps2 = pp.tile([128, N], f32, name=f"sat_ps{m}", tag="ps_main",
                              bufs=4, padded_shape=[128, N])
                nc.tensor.matmul(ps2, satt_sb[:, m * 128:(m + 1) * 128],
                                 speedT_sb, start=True, stop=True)
                cm = tp.tile([128, N], f32, name=f"c1_{m}", tag="c1", bufs=2)
                nc.vector.tensor_add(cm, s_spkr, ps2)
                c1.append(cm)

            enc_w_sb = []
            for k in range(4):
                t = wp.tile([128, ATT], f32, name=f"encw{k}", tag="enc_wT",
                            bufs=4)
                nc.sync.dma_start(out=t,
                                  in_=dram["enc_wT"].ap()[k * 128:(k + 1) * 128, :])
                enc_w_sb.append(t)
            th = []
            for m in range(2):
                ps = pp.tile([128, NA], f32, name=f"eatt_ps{m}", tag="ps_att",
                             bufs=1, padded_shape=[128, NA])
                for k in range(4):
                    nc.tensor.matmul(ps, enc_w_sb[k][:, m * 128:(m + 1) * 128],
                                     encT_sb[k], start=(k == 0), stop=(k == 3))
                e = softsign_from_psum(f"senc{m}", ps, enc_b_sb[:, m:m + 1], NA)
                ev = e.rearrange("p (n t) -> p n t", t=AW)
                nc.vector.tensor_add(
                    ev, ev, c1[m].unsqueeze(2).broadcast_to([128, N, AW]))
                nc.vector.tensor_add(
                    ev, ev, conv_sb[m].unsqueeze(1).broadcast_to([128, N, AW]))
                tm = tp.tile([128, NA], f32, name=f"th{m}", tag="th", bufs=2)
                nc.scalar.activation(tm, e, AF.Tanh)
                th.append(tm)

            ps_l = pp.tile([1, NA], f32, name="log_ps", tag="ps_small", bufs=2,
                           padded_shape=[1, NA])
            for m in range(2):
                nc.tensor.matmul(ps_l, ap_sb[:, m:m + 1], th[m],
                                 start=(m == 0), stop=(m == 1))
            logit = tp.tile([1, NA], f32, name="logit", bufs=1)
            nc.scalar.activation(logit, ps_l, AF.Identity, bias=apb_sb)
            nc.vector.tensor_add(logit, logit, maskneg_sb)
            lv = logit.rearrange("p (n t) -> p n t", t=AW)
            mx = tp.tile([1, N], f32, name="mx", tag="sm32", bufs=10)
            nc.vector.tensor_reduce(mx, lv, axis=AX.X, op=ALU.max)
            nc.vector.tensor_sub(lv, lv,
                                 mx.unsqueeze(2).broadcast_to([1, N, AW]))
            wexp = tp.tile([1, NA], f32, name="wexp", bufs=1)
            nc.scalar.activation(wexp, logit, AF.Exp)
            wv = wexp.rearrange("p (n t) -> p n t", t=AW)
            ssum = tp.tile([1, N], f32, name="ssum", tag="sm32", bufs=10)
            nc.vector.tensor_reduce(ssum, wv, axis=AX.X, op=ALU.add)
            nc.vector.tensor_scalar_max(ssum, ssum, 1e-12)
            rsum = tp.tile([1, N], f32, name="rsum", tag="sm32", bufs=10)
            nc.vector.reciprocal(rsum, ssum)
            nc.vector.tensor_mul(wv, wv,
                                 rsum.unsqueeze(2).broadcast_to([1, N, AW]))

            wb_ps = pp.tile([128, NA], f32, name="wb_ps", tag="ps_att", bufs=1,
                            padded_shape=[128, NA])
            nc.tensor.matmul(wb_ps, ones_row, wexp, start=True, stop=True)
            sb_sum = tp.tile([128, N], f32, name="sb_sum", bufs=1)
            nc.vector.tensor_reduce(sb_sum,
                                    wb_ps.rearrange("p (n t) -> p n t", t=AW),
                                    axis=AX.X, op=ALU.add)

            # ---- speed projection ----
            spd1_sb = sb_load("spd1T", (1, DEC))
            r1T = []
            for m in range(4):
                ps = pp.tile([128, N], f32, name=f"sp1_ps{m}", tag="ps_main",
                             bufs=4, padded_shape=[128, N])
                nc.tensor.matmul(ps, spd1_sb[:, m * 128:(m + 1) * 128],
                                 speedT_sb, start=True, stop=True)
                t = tp.tile([128, N], f32, name=f"r1T{m}", tag="r1T", bufs=4)
                nc.scalar.activation(t, ps, AF.Relu, bias=spd1b_sb[:, m:m + 1])
                r1T.append(t)
            spT = [None] * 4

            def evac_sp(m, msize, ps):
                t = tp.tile([128, N], f32, name=f"spT{m}", tag="spT", bufs=4)
                nc.scalar.activation(t, ps, AF.Tanh, bias=spd2b_sb[:, m:m + 1])
                spT[m] = t

            gemm(nc, wp, pp, "spd2", dram["spd2T"], DEC, ENC, r1T, evac_sp)

            # ---- context ----
            ctxT = []
            for k in range(4):
                prod = tp.tile([128, NA], f32, name=f"cprod{k}", tag="cprod",
                               bufs=2)
                nc.vector.tensor_mul(prod, encT_sb[k], wb_ps)
                a = tp.tile([128, N], f32, name=f"ctxT{k}", tag="ctxT", bufs=4)
                nc.vector.tensor_reduce(
                    a, prod.rearrange("p (n t) -> p n t", t=AW),
                    axis=AX.X, op=ALU.add)
                bt = tp.tile([128, N], f32, name=f"cb{k}", tag="cb", bufs=2)
                nc.vector.tensor_mul(bt, spT[k], sb_sum)
                nc.vector.tensor_add(a, a, bt)
                ctxT.append(a)

            if phase_limit < 2:
                return _finish(nc)
            # ---- prenet ----
            p1T = [None] * 8

            def evac_p1(m, msize, ps):
                t = tp.tile([128, N], f32, name=f"p1T{m}", tag="p1T", bufs=8)
                nc.scalar.activation(t, ps, AF.Relu, bias=pre1b_sb[:, m:m + 1])
                p1T[m] = t

            gemm(nc, wp, pp, "pre1", dram["pre1T"], OUT + SPK, 2 * DEC,
                 [pinT_sb0, pinT_sb1], evac_p1)

            preT = [None] * 4

            def evac_p2(m, msize, ps):
                t = tp.tile([128, N], f32, name=f"preT{m}", tag="preT", bufs=4)
                nc.scalar.activation(t, ps, AF.Relu, bias=pre2b_sb[:, m:m + 1])
                preT[m] = t

            gemm(nc, wp, pp, "pre2", dram["pre2T"], 2 * DEC, DEC, p1T, evac_p2)

            in_lstm_tiles = preT + ctxT + [spkrT_sb]

            if phase_limit < 3:
                return _finish(nc)
            # ---- LSTM layer 0 ----
            l0 = [None] * 6

            def evac_l0(m, msize, ps):
                func = AF.Sigmoid if (m < 2 or m >= 4) else AF.Tanh
                t = tp.tile([128, N], f32, name=f"l0_{m}", tag="l0", bufs=6)
                nc.scalar.activation(t, ps, func, bias=b0_sb[:, m:m + 1])
                l0[m] = t

            gemm(nc, wp, pp, "lstm0", dram["w0T"], ENC + DEC + SPK, GS,
                 in_lstm_tiles, evac_l0, wbufs=4)

            h1T = []
            for k in range(2):
                c = tp.tile([128, N], f32, name=f"c_{k}", tag="cc", bufs=2)
                nc.vector.tensor_mul(c, l0[k], l0[2 + k])
                tc_ = tp.tile([128, N], f32, name=f"tc_{k}", tag="cc", bufs=2)
                nc.scalar.activation(tc_, c, AF.Tanh)
                h = tp.tile([128, N], f32, name=f"h1T{k}", tag="h1T", bufs=2)
                nc.vector.tensor_mul(h, l0[4 + k], tc_)
                h1T.append(h)

            if phase_limit < 4:
                return _finish(nc)
            # ---- LSTM layer 1 partial gates ----
            g1_sb = wp.tile([128, 48 * N], f32, name="g1_sb", bufs=1)

            def evac_l1(m, msize, ps):
                nc.scalar.activation(g1_sb[:, m * N:(m + 1) * N], ps,
                                     AF.Identity, bias=b1_sb[:, m:m + 1])

            gemm(nc, wp, pp, "lstm1", dram["w1T"], HS, G3, h1T, evac_l1,
                 wbufs=4)

            if phase_limit < 5:
                return _finish(nc)
            # ---- AllReduce partial gates ----
            g1part = dr.tile([G3, N], f32, name="g1part")
            g1full = dr.tile([G3, N], f32, name="g1full", addr_space="Shared")
            nc.sync.dma_start(
                out=g1part.rearrange("(m p) n -> p m n", p=128),
                in_=g1_sb.rearrange("p (m n) -> p m n", n=N))
            nc.gpsimd.collective_compute(
                "AllReduce", ALU.add,
                replica_groups=[list(range(NCORES))],
                ins=[g1part], outs=[g1full])
            g1f_sb = wp.tile([128, 48 * N], f32, name="g1f_sb", bufs=1)
            nc.sync.dma_start(
                out=g1f_sb.rearrange("p (m n) -> p m n", n=N),
                in_=g1full.rearrange("(m p) n -> p m n", p=128))

            # ---- h2 ----
            W = 16 * N  # 512
            sig_i = tp.tile([128, W], f32, name="sig_i", tag="hw", bufs=3)
            nc.scalar.activation(sig_i, g1f_sb[:, 0:W], AF.Sigmoid)
            tan_g = tp.tile([128, W], f32, name="tan_g", tag="hw", bufs=3)
            nc.scalar.activation(tan_g, g1f_sb[:, W:2 * W], AF.Tanh)
            nc.vector.tensor_mul(sig_i, sig_i, tan_g)           # c
            nc.scalar.activation(tan_g, sig_i, AF.Tanh)         # tanh(c)
            sig_o = tp.tile([128, W], f32, name="sig_o", tag="hw", bufs=3)
            nc.scalar.activation(sig_o, g1f_sb[:, 2 * W:3 * W], AF.Sigmoid)
            h2_sb = wp.tile([128, W], f32, name="h2_sb", bufs=1)
            nc.vector.tensor_mul(h2_sb, sig_o, tan_g)

            if phase_limit < 6:
                return _finish(nc)
            # ---- outl1 (in_lstm k-tiles first to overlap the AllReduce) ----
            ol1_rhs = in_lstm_tiles + [h2_sb[:, t * N:(t + 1) * N]
                                       for t in range(16)]
            xT = [None] * 4

            def evac_ol1(m, msize, ps):
                t = tp.tile([128, N], f32, name=f"xT{m}", tag="xT", bufs=12)
                nc.scalar.activation(t, ps, AF.Identity,
                                     bias=ol1b_sb[:, m:m + 1])
                xT[m] = t

            gemm(nc, wp, pp, "outl1", dram["ol1T"], H + ENC + DEC + SPK, DEC,
                 ol1_rhs, evac_ol1, wbufs=4)

            if phase_limit < 7:
                return _finish(nc)
            # ---- transformer ----
            def layer_norm(x_tiles, g_sb, b_sb, nm):
                s_ps = pp.tile([1, N], f32, name=f"{nm}_s", tag="ps_small",
                               bufs=2, padded_shape=[1, NA])
                for k in range(4):
                    nc.tensor.matmul(s_ps, ones_col, x_tiles[k],
                                     start=(k == 0), stop=(k == 3))
                s2_ps = pp.tile([1, N], f32, name=f"{nm}_s2", tag="ps_small",
                                bufs=2, padded_shape=[1, NA])
                for k in range(4):
                    sq = tp.tile([128, N], f32, name=f"{nm}_sq{k}", tag="sq",
                                 bufs=2)
                    nc.scalar.activation(sq, x_tiles[k], AF.Square)
                    nc.tensor.matmul(s2_ps, ones_col, sq, start=(k == 0),
                                     stop=(k == 3))
                mu = tp.tile([1, N], f32, name=f"{nm}_mu", tag="sm32", bufs=10)
                nc.scalar.activation(mu, s_ps, AF.Identity, scale=1.0 / DEC)
                ms = tp.tile([1, N], f32, name=f"{nm}_ms", tag="sm32", bufs=10)
                nc.scalar.activation(ms, s2_ps, AF.Identity, scale=1.0 / DEC)
                mu2 = tp.tile([1, N], f32, name=f"{nm}_mu2", tag="sm32",
                              bufs=10)
                nc.scalar.activation(mu2, mu, AF.Square)
                var = tp.tile([1, N], f32, name=f"{nm}_var", tag="sm32",
                              bufs=10)
                nc.vector.tensor_sub(var, ms, mu2)
                sd = tp.tile([1, N], f32, name=f"{nm}_sd", tag="sm32", bufs=10)
                nc.scalar.activation(sd, var, AF.Sqrt, bias=eps_sb)
                rstd = tp.tile([1, N], f32, name=f"{nm}_rstd", tag="sm32",
                               bufs=10)
                nc.vector.reciprocal(rstd, sd)
                mub_ps = pp.tile([128, N], f32, name=f"{nm}_mub",
                                 tag="ps_main", bufs=4, padded_shape=[128, N])
                nc.tensor.matmul(mub_ps, ones_row, mu, start=True, stop=True)
                rb_ps = pp.tile([128, N], f32, name=f"{nm}_rb", tag="ps_main",
                                bufs=4, padded_shape=[128, N])
                nc.tensor.matmul(rb_ps, ones_row, rstd, start=True, stop=True)
                out = []
                for k in range(4):
                    xc = tp.tile([128, N], f32, name=f"{nm}_xc{k}", tag="sq",
                                 bufs=2)
                    nc.vector.tensor_sub(xc, x_tiles[k], mub_ps)
                    nc.vector.tensor_mul(xc, xc, rb_ps)
                    o = tp.tile([128, N], f32, name=f"{nm}_o{k}", tag="xT",
                                bufs=12)
                    nc.scalar.activation(o, xc, AF.Identity,
                                         bias=b_sb[:, k:k + 1],
                                         scale=g_sb[:, k:k + 1])
                    out.append(o)
                return out

            for l in range(2):
                vT = [None] * 4

                def evac_v(m, msize, ps, l=l):
                    t = tp.tile([128, N], f32, name=f"vT{l}_{m}", tag="vT",
                                bufs=12)
                    nc.scalar.activation(t, ps, AF.Identity,
                                         bias=lb[f"bv{l}"][:, m:m + 1])
                    vT[m] = t

                gemm(nc, wp, pp, f"v{l}", dram[f"wvT{l}"], DEC, DEC, xT,
                     evac_v)

                yT = [None] * 4
                x_res = xT

                def evac_o(m, msize, ps, l=l, x_res=x_res):
                    t = tp.tile([128, N], f32, name=f"aT{l}_{m}", tag="vT",
                                bufs=12)
                    nc.scalar.activation(t, ps, AF.Identity,
                                         bias=lb[f"bo{l}"][:, m:m + 1])
                    nc.vector.tensor_add(t, t, x_res[m])
                    yT[m] = t

                gemm(nc, wp, pp, f"o{l}", dram[f"woT{l}"], DEC, DEC, vT,
                     evac_o)

                xT = layer_norm(yT, lb[f"ln1g{l}"], lb[f"ln1b{l}"], f"ln1_{l}")

                fT = [None] * 8

                def evac_f1(m, msize, ps, l=l):
                    t = tp.tile([128, N], f32, name=f"fT{l}_{m}", tag="fT",
                                bufs=8)
                    nc.scalar.activation(t, ps, AF.Relu,
                                         bias=lb[f"bf1{l}"][:, m:m + 1])
                    fT[m] = t

                gemm(nc, wp, pp, f"f1{l}", dram[f"wf1T{l}"], DEC, FF, xT,
                     evac_f1)

                zT = [None] * 4
                x_res2 = xT

                def evac_f2(m, msize, ps, l=l, x_res2=x_res2):
                    t = tp.tile([128, N], f32, name=f"zT{l}_{m}", tag="vT",
                                bufs=12)
                    nc.scalar.activation(t, ps, AF.Identity,
                                         bias=lb[f"bf2{l}"][:, m:m + 1])
                    nc.vector.tensor_add(t, t, x_res2[m])
                    zT[m] = t

                gemm(nc, wp, pp, f"f2{l}", dram[f"wf2T{l}"], FF, DEC, fT,
                     evac_f2)

                xT = layer_norm(zT, lb[f"ln2g{l}"], lb[f"ln2b{l}"], f"ln2_{l}")

            if phase_limit < 8:
                return _finish(nc)
            # ---- outl2 ----
            oT = [None] * 2

            def evac_o2(m, msize, ps):
                t = tp.tile([msize, N], f32, name=f"oT{m}", tag="oT", bufs=2,
                            padded_shape=[128, N])
                nc.scalar.activation(t, ps, AF.Identity,
                                     bias=ol2b_sb[:msize, m:m + 1])
                oT[m] = t

            gemm(nc, wp, pp, "outl2", dram["ol2T"], DEC, 2 * OUT, xT, evac_o2)

            if phase_limit < 9:
                return _finish(nc)
            # ---- transpose outputs to batch-major, DMA out ----
            out_sb = wp.tile([N, 2 * OUT], f32, name="out_sb", bufs=1)
            tps0 = pp.tile([N, 128], f32, name="tps0", tag="ps_tr", bufs=1,
                           padded_shape=[N, 128])
            nc.tensor.transpose(tps0, oT[0], ident_sb)
            nc.vector.tensor_copy(out_sb[:, 0:128], tps0)
            tps1 = pp.tile([N, 32], f32, name="tps1", tag="ps_tr", bufs=1,
                           padded_shape=[N, 128])
            nc.tensor.transpose(tps1, oT[1], ident_sb[:32, :32])
            nc.vector.tensor_copy(out_sb[:, 128:160], tps1)
            nc.sync.dma_start(out=out_d.ap(), in_=out_sb)

            ctx_sb = wp.tile([N, ENC], f32, name="ctx_sb", bufs=1)
            for k in range(4):
                tpk = pp.tile([N, 128], f32, name=f"tpc{k}", tag="ps_tr",
                              bufs=1, padded_shape=[N, 128])
                nc.tensor.transpose(tpk, ctxT[k], ident_sb)
                nc.vector.tensor_copy(ctx_sb[:, k * 128:(k + 1) * 128], tpk)
            nc.sync.dma_start(out=ctx_d.ap(), in_=ctx_sb)

    nc.compile()
    return nc


def prep_inputs(inputs):
    fz = np.float32

    def g(name):
        return np.asarray(inputs[name], fz)

    ie = g("input_enc")
    spkr = g("spkr_vec")[:, 0, :]
    encT = np.ascontiguousarray(
        ie[:, :AW, :].transpose(2, 0, 1).reshape(ENC, NA))
    spkrT = np.ascontiguousarray(spkr.T)
    speedT = np.ascontiguousarray(g("speed").reshape(1, N))
    pinT = np.ascontiguousarray(
        np.concatenate([g("input_dec"), spkr], axis=1).T)
    lens = np.asarray(inputs["lengths_enc"]).astype(np.int64)
    t = np.arange(AW)
    mask = (t[None, :] <= np.minimum(AW - 1, lens[:, None] - 1)).astype(fz)
    maskneg = np.ascontiguousarray(((mask - 1.0) * 1e4).reshape(1, NA))
    convT = np.ascontiguousarray(g("conv_w")[:, 0, 15 - t])

    ol1T_full = g("outl1_w").T  # (3136, 512); rows: [h2 (2048), in_lstm (1088)]
    ol1T = np.ascontiguousarray(
        np.concatenate([ol1T_full[H:], ol1T_full[:H]], axis=0))

    base = {
        "encT": encT, "spkrT": spkrT, "speedT": speedT, "pinT": pinT,
        "maskneg": maskneg, "convT": convT,
        "enc_wT": np.ascontiguousarray(g("enc_w").T),
        "enc_b": np.ascontiguousarray(g("enc_b").reshape(ATT, 1)),
        "spkr_wT": np.ascontiguousarray(g("spkr_w").T),
        "sattT": np.ascontiguousarray(g("speed_att_w").T),
        "apT": np.ascontiguousarray(g("attproj_w").T),
        "apb": np.ascontiguousarray(g("attproj_b").reshape(1, 1)),
        "spd1T": np.ascontiguousarray(g("spd1_w").T),
        "spd1b": np.ascontiguousarray(g("spd1_b").reshape(DEC, 1)),
        "spd2T": np.ascontiguousarray(g("spd2_w").T),
        "spd2b": np.ascontiguousarray(g("spd2_b").reshape(ENC, 1)),
        "pre1T": np.ascontiguousarray(g("pre1_w").T),
        "pre1b": np.ascontiguousarray(g("pre1_b").reshape(2 * DEC, 1)),
        "pre2T": np.ascontiguousarray(g("pre2_w").T),
        "pre2b": np.ascontiguousarray(g("pre2_b").reshape(DEC, 1)),
        "ol1T": ol1T,
        "ol1b": np.ascontiguousarray(g("outl1_b").reshape(DEC, 1)),
        "ol2T": np.ascontiguousarray(g("outl2_w").T),
        "ol2b": np.ascontiguousarray(
            np.pad(g("outl2_b"), (0, 96)).reshape(256, 1)),
        "ident": np.eye(128, dtype=fz),
    }
    for l in range(2):
        base[f"wvT{l}"] = np.ascontiguousarray(
            g("tr_inproj_w")[l][1024:1536].T)
        base[f"bv{l}"] = np.ascontiguousarray(
            g("tr_inproj_b")[l][1024:1536].reshape(DEC, 1))
        base[f"woT{l}"] = np.ascontiguousarray(g("tr_out_w")[l].T)
        base[f"bo{l}"] = np.ascontiguousarray(
            g("tr_out_b")[l].reshape(DEC, 1))
        base[f"ln1g{l}"] = np.ascontiguousarray(
            g("tr_ln1_g")[l].reshape(DEC, 1))
        base[f"ln1b{l}"] = np.ascontiguousarray(
            g("tr_ln1_b")[l].reshape(DEC, 1))
        base[f"wf1T{l}"] = np.ascontiguousarray(g("tr_ff1_w")[l].T)
        base[f"bf1{l}"] = np.ascontiguousarray(
            g("tr_ff1_b")[l].reshape(FF, 1))
        base[f"wf2T{l}"] = np.ascontiguousarray(g("tr_ff2_w")[l].T)
        base[f"bf2{l}"] = np.ascontiguousarray(
            g("tr_ff2_b")[l].reshape(DEC, 1))
        base[f"ln2g{l}"] = np.ascontiguousarray(
            g("tr_ln2_g")[l].reshape(DEC, 1))
        base[f"ln2b{l}"] = np.ascontiguousarray(
            g("tr_ln2_b")[l].reshape(DEC, 1))

    wih0 = g("lstm_wih0")
    b0full = g("lstm_bih0") + g("lstm_bhh0")
    wih1 = g("lstm_wih1")
    b1full = g("lstm_bih1") + g("lstm_bhh1")
    rows_igo = np.concatenate(
        [np.arange(H), 2 * H + np.arange(H), 3 * H + np.arange(H)])
    w1_igo = wih1[rows_igo]
    in_maps = []
    for k in range(NCORES):
        hs = np.arange(k * HS, (k + 1) * HS)
        rows0 = np.concatenate([hs, 2 * H + hs, 3 * H + hs])
        m = dict(base)
        m["w0T"] = np.ascontiguousarray(wih0[rows0].T)
        m["b0"] = np.ascontiguousarray(b0full[rows0].reshape(GS, 1))
        m["w1T"] = np.ascontiguousarray(w1_igo[:, k * HS:(k + 1) * HS].T)
        if k == 0:
            m["b1"] = np.ascontiguousarray(b1full[rows_igo].reshape(G3, 1))
        else:
            m["b1"] = np.zeros((G3, 1), fz)
        in_maps.append(m)
    return in_maps


_NC = None


def _get_nc():
    global _NC
    if _NC is None:
        _NC = build_program()
    return _NC


_LAST_RESULTS = None


def kernel(**inputs):
    global _LAST_RESULTS
    import os
    nc = _get_nc()
    in_maps = prep_inputs(inputs)
    kw = {}
    if os.environ.get("KERNEL_TRACE"):
        kw["trace"] = True
    res = run_bass_kernel_spmd(nc, in_maps, core_ids=list(range(NCORES)), **kw)
    _LAST_RESULTS = res
    r0 = res.results[0]
    out = np.asarray(r0["out"], np.float32).reshape(N, 2, OUT)
    ctx = np.asarray(r0["ctx"], np.float32).reshape(N, 1, ENC)
    return out, ctx


# revision 13
# speedup vs baseline: 1.0494x; 1.0494x over previous
"""Trainium2 Bass kernel for nn_AttnDecoderRNN3 (sparse_attention).

Strategy (8 NeuronCores):
- Only input_enc[:, :10, :] matters: the attention mask is a fixed 10-wide
  window at t=0 (aw0 is a one-hot at t=0 -> argmax 0) and the softmax
  max-subtraction cancels exactly, so the full-T encoder GEMM is skipped.
- LSTM h0=c0=0 -> whh*/f-gate weights unused. Transformer S=1 -> att == V.
- LSTM layer 0 sharded over the hidden dim (each core computes a 256-wide
  h1 slice for all 32 batch rows); layer 1 sharded over the contraction
  dim (each core's h1 slice x its wih1 column slice -> partial full
  gates), combined with ONE AllReduce; biases added after the AllReduce.
- Everything else is computed redundantly on all cores for all 32 batch
  rows (it is tiny); final outputs are read from core 0.
- Big GEMMs: weights-MOVING through the PE at fp32r (1 col/cycle), with
  the batch (32) as the stationary operand; outputs [32, chunk] are
  PE-transposed back to feature-on-partition with bias+activation fused
  into the ScalarE evacuation. Small GEMMs stay exact fp32 stationary.
"""

import numpy as np

import concourse.bacc as bacc
import concourse.mybir as mybir
import concourse.tile as tile
from concourse.bass_utils import run_bass_kernel_spmd

NCORES = 8
N = 32            # batch
ENC = 512
ATT = 256
DEC = 512
H = 2048          # lstm hidden
SPK = 64
OUT = 80
FF = 1024
AW = 10           # attention window (ATT_RANGE)
HS = H // NCORES  # hidden slice per core = 256
G3 = 3 * H        # i,g,o gates = 6144
GS = 3 * HS       # per-core layer0 gate rows = 768
NA = N * AW       # 320
CG = 256          # gemm2 column-group (chunk) width

f32 = mybir.dt.float32
f32r = mybir.dt.float32r
AF = mybir.ActivationFunctionType
ALU = mybir.AluOpType
AX = mybir.AxisListType


def _ceil_div(a, b):
    return (a + b - 1) // b


def _kt_of(tiles):
    kt = []
    k0 = 0
    for r in tiles:
        ks = r.partition_size()
        kt.append((k0, ks))
        k0 += ks
    return kt, k0


def build_program(phase_limit=99):
    import os
    phase_limit = int(os.environ.get("KPHASE", phase_limit))
    nc = bacc.Bacc("TRN2", target_bir_lowering=False, debug=False,
                   num_devices=NCORES)

    dram = {}

    def din(name, shape, dt=f32):
        dram[name] = nc.dram_tensor(name, list(shape), dt,
                                    kind="ExternalInput")
        return dram[name]

    # ---- inputs ----
    # v2 (tiled, fp32r) weight tensors have shape (M/CG * K, CG)
    def din2(name, K, M):
        din(name, (M // CG * K, CG), f32r)

    din("encT", (ENC, NA))
    din("spkrT", (SPK, N))
    din("speedT", (1, N))
    din("pinT", (OUT + SPK, N), f32r)
    din("maskneg", (1, NA))
    din("convT", (ATT, AW))
    din("enc_wT", (ENC, ATT)); din("enc_b", (ATT, 1))
    din("spkr_wT", (SPK, ATT))
    din("sattT", (1, ATT))
    din("apT", (ATT, 1)); din("apb", (1, 1))
    din("spd1T", (1, DEC)); din("spd1b", (DEC, 1))
    din2("spd2T", DEC, ENC); din("spd2b", (ENC, 1))
    din2("pre1T", OUT + SPK, 2 * DEC); din("pre1b", (2 * DEC, 1))
    din2("pre2T", 2 * DEC, DEC); din("pre2b", (DEC, 1))
    din2("w0T", ENC + DEC + SPK, GS); din("b0", (GS, 1))
    din2("w1T", HS, G3); din("b1", (G3, 1))
    din2("ol1T", H + ENC + DEC + SPK, DEC); din("ol1b", (DEC, 1))
    for l in range(2):
        din2(f"wvT{l}", DEC, DEC); din(f"bv{l}", (DEC, 1))
        din2(f"woT{l}", DEC, DEC); din(f"bo{l}", (DEC, 1))
        din(f"ln1g{l}", (DEC, 1)); din(f"ln1b{l}", (DEC, 1))
        din2(f"wf1T{l}", DEC, FF); din(f"bf1{l}", (FF, 1))
        din2(f"wf2T{l}", FF, DEC); din(f"bf2{l}", (DEC, 1))
        din(f"ln2g{l}", (DEC, 1)); din(f"ln2b{l}", (DEC, 1))
    din("ol2T", (DEC + 1, 2 * OUT), f32r)   # bias folded in as last row
    din("ident", (128, 128))
    out_d = nc.dram_tensor("out", [N, 2 * OUT], f32, kind="ExternalOutput")
    ctx_d = nc.dram_tensor("ctx", [N, ENC], f32, kind="ExternalOutput")

    with tile.TileContext(nc) as tc:
        with tc.tile_pool(name="wp", bufs=1) as wp, \
             tc.tile_pool(name="tp", bufs=2) as tp, \
             tc.tile_pool(name="pp", bufs=1, space="PSUM") as pp, \
             tc.tile_pool(name="dr", bufs=1, space="DRAM") as dr:

            def sb_load(name, shape, tag=None, bufs=1, dt=f32):
                t = wp.tile(list(shape), dt, name=f"{name}_sb",
                            tag=tag or f"{name}_t", bufs=bufs)
                nc.sync.dma_start(out=t, in_=dram[name].ap())
                return t

            def load_bias(name, M):
                mt = _ceil_div(M, 128)
                assert M % 128 == 0
                t = wp.tile([128, mt], f32, name=f"{name}_sb",
                            tag=f"{name}_b", bufs=1)
                nc.sync.dma_start(
                    out=t,
                    in_=dram[name].ap().rearrange("(m p) o -> p (m o)", p=128))
                return t

            def softsign_from_psum(name, ps, bias_ap, w):
                val = tp.tile([128, w], f32, name=f"{name}_v",
                              tag=f"ss_v{w}", bufs=2)
                den = tp.tile([128, w], f32, name=f"{name}_d",
                              tag=f"ss_d{w}", bufs=2)
                if bias_ap is None:
                    nc.scalar.activation(val, ps, AF.Identity)
                    nc.scalar.activation(den, ps, AF.Abs)
                else:
                    nc.scalar.activation(val, ps, AF.Identity, bias=bias_ap)
                    nc.scalar.activation(den, ps, AF.Abs, bias=bias_ap)
                nc.vector.tensor_scalar_add(den, den, 1.0)
                nc.vector.reciprocal(den, den)
                nc.vector.tensor_mul(val, val, den)
                return val

            # ---- persistent SBUF loads ----
            encT_sb = []
            for k in range(4):
                t = wp.tile([128, NA], f32, name=f"encT{k}_sb", tag="encT",
                            bufs=4)
                nc.sync.dma_start(out=t,
                                  in_=dram["encT"].ap()[k * 128:(k + 1) * 128, :])
                encT_sb.append(t)
            spkrT_sb = sb_load("spkrT", (SPK, N))
            speedT_sb = sb_load("speedT", (1, N))
            pinT_sb0 = wp.tile([128, N], f32r, name="pinT0", bufs=1)
            nc.sync.dma_start(out=pinT_sb0, in_=dram["pinT"].ap()[0:128, :])
            pinT_sb1 = wp.tile([16, N], f32r, name="pinT1", bufs=1)
            nc.sync.dma_start(out=pinT_sb1, in_=dram["pinT"].ap()[128:144, :])
            maskneg_sb = sb_load("maskneg", (1, NA))
            conv_sb = []
            for k in range(2):
                t = wp.tile([128, AW], f32, name=f"conv{k}_sb", tag="convT",
                            bufs=2)
                nc.sync.dma_start(out=t,
                                  in_=dram["convT"].ap()[k * 128:(k + 1) * 128, :])
                conv_sb.append(t)
            ident_sb = sb_load("ident", (128, 128))
            apb_sb = sb_load("apb", (1, 1))

            enc_b_sb = load_bias("enc_b", ATT)
            ap_sb = load_bias("apT", ATT)
            spd1b_sb = load_bias("spd1b", DEC)
            spd2b_sb = load_bias("spd2b", ENC)
            pre1b_sb = load_bias("pre1b", 2 * DEC)
            pre2b_sb = load_bias("pre2b", DEC)
            b0_sb = load_bias("b0", GS)
            b1_sb = load_bias("b1", G3)
            ol1b_sb = load_bias("ol1b", DEC)
            lb = {}
            for l in range(2):
                for nm in (f"bv{l}", f"bo{l}", f"ln1g{l}", f"ln1b{l}",
                           f"bf1{l}", f"bf2{l}", f"ln2g{l}", f"ln2b{l}"):
                    M = FF if nm.startswith("bf1") else DEC
                    lb[nm] = load_bias(nm, M)

            ones_col = wp.tile([128, 1], f32, name="ones_col", bufs=1)
            nc.vector.memset(ones_col, 1.0)
            ones_col_r = wp.tile([128, 1], f32r, name="ones_col_r", bufs=1)
            nc.vector.tensor_copy(ones_col_r, ones_col)
            ones_row = wp.tile([1, 128], f32, name="ones_row", bufs=1)
            nc.vector.memset(ones_row, 1.0)
            ones_row_r = wp.tile([1, N], f32r, name="ones_row_r", bufs=1)
            nc.vector.tensor_copy(ones_row_r, ones_row[:, :N])
            eps_sb = wp.tile([1, 1], f32, name="eps_sb", bufs=1)
            nc.vector.memset(eps_sb, 1e-5)

            # ================= gemm v2: weights-moving fp32r =================
            def gemm2(name, w_name, K, M, lhsT_tiles, evac_t, wbufs=4,
                      split_k=None):
                """y[32, M] = x @ W^T streamed in CG-col chunks; each 128-col
                output tile is PE-transposed back to [128, 32] and handed to
                evac_t(m, psum_t) for fused bias+activation evacuation.

                split_k: optional k-tile index; matmuls with ki < split_k for
                ALL chunks are emitted first (AllReduce overlap), then the
                rest."""
                w_dram = dram[w_name]
                kt, ksum = _kt_of(lhsT_tiles)
                assert ksum == K, (name, ksum, K)
                nkt = len(kt)
                nch = M // CG
                phases = [(0, split_k), (split_k, nkt)] if split_k else [(0, nkt)]
                pss = {}
                for c in range(nch):
                    pss[c] = pp.tile([N, CG], f32, name=f"{name}_ps{c}",
                                     tag="ps_mm", bufs=2,
                                     padded_shape=[N, CG])
                    if split_k:
                        continue
                    _gemm2_chunk(name, w_dram, K, kt, lhsT_tiles, c, 0, nkt,
                                 pss[c], wbufs)
                    _gemm2_evac(name, c, pss[c], evac_t)
                if split_k:
                    for k0, k1 in phases:
                        for c in range(nch):
                            _gemm2_chunk(name, w_dram, K, kt, lhsT_tiles, c,
                                         k0, k1, pss[c], wbufs)
                    for c in range(nch):
                        _gemm2_evac(name, c, pss[c], evac_t)

            def _gemm2_chunk(name, w_dram, K, kt, lhsT_tiles, c, k0, k1,
                             ps, wbufs):
                nkt = len(kt)
                for ki in range(k0, k1):
                    kk, ks = kt[ki]
                    w_sb = wp.tile([ks, CG], f32r, name=f"{name}_w{ki}_{c}",
                                   tag=f"{name}_w", bufs=wbufs,
                                   padded_shape=[128, CG])
                    nc.sync.dma_start(
                        out=w_sb, in_=w_dram.ap()[c * K + kk:c * K + kk + ks, :])
                    nc.tensor.matmul(ps, lhsT_tiles[ki], w_sb,
                                     start=(ki == 0), stop=(ki == nkt - 1),
                                     skip_group_check=True)

            def _gemm2_evac(name, c, ps, evac_t):
                y32 = tp.tile([N, CG], f32, name=f"{name}_y{c}", tag="y32",
                              bufs=3)
                nc.scalar.activation(y32, ps, AF.Identity)
                for j in range(CG // 128):
                    m = c * (CG // 128) + j
                    pst = pp.tile([128, N], f32, name=f"{name}_pt{m}",
                                  tag="ps_tr2", bufs=1, padded_shape=[128, 128])
                    nc.tensor.transpose(pst, y32[:, j * 128:(j + 1) * 128],
                                        ident_sb[:N, :N])
                    evac_t(m, pst)

            def act_evac(out_list, tag, bufs, func, bias_sb, dt=f32r):
                def _e(m, pst):
                    t = tp.tile([128, N], dt, name=f"{tag}_{m}", tag=tag,
                                bufs=bufs)
                    f = func(m) if callable(func) else func
                    nc.scalar.activation(t, pst, f, bias=bias_sb[:, m:m + 1])
                    out_list[m] = t
                return _e

            # =========== attention (exact fp32, weights-stationary) ==========
            spkr_w_sb = sb_load("spkr_wT", (SPK, ATT))
            satt_sb = sb_load("sattT", (1, ATT))
            c1 = []
            for m in range(2):
                ps = pp.tile([128, N], f32, name=f"spk_ps{m}", tag="ps_main",
                             bufs=2, padded_shape=[128, N])
                nc.tensor.matmul(ps, spkr_w_sb[:, m * 128:(m + 1) * 128],
                                 spkrT_sb, start=True, stop=True)
                s_spkr = softsign_from_psum(f"sspkr{m}", ps, None, N)
                ps2 = pp.tile([128, N], f32, name=f"sat_ps{m}", tag="ps_main",
                              bufs=2, padded_shape=[128, N])
                nc.tensor.matmul(ps2, satt_sb[:, m * 128:(m + 1) * 128],
                                 speedT_sb, start=True, stop=True)
                cm = tp.tile([128, N], f32, name=f"c1_{m}", tag="c1", bufs=2)
                nc.vector.tensor_add(cm, s_spkr, ps2)
                c1.append(cm)

            enc_w_sb = []
            for k in range(4):
                t = wp.tile([128, ATT], f32, name=f"encw{k}", tag="enc_wT",
                            bufs=4)
                nc.sync.dma_start(out=t,
                                  in_=dram["enc_wT"].ap()[k * 128:(k + 1) * 128, :])
                enc_w_sb.append(t)
            th = []
            for m in range(2):
                ps = pp.tile([128, NA], f32, name=f"eatt_ps{m}", tag="ps_misc",
                             bufs=3, padded_shape=[128, NA])
                for k in range(4):
                    nc.tensor.matmul(ps, enc_w_sb[k][:, m * 128:(m + 1) * 128],
                                     encT_sb[k], start=(k == 0), stop=(k == 3))
                e = softsign_from_psum(f"senc{m}", ps, enc_b_sb[:, m:m + 1], NA)
                ev = e.rearrange("p (n t) -> p n t", t=AW)
                nc.vector.tensor_add(
                    ev, ev, c1[m].unsqueeze(2).broadcast_to([128, N, AW]))
                nc.vector.tensor_add(
                    ev, ev, conv_sb[m].unsqueeze(1).broadcast_to([128, N, AW]))
                tm = tp.tile([128, NA], f32, name=f"th{m}", tag="th", bufs=2)
                nc.scalar.activation(tm, e, AF.Tanh)
                th.append(tm)

            ps_l = pp.tile([1, NA], f32, name="log_ps", tag="ps_misc", bufs=3,
                           padded_shape=[128, NA])
            for m in range(2):
                nc.tensor.matmul(ps_l, ap_sb[:, m:m + 1], th[m],
                                 start=(m == 0), stop=(m == 1))
            logit = tp.tile([1, NA], f32, name="logit", bufs=1)
            nc.scalar.activation(logit, ps_l, AF.Identity, bias=apb_sb)
            nc.vector.tensor_add(logit, logit, maskneg_sb)
            lv = logit.rearrange("p (n t) -> p n t", t=AW)
            mx = tp.tile([1, N], f32, name="mx", tag="sm32", bufs=10)
            nc.vector.tensor_reduce(mx, lv, axis=AX.X, op=ALU.max)
            nc.vector.tensor_sub(lv, lv,
                                 mx.unsqueeze(2).broadcast_to([1, N, AW]))
            wexp = tp.tile([1, NA], f32, name="wexp", bufs=1)
            nc.scalar.activation(wexp, logit, AF.Exp)
            wv = wexp.rearrange("p (n t) -> p n t", t=AW)
            ssum = tp.tile([1, N], f32, name="ssum", tag="sm32", bufs=10)
            nc.vector.tensor_reduce(ssum, wv, axis=AX.X, op=ALU.add)
            nc.vector.tensor_scalar_max(ssum, ssum, 1e-12)
            rsum = tp.tile([1, N], f32, name="rsum", tag="sm32", bufs=10)
            nc.vector.reciprocal(rsum, ssum)
            nc.vector.tensor_mul(wv, wv,
                                 rsum.unsqueeze(2).broadcast_to([1, N, AW]))

            wb_ps = pp.tile([128, NA], f32, name="wb_ps", tag="ps_misc",
                            bufs=3, padded_shape=[128, NA])
            nc.tensor.matmul(wb_ps, ones_row, wexp, start=True, stop=True)
            sb_sum = tp.tile([128, N], f32, name="sb_sum", bufs=1)
            nc.vector.tensor_reduce(sb_sum,
                                    wb_ps.rearrange("p (n t) -> p n t", t=AW),
                                    axis=AX.X, op=ALU.add)

            # ---- speed projection ----
            spd1_sb = sb_load("spd1T", (1, DEC))
            r1T = [None] * 4
            for m in range(4):
                ps = pp.tile([128, N], f32, name=f"sp1_ps{m}", tag="ps_main",
                             bufs=2, padded_shape=[128, N])
                nc.tensor.matmul(ps, spd1_sb[:, m * 128:(m + 1) * 128],
                                 speedT_sb, start=True, stop=True)
                t = tp.tile([128, N], f32r, name=f"r1T{m}", tag="r1T", bufs=4)
                nc.scalar.activation(t, ps, AF.Relu, bias=spd1b_sb[:, m:m + 1])
                r1T[m] = t
            spT = [None] * 4
            gemm2("spd2", "spd2T", DEC, ENC, r1T,
                  act_evac(spT, "spT", 4, AF.Tanh, spd2b_sb, dt=f32))

            # ---- context ----
            ctxT = []
            ctxTr = []
            for k in range(4):
                prod = tp.tile([128, NA], f32, name=f"cprod{k}", tag="cprod",
                               bufs=2)
                nc.vector.tensor_mul(prod, encT_sb[k], wb_ps)
                a = tp.tile([128, N], f32, name=f"ctxT{k}", tag="ctxT", bufs=4)
                nc.vector.tensor_reduce(
                    a, prod.rearrange("p (n t) -> p n t", t=AW),
                    axis=AX.X, op=ALU.add)
                bt = tp.tile([128, N], f32, name=f"cb{k}", tag="cb", bufs=2)
                nc.vector.tensor_mul(bt, spT[k], sb_sum)
                nc.vector.tensor_add(a, a, bt)
                ctxT.append(a)
                ar = tp.tile([128, N], f32r, name=f"ctxTr{k}", tag="ctxTr",
                             bufs=4)
                nc.vector.tensor_copy(ar, a)
                ctxTr.append(ar)
            spkrTr = tp.tile([SPK, N], f32r, name="spkrTr", bufs=1)
            nc.vector.tensor_copy(spkrTr, spkrT_sb)

            if phase_limit < 2:
                return None
            # ---- prenet ----
            p1T = [None] * 8
            gemm2("pre1", "pre1T", OUT + SPK, 2 * DEC, [pinT_sb0, pinT_sb1],
                  act_evac(p1T, "p1T", 8, AF.Relu, pre1b_sb))
            preT = [None] * 4
            gemm2("pre2", "pre2T", 2 * DEC, DEC, p1T,
                  act_evac(preT, "preT", 4, AF.Relu, pre2b_sb))

            in_lstm_tiles = preT + ctxTr + [spkrTr]

            if phase_limit < 3:
                return None
            # ---- LSTM layer 0 ----
            l0 = [None] * 6
            gemm2("lstm0", "w0T", ENC + DEC + SPK, GS, in_lstm_tiles,
                  act_evac(l0, "l0", 6,
                           lambda m: AF.Sigmoid if (m < 2 or m >= 4) else AF.Tanh,
                           b0_sb, dt=f32))

            h1T = []
            for k in range(2):
                c = tp.tile([128, N], f32, name=f"c_{k}", tag="cc", bufs=2)
                nc.vector.tensor_mul(c, l0[k], l0[2 + k])
                tc_ = tp.tile([128, N], f32, name=f"tc_{k}", tag="cc", bufs=2)
                nc.scalar.activation(tc_, c, AF.Tanh)
                h = tp.tile([128, N], f32r, name=f"h1T{k}", tag="h1T", bufs=2)
                nc.vector.tensor_mul(h, l0[4 + k], tc_)
                h1T.append(h)

            if phase_limit < 4:
                return None
            # ---- LSTM layer 1 partial gates (batch-on-partition, no bias) ----
            g1p_sb = wp.tile([N, G3], f32, name="g1p_sb", bufs=1)

            def evac_l1_raw(name, c, ps):
                nc.scalar.activation(g1p_sb[:, c * CG:(c + 1) * CG], ps,
                                     AF.Identity)

            # inline gemm2 without transpose for lstm1
            kt1, _ = _kt_of(h1T)
            for c in range(G3 // CG):
                ps = pp.tile([N, CG], f32, name=f"l1_ps{c}", tag="ps_mm",
                             bufs=2, padded_shape=[N, CG])
                _gemm2_chunk("lstm1", dram["w1T"], HS, kt1, h1T, c, 0, 2,
                             ps, 4)
                evac_l1_raw("lstm1", c, ps)

            if phase_limit < 5:
                return None
            # ---- AllReduce partial gates ----
            g1part = dr.tile([N, G3], f32, name="g1part")
            g1full = dr.tile([N, G3], f32, name="g1full", addr_space="Shared")
            nc.sync.dma_start(out=g1part, in_=g1p_sb)
            nc.gpsimd.collective_compute(
                "AllReduce", ALU.add,
                replica_groups=[list(range(NCORES))],
                ins=[g1part], outs=[g1full])
            g1f_sb = wp.tile([N, G3], f32, name="g1f_sb", bufs=1)
            nc.sync.dma_start(out=g1f_sb, in_=g1full)

            # transpose gates to feature-on-partition, add bias, h2
            W = 16 * N  # 512
            igo = []
            for gi in range(3):
                big = tp.tile([128, W], f32, name=f"igo{gi}", tag="hw", bufs=6)
                igo.append(big)
            for m in range(48):
                pst = pp.tile([128, N], f32, name=f"g1t{m}", tag="ps_tr2",
                              bufs=1, padded_shape=[128, 128])
                nc.tensor.transpose(pst, g1f_sb[:, m * 128:(m + 1) * 128],
                                    ident_sb[:N, :N])
                gi, t16 = divmod(m, 16)
                nc.scalar.activation(igo[gi][:, t16 * N:(t16 + 1) * N], pst,
                                     AF.Identity, bias=b1_sb[:, m:m + 1])
            sig_i = tp.tile([128, W], f32, name="sig_i", tag="hw", bufs=6)
            nc.scalar.activation(sig_i, igo[0], AF.Sigmoid)
            tan_g = tp.tile([128, W], f32, name="tan_g", tag="hw", bufs=6)
            nc.scalar.activation(tan_g, igo[1], AF.Tanh)
            nc.vector.tensor_mul(sig_i, sig_i, tan_g)           # c
            nc.scalar.activation(tan_g, sig_i, AF.Tanh)         # tanh(c)
            sig_o = tp.tile([128, W], f32, name="sig_o", tag="hw", bufs=6)
            nc.scalar.activation(sig_o, igo[2], AF.Sigmoid)
            h2_sb = wp.tile([128, W], f32r, name="h2_sb", bufs=1)
            nc.vector.tensor_mul(h2_sb, sig_o, tan_g)

            if phase_limit < 6:
                return None
            # ---- outl1 (in_lstm k-tiles first for AllReduce overlap) ----
            ol1_lhs = in_lstm_tiles + [h2_sb[:, t * N:(t + 1) * N]
                                       for t in range(16)]
            xT = [None] * 4
            gemm2("outl1", "ol1T", H + ENC + DEC + SPK, DEC, ol1_lhs,
                  act_evac(xT, "xT", 14, AF.Identity, ol1b_sb), split_k=9)

            if phase_limit < 7:
                return None

            # ---- transformer ----
            def layer_norm(x_tiles, g_sb, b_sb, nm):
                s_ps = pp.tile([1, N], f32, name=f"{nm}_s", tag="ps_misc",
                               bufs=3, padded_shape=[128, NA])
                for k in range(4):
                    nc.tensor.matmul(s_ps, ones_col_r, x_tiles[k],
                                     start=(k == 0), stop=(k == 3))
                s2_ps = pp.tile([1, N], f32, name=f"{nm}_s2", tag="ps_misc",
                                bufs=3, padded_shape=[128, NA])
                for k in range(4):
                    sq = tp.tile([128, N], f32r, name=f"{nm}_sq{k}", tag="sq",
                                 bufs=2)
                    nc.scalar.activation(sq, x_tiles[k], AF.Square)
                    nc.tensor.matmul(s2_ps, ones_col_r, sq, start=(k == 0),
                                     stop=(k == 3))
                mu = tp.tile([1, N], f32, name=f"{nm}_mu", tag="sm32", bufs=10)
                nc.scalar.activation(mu, s_ps, AF.Identity, scale=1.0 / DEC)
                ms = tp.tile([1, N], f32, name=f"{nm}_ms", tag="sm32", bufs=10)
                nc.scalar.activation(ms, s2_ps, AF.Identity, scale=1.0 / DEC)
                mu2 = tp.tile([1, N], f32, name=f"{nm}_mu2", tag="sm32",
                              bufs=10)
                nc.scalar.activation(mu2, mu, AF.Square)
                var = tp.tile([1, N], f32, name=f"{nm}_var", tag="sm32",
                              bufs=10)
                nc.vector.tensor_sub(var, ms, mu2)
                sd = tp.tile([1, N], f32, name=f"{nm}_sd", tag="sm32", bufs=10)
                nc.scalar.activation(sd, var, AF.Sqrt, bias=eps_sb)
                rstd = tp.tile([1, N], f32, name=f"{nm}_rstd", tag="sm32",
                               bufs=10)
                nc.vector.reciprocal(rstd, sd)
                mub_ps = pp.tile([128, N], f32, name=f"{nm}_mub",
                                 tag="ps_main", bufs=2, padded_shape=[128, N])
                nc.tensor.matmul(mub_ps, ones_row, mu, start=True, stop=True)
                rb_ps = pp.tile([128, N], f32, name=f"{nm}_rb", tag="ps_main",
                                bufs=2, padded_shape=[128, N])
                nc.tensor.matmul(rb_ps, ones_row, rstd, start=True, stop=True)
                out = []
                for k in range(4):
                    xc = tp.tile([128, N], f32, name=f"{nm}_xc{k}", tag="sq2",
                                 bufs=2)
                    nc.vector.tensor_sub(xc, x_tiles[k], mub_ps)
                    nc.vector.tensor_mul(xc, xc, rb_ps)
                    o = tp.tile([128, N], f32r, name=f"{nm}_o{k}", tag="xT",
                                bufs=14)
                    nc.scalar.activation(o, xc, AF.Identity,
                                         bias=b_sb[:, k:k + 1],
                                         scale=g_sb[:, k:k + 1])
                    out.append(o)
                return out

            for l in range(2):
                vT = [None] * 4
                gemm2(f"v{l}", f"wvT{l}", DEC, DEC, xT,
                      act_evac(vT, f"vT{l}", 12, AF.Identity, lb[f"bv{l}"]))

                yT = [None] * 4
                x_res = xT

                def evac_o(m, pst, l=l, x_res=x_res, yT=yT):
                    t = tp.tile([128, N], f32r, name=f"aT{l}_{m}", tag="xT",
                                bufs=14)
                    nc.scalar.activation(t, pst, AF.Identity,
                                         bias=lb[f"bo{l}"][:, m:m + 1])
                    nc.vector.tensor_add(t, t, x_res[m])
                    yT[m] = t

                gemm2(f"o{l}", f"woT{l}", DEC, DEC, vT, evac_o)
                xT = layer_norm(yT, lb[f"ln1g{l}"], lb[f"ln1b{l}"], f"ln1_{l}")

                fT = [None] * 8
                gemm2(f"f1{l}", f"wf1T{l}", DEC, FF, xT,
                      act_evac(fT, f"fT{l}", 8, AF.Relu, lb[f"bf1{l}"]))

                zT = [None] * 4
                x_res2 = xT

                def evac_f2(m, pst, l=l, x_res2=x_res2, zT=zT):
                    t = tp.tile([128, N], f32r, name=f"zT{l}_{m}", tag="xT",
                                bufs=14)
                    nc.scalar.activation(t, pst, AF.Identity,
                                         bias=lb[f"bf2{l}"][:, m:m + 1])
                    nc.vector.tensor_add(t, t, x_res2[m])
                    zT[m] = t

                gemm2(f"f2{l}", f"wf2T{l}", FF, DEC, fT, evac_f2)
                xT = layer_norm(zT, lb[f"ln2g{l}"], lb[f"ln2b{l}"], f"ln2_{l}")

            if phase_limit < 8:
                return None
            # ---- outl2: weights-moving, bias as extra ones-row k-tile ----
            ol2_lhs = xT + [ones_row_r]
            kt2, _ = _kt_of(ol2_lhs)
            ps_o2 = pp.tile([N, 2 * OUT], f32, name="o2_ps", tag="ps_mm",
                            bufs=2, padded_shape=[N, CG])
            for ki, (kk, ks) in enumerate(kt2):
                w_sb = wp.tile([ks, 2 * OUT], f32r, name=f"ol2_w{ki}",
                               tag="ol2_w", bufs=5, padded_shape=[128, 2 * OUT])
                nc.sync.dma_start(out=w_sb,
                                  in_=dram["ol2T"].ap()[kk:kk + ks, :])
                nc.tensor.matmul(ps_o2, ol2_lhs[ki], w_sb, start=(ki == 0),
                                 stop=(ki == 4), skip_group_check=True)
            out_sb = wp.tile([N, 2 * OUT], f32, name="out_sb", bufs=1)
            nc.scalar.activation(out_sb, ps_o2, AF.Identity)
            nc.sync.dma_start(out=out_d.ap(), in_=out_sb)

            if phase_limit < 9:
                return None
            # ---- ctx output: transpose to batch-major ----
            ctx_sb = wp.tile([N, ENC], f32, name="ctx_sb", bufs=1)
            for k in range(4):
                tpk = pp.tile([N, 128], f32, name=f"tpc{k}", tag="ps_tr2",
                              bufs=1, padded_shape=[128, 128])
                nc.tensor.transpose(tpk, ctxT[k], ident_sb)
                nc.vector.tensor_copy(ctx_sb[:, k * 128:(k + 1) * 128], tpk)
            nc.sync.dma_start(out=ctx_d.ap(), in_=ctx_sb)

    nc.compile()
    return nc


def _tcols(wT):
    """(K, M) fp32 -> chunk-tiled (M//CG * K, CG): block c rows [c*K:(c+1)*K]
    hold columns [c*CG:(c+1)*CG]."""
    K, M = wT.shape
    nch = M // CG
    assert nch * CG == M, (K, M)
    return np.ascontiguousarray(
        wT.reshape(K, nch, CG).transpose(1, 0, 2).reshape(nch * K, CG))


def prep_inputs(inputs):
    fz = np.float32

    def g(name):
        return np.asarray(inputs[name], fz)

    ie = g("input_enc")
    spkr = g("spkr_vec")[:, 0, :]
    encT = np.ascontiguousarray(
        ie[:, :AW, :].transpose(2, 0, 1).reshape(ENC, NA))
    spkrT = np.ascontiguousarray(spkr.T)
    speedT = np.ascontiguousarray(g("speed").reshape(1, N))
    pinT = np.ascontiguousarray(
        np.concatenate([g("input_dec"), spkr], axis=1).T)
    lens = np.asarray(inputs["lengths_enc"]).astype(np.int64)
    t = np.arange(AW)
    mask = (t[None, :] <= np.minimum(AW - 1, lens[:, None] - 1)).astype(fz)
    maskneg = np.ascontiguousarray(((mask - 1.0) * 1e4).reshape(1, NA))
    convT = np.ascontiguousarray(g("conv_w")[:, 0, 15 - t])

    ol1T_full = g("outl1_w").T  # (3136, 512); rows: [h2 (2048), in_lstm (1088)]
    ol1T = np.concatenate([ol1T_full[H:], ol1T_full[:H]], axis=0)

    base = {
        "encT": encT, "spkrT": spkrT, "speedT": speedT, "pinT": pinT,
        "maskneg": maskneg, "convT": convT,
        "enc_wT": np.ascontiguousarray(g("enc_w").T),
        "enc_b": np.ascontiguousarray(g("enc_b").reshape(ATT, 1)),
        "spkr_wT": np.ascontiguousarray(g("spkr_w").T),
        "sattT": np.ascontiguousarray(g("speed_att_w").T),
        "apT": np.ascontiguousarray(g("attproj_w").T),
        "apb": np.ascontiguousarray(g("attproj_b").reshape(1, 1)),
        "spd1T": np.ascontiguousarray(g("spd1_w").T),
        "spd1b": np.ascontiguousarray(g("spd1_b").reshape(DEC, 1)),
        "spd2T": _tcols(g("spd2_w").T),
        "spd2b": np.ascontiguousarray(g("spd2_b").reshape(ENC, 1)),
        "pre1T": _tcols(g("pre1_w").T),
        "pre1b": np.ascontiguousarray(g("pre1_b").reshape(2 * DEC, 1)),
        "pre2T": _tcols(g("pre2_w").T),
        "pre2b": np.ascontiguousarray(g("pre2_b").reshape(DEC, 1)),
        "ol1T": _tcols(ol1T),
        "ol1b": np.ascontiguousarray(g("outl1_b").reshape(DEC, 1)),
        "ol2T": np.ascontiguousarray(
            np.concatenate([g("outl2_w").T, g("outl2_b").reshape(1, 2 * OUT)],
                           axis=0)),
        "ident": np.eye(128, dtype=fz),
    }
    for l in range(2):
        base[f"wvT{l}"] = _tcols(g("tr_inproj_w")[l][1024:1536].T)
        base[f"bv{l}"] = np.ascontiguousarray(
            g("tr_inproj_b")[l][1024:1536].reshape(DEC, 1))
        base[f"woT{l}"] = _tcols(g("tr_out_w")[l].T)
        base[f"bo{l}"] = np.ascontiguousarray(
            g("tr_out_b")[l].reshape(DEC, 1))
        base[f"ln1g{l}"] = np.ascontiguousarray(
            g("tr_ln1_g")[l].reshape(DEC, 1))
        base[f"ln1b{l}"] = np.ascontiguousarray(
            g("tr_ln1_b")[l].reshape(DEC, 1))
        base[f"wf1T{l}"] = _tcols(g("tr_ff1_w")[l].T)
        base[f"bf1{l}"] = np.ascontiguousarray(
            g("tr_ff1_b")[l].reshape(FF, 1))
        base[f"wf2T{l}"] = _tcols(g("tr_ff2_w")[l].T)
        base[f"bf2{l}"] = np.ascontiguousarray(
            g("tr_ff2_b")[l].reshape(DEC, 1))
        base[f"ln2g{l}"] = np.ascontiguousarray(
            g("tr_ln2_g")[l].reshape(DEC, 1))
        base[f"ln2b{l}"] = np.ascontiguousarray(
            g("tr_ln2_b")[l].reshape(DEC, 1))

    wih0 = g("lstm_wih0")
    b0full = g("lstm_bih0") + g("lstm_bhh0")
    wih1 = g("lstm_wih1")
    b1full = g("lstm_bih1") + g("lstm_bhh1")
    rows_igo = np.concatenate(
        [np.arange(H), 2 * H + np.arange(H), 3 * H + np.arange(H)])
    w1_igo = wih1[rows_igo]
    base["b1"] = np.ascontiguousarray(b1full[rows_igo].reshape(G3, 1))
    in_maps = []
    for k in range(NCORES):
        hs = np.arange(k * HS, (k + 1) * HS)
        rows0 = np.concatenate([hs, 2 * H + hs, 3 * H + hs])
        m = dict(base)
        m["w0T"] = _tcols(np.ascontiguousarray(wih0[rows0].T))
        m["b0"] = np.ascontiguousarray(b0full[rows0].reshape(GS, 1))
        m["w1T"] = _tcols(
            np.ascontiguousarray(w1_igo[:, k * HS:(k + 1) * HS].T))
        in_maps.append(m)
    return in_maps


_NC = None


def _get_nc():
    global _NC
    if _NC is None:
        _NC = build_program()
    return _NC


_LAST_RESULTS = None


def kernel(**inputs):
    global _LAST_RESULTS
    import os
    nc = _get_nc()
    in_maps = prep_inputs(inputs)
    kw = {}
    if os.environ.get("KERNEL_TRACE"):
        kw["trace"] = True
    res = run_bass_kernel_spmd(nc, in_maps, core_ids=list(range(NCORES)), **kw)
    _LAST_RESULTS = res
    r0 = res.results[0]
    out = np.asarray(r0["out"], np.float32).reshape(N, 2, OUT)
    ctx = np.asarray(r0["ctx"], np.float32).reshape(N, 1, ENC)
    return out, ctx


# revision 19
# speedup vs baseline: 1.0943x; 1.0428x over previous
"""Trainium2 Bass kernel for nn_AttnDecoderRNN3 (sparse_attention).

Strategy (8 NeuronCores):
- Only input_enc[:, :10, :] matters: the attention mask is a fixed 10-wide
  window at t=0 (aw0 is a one-hot at t=0 -> argmax 0) and the softmax
  max-subtraction cancels exactly, so the full-T encoder GEMM is skipped.
- LSTM h0=c0=0 -> whh*/f-gate weights unused. Transformer S=1 -> att == V.
- LSTM layer 0 sharded over the hidden dim (each core computes a 256-wide
  h1 slice for all 32 batch rows); layer 1 sharded over the contraction
  dim (each core's h1 slice x its wih1 column slice -> partial full
  gates), combined with ONE AllReduce; biases added after the AllReduce.
- Everything else is computed redundantly on all cores for all 32 batch
  rows (it is tiny); final outputs are read from core 0.
- Big GEMMs: weights-MOVING through the PE at fp32r (1 col/cycle), with
  the batch (32) as the stationary operand; outputs [32, chunk] are
  PE-transposed back to feature-on-partition with bias+activation fused
  into the ScalarE evacuation. Small GEMMs stay exact fp32 stationary.
"""

import numpy as np

import concourse.bacc as bacc
import concourse.mybir as mybir
import concourse.tile as tile
from concourse.bass_utils import run_bass_kernel_spmd

NCORES = 8
N = 32            # batch
ENC = 512
ATT = 256
DEC = 512
H = 2048          # lstm hidden
SPK = 64
OUT = 80
FF = 1024
AW = 10           # attention window (ATT_RANGE)
HS = H // NCORES  # hidden slice per core = 256
G3 = 3 * H        # i,g,o gates = 6144
GS = 3 * HS       # per-core layer0 gate rows = 768
NA = N * AW       # 320
CG = 256          # gemm2 column-group (chunk) width

f32 = mybir.dt.float32
f32r = mybir.dt.float32r
AF = mybir.ActivationFunctionType
ALU = mybir.AluOpType
AX = mybir.AxisListType


def _ceil_div(a, b):
    return (a + b - 1) // b


def _kt_of(tiles):
    kt = []
    k0 = 0
    for r in tiles:
        ks = r.partition_size()
        kt.append((k0, ks))
        k0 += ks
    return kt, k0


def build_program(phase_limit=99):
    import os
    phase_limit = int(os.environ.get("KPHASE", phase_limit))
    nc = bacc.Bacc("TRN2", target_bir_lowering=False, debug=False,
                   num_devices=NCORES)

    dram = {}

    def din(name, shape, dt=f32):
        dram[name] = nc.dram_tensor(name, list(shape), dt,
                                    kind="ExternalInput")
        return dram[name]

    # ---- inputs ----
    # v2 (tiled, fp32r) weight tensors have shape (M/CG * K, CG)
    def din2(name, K, M):
        din(name, (M // CG * K, CG), f32r)

    din("encT", (ENC, NA))
    din("spkrT", (SPK, N))
    din("speedT", (1, N))
    din("pinT", (OUT + SPK, N), f32r)
    din("maskneg", (1, NA))
    din("convT", (ATT, AW))
    din("enc_wT", (ENC, ATT)); din("enc_b", (128, (ATT) // 128))
    din("spkr_wT", (SPK, ATT))
    din("sattT", (1, ATT))
    din("apT", (128, 2)); din("apb", (1, 1))
    din("spd1T", (1, DEC)); din("spd1b", (128, (DEC) // 128))
    din2("spd2T", DEC, ENC); din("spd2b", (128, (ENC) // 128))
    din2("pre1T", OUT + SPK, 2 * DEC); din("pre1b", (128, (2 * DEC) // 128))
    din2("pre2T", 2 * DEC, DEC); din("pre2b", (128, (DEC) // 128))
    din2("w0T", ENC + DEC + SPK, GS); din("b0", (128, (GS) // 128))
    din2("w1T", HS, G3); din("b1", (128, (G3) // 128))
    din2("ol1T", H + ENC + DEC + SPK, DEC); din("ol1b", (128, (DEC) // 128))
    for l in range(2):
        din2(f"wvT{l}", DEC, DEC); din(f"bv{l}", (128, 4))
        din2(f"woT{l}", DEC, DEC); din(f"bo{l}", (128, 4))
        din(f"ln1g{l}", (128, 4)); din(f"ln1b{l}", (128, 4))
        din2(f"wf1T{l}", DEC, FF); din(f"bf1{l}", (128, 8))
        din2(f"wf2T{l}", FF, DEC); din(f"bf2{l}", (128, 4))
        din(f"ln2g{l}", (128, 4)); din(f"ln2b{l}", (128, 4))
    din("ol2T", (DEC + 1, 2 * OUT), f32r)   # bias folded in as last row
    din("ident", (128, 128))
    out_d = nc.dram_tensor("out", [N, 2 * OUT], f32, kind="ExternalOutput")
    ctx_d = nc.dram_tensor("ctx", [N, ENC], f32, kind="ExternalOutput")

    with tile.TileContext(nc) as tc:
        with tc.tile_pool(name="wp", bufs=1) as wp, \
             tc.tile_pool(name="tp", bufs=2) as tp, \
             tc.tile_pool(name="pp", bufs=1, space="PSUM") as pp, \
             tc.tile_pool(name="dr", bufs=1, space="DRAM") as dr:

            _rr = [0]
            _dma_engs = [nc.sync, nc.gpsimd, nc.scalar]

            def dma_rr(out, in_):
                eng = _dma_engs[_rr[0] % len(_dma_engs)]
                _rr[0] += 1
                eng.dma_start(out=out, in_=in_)

            def sb_load(name, shape, tag=None, bufs=1, dt=f32):
                t = wp.tile(list(shape), dt, name=f"{name}_sb",
                            tag=tag or f"{name}_t", bufs=bufs)
                nc.sync.dma_start(out=t, in_=dram[name].ap())
                return t

            def load_bias(name, M):
                mt = _ceil_div(M, 128)
                assert M % 128 == 0
                t = wp.tile([128, mt], f32, name=f"{name}_sb",
                            tag=f"{name}_b", bufs=1)
                dma_rr(t, dram[name].ap())
                return t

            def softsign_from_psum(name, ps, bias_ap, w):
                val = tp.tile([128, w], f32, name=f"{name}_v",
                              tag=f"ss_v{w}", bufs=2)
                den = tp.tile([128, w], f32, name=f"{name}_d",
                              tag=f"ss_d{w}", bufs=2)
                if bias_ap is None:
                    nc.scalar.activation(val, ps, AF.Identity)
                    nc.scalar.activation(den, ps, AF.Abs)
                else:
                    nc.scalar.activation(val, ps, AF.Identity, bias=bias_ap)
                    nc.scalar.activation(den, ps, AF.Abs, bias=bias_ap)
                nc.vector.tensor_scalar_add(den, den, 1.0)
                nc.vector.reciprocal(den, den)
                nc.vector.tensor_mul(val, val, den)
                return val

            # ---- persistent SBUF loads ----
            encT_sb = []
            for k in range(4):
                t = wp.tile([128, NA], f32, name=f"encT{k}_sb", tag="encT",
                            bufs=4)
                dma_rr(t, dram["encT"].ap()[k * 128:(k + 1) * 128, :])
                encT_sb.append(t)
            spkrT_sb = sb_load("spkrT", (SPK, N))
            speedT_sb = sb_load("speedT", (1, N))
            pinT_sb0 = wp.tile([128, N], f32r, name="pinT0", bufs=1)
            nc.sync.dma_start(out=pinT_sb0, in_=dram["pinT"].ap()[0:128, :])
            pinT_sb1 = wp.tile([16, N], f32r, name="pinT1", bufs=1)
            nc.sync.dma_start(out=pinT_sb1, in_=dram["pinT"].ap()[128:144, :])
            maskneg_sb = sb_load("maskneg", (1, NA))
            conv_sb = []
            for k in range(2):
                t = wp.tile([128, AW], f32, name=f"conv{k}_sb", tag="convT",
                            bufs=2)
                nc.sync.dma_start(out=t,
                                  in_=dram["convT"].ap()[k * 128:(k + 1) * 128, :])
                conv_sb.append(t)
            ident_sb = sb_load("ident", (128, 128))
            apb_sb = sb_load("apb", (1, 1))

            enc_b_sb = load_bias("enc_b", ATT)
            ap_sb = load_bias("apT", ATT)
            spd1b_sb = load_bias("spd1b", DEC)
            spd2b_sb = load_bias("spd2b", ENC)
            pre1b_sb = load_bias("pre1b", 2 * DEC)
            pre2b_sb = load_bias("pre2b", DEC)
            b0_sb = load_bias("b0", GS)
            b1_sb = load_bias("b1", G3)
            ol1b_sb = load_bias("ol1b", DEC)
            lb = {}
            for l in range(2):
                for nm in (f"bv{l}", f"bo{l}", f"ln1g{l}", f"ln1b{l}",
                           f"bf1{l}", f"bf2{l}", f"ln2g{l}", f"ln2b{l}"):
                    M = FF if nm.startswith("bf1") else DEC
                    lb[nm] = load_bias(nm, M)

            ones_col = wp.tile([128, 1], f32, name="ones_col", bufs=1)
            nc.vector.memset(ones_col, 1.0)
            ones_col_r = wp.tile([128, 1], f32r, name="ones_col_r", bufs=1)
            nc.vector.tensor_copy(ones_col_r, ones_col)
            ones_row = wp.tile([1, 128], f32, name="ones_row", bufs=1)
            nc.vector.memset(ones_row, 1.0)
            ones_row_r = wp.tile([1, N], f32r, name="ones_row_r", bufs=1)
            nc.vector.tensor_copy(ones_row_r, ones_row[:, :N])
            eps_sb = wp.tile([1, 1], f32, name="eps_sb", bufs=1)
            nc.vector.memset(eps_sb, 1e-5)

            # ================= gemm v2: weights-moving fp32r =================
            def _kt_runs(kt):
                """Maximal runs of consecutive full-128 k-tiles; <128 tiles
                become singleton runs."""
                runs = []
                i = 0
                while i < len(kt):
                    if kt[i][1] == 128:
                        j = i
                        while j < len(kt) and kt[j][1] == 128 and j - i < 8:
                            j += 1
                        runs.append((i, j - i))
                        i = j
                    else:
                        runs.append((i, 1))
                        i += 1
                return runs

            def gemm2(name, w_name, K, M, lhsT_tiles, evac_t, wbufs=3,
                      split_k=None):
                """y[32, M] = x @ W^T streamed in CG-col chunks; each 128-col
                output tile is PE-transposed back to [128, 32] and handed to
                evac_t(m, psum_t) for fused bias+activation evacuation.

                One weight DMA per (chunk, k-tile-run). split_k: k-tiles <
                split_k are emitted for ALL chunks first (AllReduce overlap),
                then the rest (split must fall on a run boundary)."""
                w_dram = dram[w_name]
                kt, ksum = _kt_of(lhsT_tiles)
                assert ksum == K, (name, ksum, K)
                nkt = len(kt)
                nch = M // CG
                runs = _kt_runs(kt)

                def load_run(c, i0, cnt):
                    kk, ks0 = kt[i0]
                    w_sb = wp.tile([ks0, cnt * CG], f32r,
                                   name=f"{name}_w{i0}_{c}",
                                   tag="w2" if ks0 == 128 else "w2r",
                                   bufs=4, padded_shape=[128, 8 * CG])
                    rows = w_dram.ap()[c * K + kk:c * K + kk + cnt * ks0, :]
                    if cnt == 1:
                        dma_rr(w_sb, rows)
                    else:
                        dma_rr(w_sb.rearrange("p (t j) -> p t j", j=CG),
                               rows.rearrange("(t p) j -> p t j", p=128))
                    return w_sb

                def mm_runs(ps, c, sel_runs):
                    for i0, cnt in sel_runs:
                        w_sb = load_run(c, i0, cnt)
                        for idx in range(cnt):
                            ki = i0 + idx
                            nc.tensor.matmul(
                                ps, lhsT_tiles[ki],
                                w_sb[:, idx * CG:(idx + 1) * CG],
                                start=(ki == 0), stop=(ki == nkt - 1),
                                skip_group_check=True)

                if split_k is None:
                    for c in range(nch):
                        ps = pp.tile([N, CG], f32, name=f"{name}_ps{c}",
                                     tag="ps_mm", bufs=2,
                                     padded_shape=[N, CG])
                        mm_runs(ps, c, runs)
                        _gemm2_evac(name, c, ps, evac_t)
                else:
                    runs_a = [r for r in runs if r[0] < split_k]
                    runs_b = [r for r in runs if r[0] >= split_k]
                    assert sum(cnt for _, cnt in runs_a) == split_k
                    pss = {}
                    for c in range(nch):
                        pss[c] = pp.tile([N, CG], f32, name=f"{name}_ps{c}",
                                         tag="ps_mm", bufs=2,
                                         padded_shape=[N, CG])
                    for c in range(nch):
                        mm_runs(pss[c], c, runs_a)
                    for c in range(nch):
                        mm_runs(pss[c], c, runs_b)
                    for c in range(nch):
                        _gemm2_evac(name, c, pss[c], evac_t)

            def _gemm2_evac(name, c, ps, evac_t):
                y32 = tp.tile([N, CG], f32, name=f"{name}_y{c}", tag="y32",
                              bufs=3)
                nc.scalar.activation(y32, ps, AF.Identity)
                for j in range(CG // 128):
                    m = c * (CG // 128) + j
                    pst = pp.tile([128, N], f32, name=f"{name}_pt{m}",
                                  tag="ps_tr2", bufs=1, padded_shape=[128, 128])
                    nc.tensor.transpose(pst, y32[:, j * 128:(j + 1) * 128],
                                        ident_sb[:N, :N])
                    evac_t(m, pst)

            def act_evac(out_list, tag, bufs, func, bias_sb, dt=f32r):
                def _e(m, pst):
                    t = tp.tile([128, N], dt, name=f"{tag}_{m}", tag=tag,
                                bufs=bufs)
                    f = func(m) if callable(func) else func
                    nc.scalar.activation(t, pst, f, bias=bias_sb[:, m:m + 1])
                    out_list[m] = t
                return _e

            # =========== attention (exact fp32, weights-stationary) ==========
            spkr_w_sb = sb_load("spkr_wT", (SPK, ATT))
            satt_sb = sb_load("sattT", (1, ATT))
            c1 = []
            for m in range(2):
                ps = pp.tile([128, N], f32, name=f"spk_ps{m}", tag="ps_main",
                             bufs=2, padded_shape=[128, N])
                nc.tensor.matmul(ps, spkr_w_sb[:, m * 128:(m + 1) * 128],
                                 spkrT_sb, start=True, stop=True)
                s_spkr = softsign_from_psum(f"sspkr{m}", ps, None, N)
                ps2 = pp.tile([128, N], f32, name=f"sat_ps{m}", tag="ps_main",
                              bufs=2, padded_shape=[128, N])
                nc.tensor.matmul(ps2, satt_sb[:, m * 128:(m + 1) * 128],
                                 speedT_sb, start=True, stop=True)
                cm = tp.tile([128, N], f32, name=f"c1_{m}", tag="c1", bufs=2)
                nc.vector.tensor_add(cm, s_spkr, ps2)
                c1.append(cm)

            enc_w_sb = []
            for k in range(4):
                t = wp.tile([128, ATT], f32, name=f"encw{k}", tag="enc_wT",
                            bufs=4)
                dma_rr(t, dram["enc_wT"].ap()[k * 128:(k + 1) * 128, :])
                enc_w_sb.append(t)
            th = []
            for m in range(2):
                ps = pp.tile([128, NA], f32, name=f"eatt_ps{m}", tag="ps_misc",
                             bufs=3, padded_shape=[128, NA])
                for k in range(4):
                    nc.tensor.matmul(ps, enc_w_sb[k][:, m * 128:(m + 1) * 128],
                                     encT_sb[k], start=(k == 0), stop=(k == 3))
                e = softsign_from_psum(f"senc{m}", ps, enc_b_sb[:, m:m + 1], NA)
                ev = e.rearrange("p (n t) -> p n t", t=AW)
                nc.vector.tensor_add(
                    ev, ev, c1[m].unsqueeze(2).broadcast_to([128, N, AW]))
                nc.vector.tensor_add(
                    ev, ev, conv_sb[m].unsqueeze(1).broadcast_to([128, N, AW]))
                tm = tp.tile([128, NA], f32, name=f"th{m}", tag="th", bufs=2)
                nc.scalar.activation(tm, e, AF.Tanh)
                th.append(tm)

            ps_l = pp.tile([1, NA], f32, name="log_ps", tag="ps_misc", bufs=3,
                           padded_shape=[128, NA])
            for m in range(2):
                nc.tensor.matmul(ps_l, ap_sb[:, m:m + 1], th[m],
                                 start=(m == 0), stop=(m == 1))
            logit = tp.tile([1, NA], f32, name="logit", bufs=1)
            nc.scalar.activation(logit, ps_l, AF.Identity, bias=apb_sb)
            nc.vector.tensor_add(logit, logit, maskneg_sb)
            lv = logit.rearrange("p (n t) -> p n t", t=AW)
            mx = tp.tile([1, N], f32, name="mx", tag="sm32", bufs=10)
            nc.vector.tensor_reduce(mx, lv, axis=AX.X, op=ALU.max)
            nc.vector.tensor_sub(lv, lv,
                                 mx.unsqueeze(2).broadcast_to([1, N, AW]))
            wexp = tp.tile([1, NA], f32, name="wexp", bufs=1)
            nc.scalar.activation(wexp, logit, AF.Exp)
            wv = wexp.rearrange("p (n t) -> p n t", t=AW)
            ssum = tp.tile([1, N], f32, name="ssum", tag="sm32", bufs=10)
            nc.vector.tensor_reduce(ssum, wv, axis=AX.X, op=ALU.add)
            nc.vector.tensor_scalar_max(ssum, ssum, 1e-12)
            rsum = tp.tile([1, N], f32, name="rsum", tag="sm32", bufs=10)
            nc.vector.reciprocal(rsum, ssum)
            nc.vector.tensor_mul(wv, wv,
                                 rsum.unsqueeze(2).broadcast_to([1, N, AW]))

            wb_ps = pp.tile([128, NA], f32, name="wb_ps", tag="ps_misc",
                            bufs=3, padded_shape=[128, NA])
            nc.tensor.matmul(wb_ps, ones_row, wexp, start=True, stop=True)
            sb_sum = tp.tile([128, N], f32, name="sb_sum", bufs=1)
            nc.vector.tensor_reduce(sb_sum,
                                    wb_ps.rearrange("p (n t) -> p n t", t=AW),
                                    axis=AX.X, op=ALU.add)

            # ---- speed projection ----
            spd1_sb = sb_load("spd1T", (1, DEC))
            r1T = [None] * 4
            for m in range(4):
                ps = pp.tile([128, N], f32, name=f"sp1_ps{m}", tag="ps_main",
                             bufs=2, padded_shape=[128, N])
                nc.tensor.matmul(ps, spd1_sb[:, m * 128:(m + 1) * 128],
                                 speedT_sb, start=True, stop=True)
                t = tp.tile([128, N], f32r, name=f"r1T{m}", tag="r1T", bufs=4)
                nc.scalar.activation(t, ps, AF.Relu, bias=spd1b_sb[:, m:m + 1])
                r1T[m] = t
            spT = [None] * 4
            gemm2("spd2", "spd2T", DEC, ENC, r1T,
                  act_evac(spT, "spT", 4, AF.Tanh, spd2b_sb, dt=f32))

            # ---- context ----
            ctxT = []
            ctxTr = []
            for k in range(4):
                prod = tp.tile([128, NA], f32, name=f"cprod{k}", tag="cprod",
                               bufs=2)
                nc.vector.tensor_mul(prod, encT_sb[k], wb_ps)
                a = tp.tile([128, N], f32, name=f"ctxT{k}", tag="ctxT", bufs=4)
                nc.vector.tensor_reduce(
                    a, prod.rearrange("p (n t) -> p n t", t=AW),
                    axis=AX.X, op=ALU.add)
                bt = tp.tile([128, N], f32, name=f"cb{k}", tag="cb", bufs=2)
                nc.vector.tensor_mul(bt, spT[k], sb_sum)
                nc.vector.tensor_add(a, a, bt)
                ctxT.append(a)
                ar = tp.tile([128, N], f32r, name=f"ctxTr{k}", tag="ctxTr",
                             bufs=4)
                nc.vector.tensor_copy(ar, a)
                ctxTr.append(ar)
            spkrTr = tp.tile([SPK, N], f32r, name="spkrTr", bufs=1)
            nc.vector.tensor_copy(spkrTr, spkrT_sb)

            if phase_limit < 2:
                return None
            # ---- prenet ----
            p1T = [None] * 8
            gemm2("pre1", "pre1T", OUT + SPK, 2 * DEC, [pinT_sb0, pinT_sb1],
                  act_evac(p1T, "p1T", 8, AF.Relu, pre1b_sb))
            preT = [None] * 4
            gemm2("pre2", "pre2T", 2 * DEC, DEC, p1T,
                  act_evac(preT, "preT", 4, AF.Relu, pre2b_sb))

            in_lstm_tiles = preT + ctxTr + [spkrTr]

            if phase_limit < 3:
                return None
            # ---- LSTM layer 0 ----
            l0 = [None] * 6
            gemm2("lstm0", "w0T", ENC + DEC + SPK, GS, in_lstm_tiles,
                  act_evac(l0, "l0", 6,
                           lambda m: AF.Sigmoid if (m < 2 or m >= 4) else AF.Tanh,
                           b0_sb, dt=f32))

            h1T = []
            for k in range(2):
                c = tp.tile([128, N], f32, name=f"c_{k}", tag="cc", bufs=2)
                nc.vector.tensor_mul(c, l0[k], l0[2 + k])
                tc_ = tp.tile([128, N], f32, name=f"tc_{k}", tag="cc", bufs=2)
                nc.scalar.activation(tc_, c, AF.Tanh)
                h = tp.tile([128, N], f32r, name=f"h1T{k}", tag="h1T", bufs=2)
                nc.vector.tensor_mul(h, l0[4 + k], tc_)
                h1T.append(h)

            if phase_limit < 4:
                return None
            # ---- LSTM layer 1 partial gates (batch-on-partition, no bias) ----
            g1p_sb = wp.tile([N, G3], f32, name="g1p_sb", bufs=1)

            def evac_l1_raw(name, c, ps):
                nc.scalar.activation(g1p_sb[:, c * CG:(c + 1) * CG], ps,
                                     AF.Identity)

            # inline gemm2 without transpose for lstm1
            kt1, _ = _kt_of(h1T)
            for c in range(G3 // CG):
                w_sb = wp.tile([128, 2 * CG], f32r, name=f"l1_w{c}",
                               tag="w2", bufs=4,
                               padded_shape=[128, 8 * CG])
                rows = dram["w1T"].ap()[c * HS:(c + 1) * HS, :]
                dma_rr(w_sb.rearrange("p (t j) -> p t j", j=CG),
                       rows.rearrange("(t p) j -> p t j", p=128))
                ps = pp.tile([N, CG], f32, name=f"l1_ps{c}", tag="ps_mm",
                             bufs=2, padded_shape=[N, CG])
                for ki in range(2):
                    nc.tensor.matmul(ps, h1T[ki],
                                     w_sb[:, ki * CG:(ki + 1) * CG],
                                     start=(ki == 0), stop=(ki == 1),
                                     skip_group_check=True)
                evac_l1_raw("lstm1", c, ps)

            if phase_limit < 5:
                return None
            # ---- AllReduce partial gates ----
            g1part = dr.tile([N, G3], f32, name="g1part")
            g1full = dr.tile([N, G3], f32, name="g1full", addr_space="Shared")
            nc.sync.dma_start(out=g1part, in_=g1p_sb)
            nc.gpsimd.collective_compute(
                "AllReduce", ALU.add,
                replica_groups=[list(range(NCORES))],
                ins=[g1part], outs=[g1full])
            g1f_sb = wp.tile([N, G3], f32, name="g1f_sb", bufs=1)
            nc.sync.dma_start(out=g1f_sb, in_=g1full)

            # transpose gates to feature-on-partition, add bias, h2
            W = 16 * N  # 512
            igo = []
            for gi in range(3):
                big = tp.tile([128, W], f32, name=f"igo{gi}", tag="hw", bufs=6)
                igo.append(big)
            for m in range(48):
                pst = pp.tile([128, N], f32, name=f"g1t{m}", tag="ps_tr2",
                              bufs=1, padded_shape=[128, 128])
                nc.tensor.transpose(pst, g1f_sb[:, m * 128:(m + 1) * 128],
                                    ident_sb[:N, :N])
                gi, t16 = divmod(m, 16)
                nc.scalar.activation(igo[gi][:, t16 * N:(t16 + 1) * N], pst,
                                     AF.Identity, bias=b1_sb[:, m:m + 1])
            sig_i = tp.tile([128, W], f32, name="sig_i", tag="hw", bufs=6)
            nc.scalar.activation(sig_i, igo[0], AF.Sigmoid)
            tan_g = tp.tile([128, W], f32, name="tan_g", tag="hw", bufs=6)
            nc.scalar.activation(tan_g, igo[1], AF.Tanh)
            nc.vector.tensor_mul(sig_i, sig_i, tan_g)           # c
            nc.scalar.activation(tan_g, sig_i, AF.Tanh)         # tanh(c)
            sig_o = tp.tile([128, W], f32, name="sig_o", tag="hw", bufs=6)
            nc.scalar.activation(sig_o, igo[2], AF.Sigmoid)
            h2_sb = wp.tile([128, W], f32r, name="h2_sb", bufs=1)
            nc.vector.tensor_mul(h2_sb, sig_o, tan_g)

            if phase_limit < 6:
                return None
            # ---- outl1 (in_lstm k-tiles first for AllReduce overlap) ----
            ol1_lhs = in_lstm_tiles + [h2_sb[:, t * N:(t + 1) * N]
                                       for t in range(16)]
            xT = [None] * 4
            gemm2("outl1", "ol1T", H + ENC + DEC + SPK, DEC, ol1_lhs,
                  act_evac(xT, "xT", 14, AF.Identity, ol1b_sb), split_k=9)

            if phase_limit < 7:
                return None

            # ---- transformer ----
            def layer_norm(x_tiles, g_sb, b_sb, nm):
                s_ps = pp.tile([1, N], f32, name=f"{nm}_s", tag="ps_misc",
                               bufs=3, padded_shape=[128, NA])
                for k in range(4):
                    nc.tensor.matmul(s_ps, ones_col_r, x_tiles[k],
                                     start=(k == 0), stop=(k == 3))
                s2_ps = pp.tile([1, N], f32, name=f"{nm}_s2", tag="ps_misc",
                                bufs=3, padded_shape=[128, NA])
                for k in range(4):
                    sq = tp.tile([128, N], f32r, name=f"{nm}_sq{k}", tag="sq",
                                 bufs=2)
                    nc.scalar.activation(sq, x_tiles[k], AF.Square)
                    nc.tensor.matmul(s2_ps, ones_col_r, sq, start=(k == 0),
                                     stop=(k == 3))
                mu = tp.tile([1, N], f32, name=f"{nm}_mu", tag="sm32", bufs=10)
                nc.scalar.activation(mu, s_ps, AF.Identity, scale=1.0 / DEC)
                ms = tp.tile([1, N], f32, name=f"{nm}_ms", tag="sm32", bufs=10)
                nc.scalar.activation(ms, s2_ps, AF.Identity, scale=1.0 / DEC)
                mu2 = tp.tile([1, N], f32, name=f"{nm}_mu2", tag="sm32",
                              bufs=10)
                nc.scalar.activation(mu2, mu, AF.Square)
                var = tp.tile([1, N], f32, name=f"{nm}_var", tag="sm32",
                              bufs=10)
                nc.vector.tensor_sub(var, ms, mu2)
                sd = tp.tile([1, N], f32, name=f"{nm}_sd", tag="sm32", bufs=10)
                nc.scalar.activation(sd, var, AF.Sqrt, bias=eps_sb)
                rstd = tp.tile([1, N], f32, name=f"{nm}_rstd", tag="sm32",
                               bufs=10)
                nc.vector.reciprocal(rstd, sd)
                mub_ps = pp.tile([128, N], f32, name=f"{nm}_mub",
                                 tag="ps_main", bufs=2, padded_shape=[128, N])
                nc.tensor.matmul(mub_ps, ones_row, mu, start=True, stop=True)
                rb_ps = pp.tile([128, N], f32, name=f"{nm}_rb", tag="ps_main",
                                bufs=2, padded_shape=[128, N])
                nc.tensor.matmul(rb_ps, ones_row, rstd, start=True, stop=True)
                out = []
                for k in range(4):
                    xc = tp.tile([128, N], f32, name=f"{nm}_xc{k}", tag="sq2",
                                 bufs=2)
                    nc.vector.tensor_sub(xc, x_tiles[k], mub_ps)
                    nc.vector.tensor_mul(xc, xc, rb_ps)
                    o = tp.tile([128, N], f32r, name=f"{nm}_o{k}", tag="xT",
                                bufs=14)
                    nc.scalar.activation(o, xc, AF.Identity,
                                         bias=b_sb[:, k:k + 1],
                                         scale=g_sb[:, k:k + 1])
                    out.append(o)
                return out

            for l in range(2):
                vT = [None] * 4
                gemm2(f"v{l}", f"wvT{l}", DEC, DEC, xT,
                      act_evac(vT, f"vT{l}", 12, AF.Identity, lb[f"bv{l}"]))

                yT = [None] * 4
                x_res = xT

                def evac_o(m, pst, l=l, x_res=x_res, yT=yT):
                    t = tp.tile([128, N], f32r, name=f"aT{l}_{m}", tag="xT",
                                bufs=14)
                    nc.scalar.activation(t, pst, AF.Identity,
                                         bias=lb[f"bo{l}"][:, m:m + 1])
                    nc.vector.tensor_add(t, t, x_res[m])
                    yT[m] = t

                gemm2(f"o{l}", f"woT{l}", DEC, DEC, vT, evac_o)
                xT = layer_norm(yT, lb[f"ln1g{l}"], lb[f"ln1b{l}"], f"ln1_{l}")

                fT = [None] * 8
                gemm2(f"f1{l}", f"wf1T{l}", DEC, FF, xT,
                      act_evac(fT, f"fT{l}", 8, AF.Relu, lb[f"bf1{l}"]))

                zT = [None] * 4
                x_res2 = xT

                def evac_f2(m, pst, l=l, x_res2=x_res2, zT=zT):
                    t = tp.tile([128, N], f32r, name=f"zT{l}_{m}", tag="xT",
                                bufs=14)
                    nc.scalar.activation(t, pst, AF.Identity,
                                         bias=lb[f"bf2{l}"][:, m:m + 1])
                    nc.vector.tensor_add(t, t, x_res2[m])
                    zT[m] = t

                gemm2(f"f2{l}", f"wf2T{l}", FF, DEC, fT, evac_f2)
                xT = layer_norm(zT, lb[f"ln2g{l}"], lb[f"ln2b{l}"], f"ln2_{l}")

            if phase_limit < 8:
                return None
            # ---- outl2: weights-moving, bias as extra ones-row k-tile ----
            ol2_lhs = xT + [ones_row_r]
            kt2, _ = _kt_of(ol2_lhs)
            ps_o2 = pp.tile([N, 2 * OUT], f32, name="o2_ps", tag="ps_mm",
                            bufs=2, padded_shape=[N, CG])
            for ki, (kk, ks) in enumerate(kt2):
                w_sb = wp.tile([ks, 2 * OUT], f32r, name=f"ol2_w{ki}",
                               tag="ol2_w", bufs=5, padded_shape=[128, 2 * OUT])
                dma_rr(w_sb, dram["ol2T"].ap()[kk:kk + ks, :])
                nc.tensor.matmul(ps_o2, ol2_lhs[ki], w_sb, start=(ki == 0),
                                 stop=(ki == 4), skip_group_check=True)
            out_sb = wp.tile([N, 2 * OUT], f32, name="out_sb", bufs=1)
            nc.scalar.activation(out_sb, ps_o2, AF.Identity)
            nc.sync.dma_start(out=out_d.ap(), in_=out_sb)

            if phase_limit < 9:
                return None
            # ---- ctx output: transpose to batch-major ----
            ctx_sb = wp.tile([N, ENC], f32, name="ctx_sb", bufs=1)
            for k in range(4):
                tpk = pp.tile([N, 128], f32, name=f"tpc{k}", tag="ps_tr2",
                              bufs=1, padded_shape=[128, 128])
                nc.tensor.transpose(tpk, ctxT[k], ident_sb)
                nc.vector.tensor_copy(ctx_sb[:, k * 128:(k + 1) * 128], tpk)
            nc.sync.dma_start(out=ctx_d.ap(), in_=ctx_sb)

    nc.compile()
    return nc


def _tcols(wT):
    """(K, M) fp32 -> chunk-tiled (M//CG * K, CG): block c rows [c*K:(c+1)*K]
    hold columns [c*CG:(c+1)*CG]."""
    K, M = wT.shape
    nch = M // CG
    assert nch * CG == M, (K, M)
    return np.ascontiguousarray(
        wT.reshape(K, nch, CG).transpose(1, 0, 2).reshape(nch * K, CG))


def _bias128(b):
    """(M,) -> (128, M//128): col m holds b[m*128:(m+1)*128]."""
    M = b.shape[0]
    return np.ascontiguousarray(b.reshape(M // 128, 128).T)


def prep_inputs(inputs):
    fz = np.float32

    def g(name):
        return np.asarray(inputs[name], fz)

    ie = g("input_enc")
    spkr = g("spkr_vec")[:, 0, :]
    encT = np.ascontiguousarray(
        ie[:, :AW, :].transpose(2, 0, 1).reshape(ENC, NA))
    spkrT = np.ascontiguousarray(spkr.T)
    speedT = np.ascontiguousarray(g("speed").reshape(1, N))
    pinT = np.ascontiguousarray(
        np.concatenate([g("input_dec"), spkr], axis=1).T)
    lens = np.asarray(inputs["lengths_enc"]).astype(np.int64)
    t = np.arange(AW)
    mask = (t[None, :] <= np.minimum(AW - 1, lens[:, None] - 1)).astype(fz)
    maskneg = np.ascontiguousarray(((mask - 1.0) * 1e4).reshape(1, NA))
    convT = np.ascontiguousarray(g("conv_w")[:, 0, 15 - t])

    ol1T_full = g("outl1_w").T  # (3136, 512); rows: [h2 (2048), in_lstm (1088)]
    ol1T = np.concatenate([ol1T_full[H:], ol1T_full[:H]], axis=0)

    base = {
        "encT": encT, "spkrT": spkrT, "speedT": speedT, "pinT": pinT,
        "maskneg": maskneg, "convT": convT,
        "enc_wT": np.ascontiguousarray(g("enc_w").T),
        "enc_b": _bias128(g("enc_b")),
        "spkr_wT": np.ascontiguousarray(g("spkr_w").T),
        "sattT": np.ascontiguousarray(g("speed_att_w").T),
        "apT": _bias128(g("attproj_w").reshape(ATT)),
        "apb": np.ascontiguousarray(g("attproj_b").reshape(1, 1)),
        "spd1T": np.ascontiguousarray(g("spd1_w").T),
        "spd1b": _bias128(g("spd1_b")),
        "spd2T": _tcols(g("spd2_w").T),
        "spd2b": _bias128(g("spd2_b")),
        "pre1T": _tcols(g("pre1_w").T),
        "pre1b": _bias128(g("pre1_b")),
        "pre2T": _tcols(g("pre2_w").T),
        "pre2b": _bias128(g("pre2_b")),
        "ol1T": _tcols(ol1T),
        "ol1b": _bias128(g("outl1_b")),
        "ol2T": np.ascontiguousarray(
            np.concatenate([g("outl2_w").T, g("outl2_b").reshape(1, 2 * OUT)],
                           axis=0)),
        "ident": np.eye(128, dtype=fz),
    }
    for l in range(2):
        base[f"wvT{l}"] = _tcols(g("tr_inproj_w")[l][1024:1536].T)
        base[f"bv{l}"] = _bias128(g("tr_inproj_b")[l][1024:1536])
        base[f"woT{l}"] = _tcols(g("tr_out_w")[l].T)
        base[f"bo{l}"] = _bias128(g("tr_out_b")[l])
        base[f"ln1g{l}"] = _bias128(g("tr_ln1_g")[l])
        base[f"ln1b{l}"] = _bias128(g("tr_ln1_b")[l])
        base[f"wf1T{l}"] = _tcols(g("tr_ff1_w")[l].T)
        base[f"bf1{l}"] = _bias128(g("tr_ff1_b")[l])
        base[f"wf2T{l}"] = _tcols(g("tr_ff2_w")[l].T)
        base[f"bf2{l}"] = _bias128(g("tr_ff2_b")[l])
        base[f"ln2g{l}"] = _bias128(g("tr_ln2_g")[l])
        base[f"ln2b{l}"] = _bias128(g("tr_ln2_b")[l])

    wih0 = g("lstm_wih0")
    b0full = g("lstm_bih0") + g("lstm_bhh0")
    wih1 = g("lstm_wih1")
    b1full = g("lstm_bih1") + g("lstm_bhh1")
    rows_igo = np.concatenate(
        [np.arange(H), 2 * H + np.arange(H), 3 * H + np.arange(H)])
    w1_igo = wih1[rows_igo]
    base["b1"] = _bias128(b1full[rows_igo])
    in_maps = []
    for k in range(NCORES):
        hs = np.arange(k * HS, (k + 1) * HS)
        rows0 = np.concatenate([hs, 2 * H + hs, 3 * H + hs])
        m = dict(base)
        m["w0T"] = _tcols(np.ascontiguousarray(wih0[rows0].T))
        m["b0"] = _bias128(b0full[rows0])
        m["w1T"] = _tcols(
            np.ascontiguousarray(w1_igo[:, k * HS:(k + 1) * HS].T))
        in_maps.append(m)
    return in_maps


_NC = None


def _get_nc():
    global _NC
    if _NC is None:
        _NC = build_program()
    return _NC


_LAST_RESULTS = None


def kernel(**inputs):
    global _LAST_RESULTS
    import os
    nc = _get_nc()
    in_maps = prep_inputs(inputs)
    kw = {}
    if os.environ.get("KERNEL_TRACE"):
        kw["trace"] = True
    res = run_bass_kernel_spmd(nc, in_maps, core_ids=list(range(NCORES)), **kw)
    _LAST_RESULTS = res
    r0 = res.results[0]
    out = np.asarray(r0["out"], np.float32).reshape(N, 2, OUT)
    ctx = np.asarray(r0["ctx"], np.float32).reshape(N, 1, ENC)
    return out, ctx


# revision 21
# speedup vs baseline: 1.3747x; 1.2562x over previous
"""Trainium2 Bass kernel for nn_AttnDecoderRNN3 (sparse_attention).

Strategy (8 NeuronCores):
- Only input_enc[:, :10, :] matters: the attention mask is a fixed 10-wide
  window at t=0 (aw0 is a one-hot at t=0 -> argmax 0) and the softmax
  max-subtraction cancels exactly, so the full-T encoder GEMM is skipped.
- LSTM h0=c0=0 -> whh*/f-gate weights unused. Transformer S=1 -> att == V.
- LSTM layer 0 sharded over the hidden dim (each core computes a 256-wide
  h1 slice for all 32 batch rows); layer 1 sharded over the contraction
  dim (each core's h1 slice x its wih1 column slice -> partial full
  gates), combined with ONE AllReduce; biases added after the AllReduce.
- Everything else is computed redundantly on all cores for all 32 batch
  rows (it is tiny); final outputs are read from core 0.
- Big GEMMs: weights-MOVING through the PE at fp32r (1 col/cycle), with
  the batch (32) as the stationary operand; outputs [32, chunk] are
  PE-transposed back to feature-on-partition with bias+activation fused
  into the ScalarE evacuation. Small GEMMs stay exact fp32 stationary.
"""

import numpy as np

import concourse.bacc as bacc
import concourse.mybir as mybir
import concourse.tile as tile
from concourse.bass_utils import run_bass_kernel_spmd

NCORES = 8
N = 32            # batch
ENC = 512
ATT = 256
DEC = 512
H = 2048          # lstm hidden
SPK = 64
OUT = 80
FF = 1024
AW = 10           # attention window (ATT_RANGE)
HS = H // NCORES  # hidden slice per core = 256
G3 = 3 * H        # i,g,o gates = 6144
GS = 3 * HS       # per-core layer0 gate rows = 768
NA = N * AW       # 320
CG = 512          # gemm2 column-group (chunk) width

f32 = mybir.dt.float32
f32r = mybir.dt.float32r
AF = mybir.ActivationFunctionType
ALU = mybir.AluOpType
AX = mybir.AxisListType


def _ceil_div(a, b):
    return (a + b - 1) // b


def _kt_of(tiles):
    kt = []
    k0 = 0
    for r in tiles:
        ks = r.partition_size()
        kt.append((k0, ks))
        k0 += ks
    return kt, k0


def build_program(phase_limit=99):
    import os
    phase_limit = int(os.environ.get("KPHASE", phase_limit))
    nc = bacc.Bacc("TRN2", target_bir_lowering=False, debug=False,
                   num_devices=NCORES)

    dram = {}

    def din(name, shape, dt=f32):
        dram[name] = nc.dram_tensor(name, list(shape), dt,
                                    kind="ExternalInput")
        return dram[name]

    # ---- inputs ----
    # v2 (tiled, fp32r) weight tensors have shape (M/CG * K, CG)
    def din2(name, K, M):
        din(name, (_ceil_div(M, CG) * K, CG), f32r)

    din("encT", (ENC, NA))
    din("spkrT", (SPK, N))
    din("speedT", (1, N))
    din("pinT", (OUT + SPK, N), f32r)
    din("maskneg", (1, NA))
    din("convT", (ATT, AW))
    din("enc_wT", (ENC, ATT)); din("enc_b", (128, (ATT) // 128))
    din("spkr_wT", (SPK, ATT))
    din("sattT", (1, ATT))
    din("apT", (128, 2)); din("apb", (1, 1))
    din("spd1T", (1, DEC)); din("spd1b", (128, (DEC) // 128))
    din2("spd2T", DEC, ENC); din("spd2b", (128, (ENC) // 128))
    din2("pre1T", OUT + SPK, 2 * DEC); din("pre1b", (128, (2 * DEC) // 128))
    din2("pre2T", 2 * DEC, DEC); din("pre2b", (128, (DEC) // 128))
    din2("w0T", ENC + DEC + SPK, GS); din("b0", (128, (GS) // 128))
    din2("w1T", HS, G3); din("b1", (128, (G3) // 128))
    din2("ol1T", H + ENC + DEC + SPK, DEC); din("ol1b", (128, (DEC) // 128))
    for l in range(2):
        din2(f"wvT{l}", DEC, DEC); din(f"bv{l}", (128, 4))
        din2(f"woT{l}", DEC, DEC); din(f"bo{l}", (128, 4))
        din(f"ln1g{l}", (128, 4)); din(f"ln1b{l}", (128, 4))
        din2(f"wf1T{l}", DEC, FF); din(f"bf1{l}", (128, 8))
        din2(f"wf2T{l}", FF, DEC); din(f"bf2{l}", (128, 4))
        din(f"ln2g{l}", (128, 4)); din(f"ln2b{l}", (128, 4))
    din("ol2T", (DEC + 1, 2 * OUT), f32r)   # bias folded in as last row
    din("ident", (128, 128))
    out_d = nc.dram_tensor("out", [N, 2 * OUT], f32, kind="ExternalOutput")
    ctx_d = nc.dram_tensor("ctx", [N, ENC], f32, kind="ExternalOutput")

    with tile.TileContext(nc) as tc:
        with tc.tile_pool(name="wp", bufs=1) as wp, \
             tc.tile_pool(name="tp", bufs=2) as tp, \
             tc.tile_pool(name="pp", bufs=1, space="PSUM") as pp, \
             tc.tile_pool(name="dr", bufs=1, space="DRAM") as dr:

            _rr = [0]
            _dma_engs = [nc.sync, nc.gpsimd, nc.scalar]

            def dma_rr(out, in_):
                eng = _dma_engs[_rr[0] % len(_dma_engs)]
                _rr[0] += 1
                eng.dma_start(out=out, in_=in_)

            def sb_load(name, shape, tag=None, bufs=1, dt=f32):
                t = wp.tile(list(shape), dt, name=f"{name}_sb",
                            tag=tag or f"{name}_t", bufs=bufs)
                nc.sync.dma_start(out=t, in_=dram[name].ap())
                return t

            def load_bias(name, M):
                mt = _ceil_div(M, 128)
                assert M % 128 == 0
                t = wp.tile([128, mt], f32, name=f"{name}_sb",
                            tag=f"{name}_b", bufs=1)
                dma_rr(t, dram[name].ap())
                return t

            def softsign_from_psum(name, ps, bias_ap, w):
                val = tp.tile([128, w], f32, name=f"{name}_v",
                              tag=f"ss_v{w}", bufs=2)
                den = tp.tile([128, w], f32, name=f"{name}_d",
                              tag=f"ss_d{w}", bufs=2)
                if bias_ap is None:
                    nc.scalar.activation(val, ps, AF.Identity)
                    nc.scalar.activation(den, ps, AF.Abs)
                else:
                    nc.scalar.activation(val, ps, AF.Identity, bias=bias_ap)
                    nc.scalar.activation(den, ps, AF.Abs, bias=bias_ap)
                nc.vector.tensor_scalar_add(den, den, 1.0)
                nc.vector.reciprocal(den, den)
                nc.vector.tensor_mul(val, val, den)
                return val

            # ---- persistent SBUF loads ----
            encT_sb = []
            for k in range(4):
                t = wp.tile([128, NA], f32, name=f"encT{k}_sb", tag="encT",
                            bufs=4)
                dma_rr(t, dram["encT"].ap()[k * 128:(k + 1) * 128, :])
                encT_sb.append(t)
            spkrT_sb = sb_load("spkrT", (SPK, N))
            speedT_sb = sb_load("speedT", (1, N))
            pinT_sb0 = wp.tile([128, N], f32r, name="pinT0", bufs=1)
            nc.sync.dma_start(out=pinT_sb0, in_=dram["pinT"].ap()[0:128, :])
            pinT_sb1 = wp.tile([16, N], f32r, name="pinT1", bufs=1)
            nc.sync.dma_start(out=pinT_sb1, in_=dram["pinT"].ap()[128:144, :])
            maskneg_sb = sb_load("maskneg", (1, NA))
            conv_sb = []
            for k in range(2):
                t = wp.tile([128, AW], f32, name=f"conv{k}_sb", tag="convT",
                            bufs=2)
                nc.sync.dma_start(out=t,
                                  in_=dram["convT"].ap()[k * 128:(k + 1) * 128, :])
                conv_sb.append(t)
            ident_sb = sb_load("ident", (128, 128))
            apb_sb = sb_load("apb", (1, 1))

            enc_b_sb = load_bias("enc_b", ATT)
            ap_sb = load_bias("apT", ATT)
            spd1b_sb = load_bias("spd1b", DEC)
            spd2b_sb = load_bias("spd2b", ENC)
            pre1b_sb = load_bias("pre1b", 2 * DEC)
            pre2b_sb = load_bias("pre2b", DEC)
            b0_sb = load_bias("b0", GS)
            b1_sb = load_bias("b1", G3)
            ol1b_sb = load_bias("ol1b", DEC)
            lb = {}
            for l in range(2):
                for nm in (f"bv{l}", f"bo{l}", f"ln1g{l}", f"ln1b{l}",
                           f"bf1{l}", f"bf2{l}", f"ln2g{l}", f"ln2b{l}"):
                    M = FF if nm.startswith("bf1") else DEC
                    lb[nm] = load_bias(nm, M)

            ones_col = wp.tile([128, 1], f32, name="ones_col", bufs=1)
            nc.vector.memset(ones_col, 1.0)
            ones_col_r = wp.tile([128, 1], f32r, name="ones_col_r", bufs=1)
            nc.vector.tensor_copy(ones_col_r, ones_col)
            ones_row = wp.tile([1, 128], f32, name="ones_row", bufs=1)
            nc.vector.memset(ones_row, 1.0)
            ones_row_r = wp.tile([1, N], f32r, name="ones_row_r", bufs=1)
            nc.vector.tensor_copy(ones_row_r, ones_row[:, :N])
            eps_sb = wp.tile([1, 1], f32, name="eps_sb", bufs=1)
            nc.vector.memset(eps_sb, 1e-5)

            # ================= gemm v2: weights-moving fp32r =================
            def _kt_runs(kt):
                """Maximal runs (cap 4) of consecutive full-128 k-tiles;
                <128 tiles become singleton runs."""
                runs = []
                i = 0
                while i < len(kt):
                    if kt[i][1] == 128:
                        j = i
                        while j < len(kt) and kt[j][1] == 128 and j - i < 4:
                            j += 1
                        runs.append((i, j - i))
                        i = j
                    else:
                        runs.append((i, 1))
                        i += 1
                return runs

            def gemm2(name, w_name, K, M, lhsT_tiles, evac_t, wbufs=4,
                      split_k=None):
                """y[32, M] = x @ W^T streamed in <=CG-col chunks; each
                128-col output tile is PE-transposed back to [128, 32] and
                handed to evac_t(m, psum_t) for fused bias+activation
                evacuation. One weight DMA per (chunk, k-tile-run)."""
                w_dram = dram[w_name]
                kt, ksum = _kt_of(lhsT_tiles)
                assert ksum == K, (name, ksum, K)
                nkt = len(kt)
                widths = _chunks_of(M)
                runs = _kt_runs(kt)

                def load_run(c, wc, i0, cnt):
                    kk, ks0 = kt[i0]
                    w_sb = wp.tile([ks0, cnt * wc], f32r,
                                   name=f"{name}_w{i0}_{c}",
                                   tag="w2" if ks0 == 128 else "w2r",
                                   bufs=wbufs, padded_shape=[128, 4 * CG])
                    rows = w_dram.ap()[c * K + kk:c * K + kk + cnt * ks0, :wc]
                    if cnt == 1:
                        dma_rr(w_sb, rows)
                    else:
                        dma_rr(w_sb.rearrange("p (t j) -> p t j", j=wc),
                               rows.rearrange("(t p) j -> p t j", p=128))
                    return w_sb

                def mm_runs(ps, c, wc, sel_runs):
                    for i0, cnt in sel_runs:
                        w_sb = load_run(c, wc, i0, cnt)
                        for idx in range(cnt):
                            ki = i0 + idx
                            nc.tensor.matmul(
                                ps, lhsT_tiles[ki],
                                w_sb[:, idx * wc:(idx + 1) * wc],
                                start=(ki == 0), stop=(ki == nkt - 1),
                                skip_group_check=True)

                m = 0
                if split_k is None:
                    for c, wc in enumerate(widths):
                        ps = pp.tile([N, wc], f32, name=f"{name}_ps{c}",
                                     tag="ps_mm", bufs=2,
                                     padded_shape=[N, CG])
                        mm_runs(ps, c, wc, runs)
                        _gemm2_evac(name, c, wc, m, ps, evac_t)
                        m += wc // 128
                else:
                    runs_a = [r for r in runs if r[0] < split_k]
                    runs_b = [r for r in runs if r[0] >= split_k]
                    assert sum(cnt for _, cnt in runs_a) == split_k
                    pss = {}
                    for c, wc in enumerate(widths):
                        pss[c] = pp.tile([N, wc], f32, name=f"{name}_ps{c}",
                                         tag="ps_mm", bufs=2,
                                         padded_shape=[N, CG])
                    for c, wc in enumerate(widths):
                        mm_runs(pss[c], c, wc, runs_a)
                    for c, wc in enumerate(widths):
                        mm_runs(pss[c], c, wc, runs_b)
                    for c, wc in enumerate(widths):
                        _gemm2_evac(name, c, wc, m, pss[c], evac_t)
                        m += wc // 128

            def _gemm2_evac(name, c, wc, m0, ps, evac_t):
                y32 = tp.tile([N, wc], f32, name=f"{name}_y{c}", tag="y32",
                              bufs=3, padded_shape=[N, CG])
                nc.vector.tensor_copy(y32, ps)
                for j in range(wc // 128):
                    m = m0 + j
                    pst = pp.tile([128, N], f32, name=f"{name}_pt{m}",
                                  tag="ps_tr2", bufs=1,
                                  padded_shape=[128, 128])
                    nc.tensor.transpose(pst, y32[:, j * 128:(j + 1) * 128],
                                        ident_sb[:N, :N])
                    evac_t(m, pst)

            def act_evac(out_list, tag, bufs, func, bias_sb, dt=f32r):
                def _e(m, pst):
                    t = tp.tile([128, N], dt, name=f"{tag}_{m}", tag=tag,
                                bufs=bufs)
                    f = func(m) if callable(func) else func
                    nc.scalar.activation(t, pst, f, bias=bias_sb[:, m:m + 1])
                    out_list[m] = t
                return _e

            # =========== attention (exact fp32, weights-stationary) ==========
            spkr_w_sb = sb_load("spkr_wT", (SPK, ATT))
            satt_sb = sb_load("sattT", (1, ATT))
            c1 = []
            for m in range(2):
                ps = pp.tile([128, N], f32, name=f"spk_ps{m}", tag="ps_main",
                             bufs=2, padded_shape=[128, N])
                nc.tensor.matmul(ps, spkr_w_sb[:, m * 128:(m + 1) * 128],
                                 spkrT_sb, start=True, stop=True)
                s_spkr = softsign_from_psum(f"sspkr{m}", ps, None, N)
                ps2 = pp.tile([128, N], f32, name=f"sat_ps{m}", tag="ps_main",
                              bufs=2, padded_shape=[128, N])
                nc.tensor.matmul(ps2, satt_sb[:, m * 128:(m + 1) * 128],
                                 speedT_sb, start=True, stop=True)
                cm = tp.tile([128, N], f32, name=f"c1_{m}", tag="c1", bufs=2)
                nc.vector.tensor_add(cm, s_spkr, ps2)
                c1.append(cm)

            enc_w_sb = []
            for k in range(4):
                t = wp.tile([128, ATT], f32, name=f"encw{k}", tag="enc_wT",
                            bufs=4)
                dma_rr(t, dram["enc_wT"].ap()[k * 128:(k + 1) * 128, :])
                enc_w_sb.append(t)
            th = []
            for m in range(2):
                ps = pp.tile([128, NA], f32, name=f"eatt_ps{m}", tag="ps_misc",
                             bufs=3, padded_shape=[128, NA])
                for k in range(4):
                    nc.tensor.matmul(ps, enc_w_sb[k][:, m * 128:(m + 1) * 128],
                                     encT_sb[k], start=(k == 0), stop=(k == 3))
                e = softsign_from_psum(f"senc{m}", ps, enc_b_sb[:, m:m + 1], NA)
                ev = e.rearrange("p (n t) -> p n t", t=AW)
                nc.vector.tensor_add(
                    ev, ev, c1[m].unsqueeze(2).broadcast_to([128, N, AW]))
                nc.vector.tensor_add(
                    ev, ev, conv_sb[m].unsqueeze(1).broadcast_to([128, N, AW]))
                tm = tp.tile([128, NA], f32, name=f"th{m}", tag="th", bufs=2)
                nc.scalar.activation(tm, e, AF.Tanh)
                th.append(tm)

            ps_l = pp.tile([1, NA], f32, name="log_ps", tag="ps_misc", bufs=3,
                           padded_shape=[128, NA])
            for m in range(2):
                nc.tensor.matmul(ps_l, ap_sb[:, m:m + 1], th[m],
                                 start=(m == 0), stop=(m == 1))
            logit = tp.tile([1, NA], f32, name="logit", bufs=1)
            nc.scalar.activation(logit, ps_l, AF.Identity, bias=apb_sb)
            nc.vector.tensor_add(logit, logit, maskneg_sb)
            lv = logit.rearrange("p (n t) -> p n t", t=AW)
            mx = tp.tile([1, N], f32, name="mx", tag="sm32", bufs=10)
            nc.vector.tensor_reduce(mx, lv, axis=AX.X, op=ALU.max)
            nc.vector.tensor_sub(lv, lv,
                                 mx.unsqueeze(2).broadcast_to([1, N, AW]))
            wexp = tp.tile([1, NA], f32, name="wexp", bufs=1)
            nc.scalar.activation(wexp, logit, AF.Exp)
            wv = wexp.rearrange("p (n t) -> p n t", t=AW)
            ssum = tp.tile([1, N], f32, name="ssum", tag="sm32", bufs=10)
            nc.vector.tensor_reduce(ssum, wv, axis=AX.X, op=ALU.add)
            nc.vector.tensor_scalar_max(ssum, ssum, 1e-12)
            rsum = tp.tile([1, N], f32, name="rsum", tag="sm32", bufs=10)
            nc.vector.reciprocal(rsum, ssum)
            nc.vector.tensor_mul(wv, wv,
                                 rsum.unsqueeze(2).broadcast_to([1, N, AW]))

            wb_ps = pp.tile([128, NA], f32, name="wb_ps", tag="ps_misc",
                            bufs=3, padded_shape=[128, NA])
            nc.tensor.matmul(wb_ps, ones_row, wexp, start=True, stop=True)
            sb_sum = tp.tile([128, N], f32, name="sb_sum", bufs=1)
            nc.vector.tensor_reduce(sb_sum,
                                    wb_ps.rearrange("p (n t) -> p n t", t=AW),
                                    axis=AX.X, op=ALU.add)

            # ---- speed projection ----
            spd1_sb = sb_load("spd1T", (1, DEC))
            r1T = [None] * 4
            for m in range(4):
                ps = pp.tile([128, N], f32, name=f"sp1_ps{m}", tag="ps_main",
                             bufs=2, padded_shape=[128, N])
                nc.tensor.matmul(ps, spd1_sb[:, m * 128:(m + 1) * 128],
                                 speedT_sb, start=True, stop=True)
                t = tp.tile([128, N], f32r, name=f"r1T{m}", tag="r1T", bufs=4)
                nc.scalar.activation(t, ps, AF.Relu, bias=spd1b_sb[:, m:m + 1])
                r1T[m] = t
            spT = [None] * 4
            gemm2("spd2", "spd2T", DEC, ENC, r1T,
                  act_evac(spT, "spT", 4, AF.Tanh, spd2b_sb, dt=f32))

            # ---- context ----
            ctxT = []
            ctxTr = []
            for k in range(4):
                prod = tp.tile([128, NA], f32, name=f"cprod{k}", tag="cprod",
                               bufs=2)
                nc.vector.tensor_mul(prod, encT_sb[k], wb_ps)
                a = tp.tile([128, N], f32, name=f"ctxT{k}", tag="ctxT", bufs=4)
                nc.vector.tensor_reduce(
                    a, prod.rearrange("p (n t) -> p n t", t=AW),
                    axis=AX.X, op=ALU.add)
                bt = tp.tile([128, N], f32, name=f"cb{k}", tag="cb", bufs=2)
                nc.vector.tensor_mul(bt, spT[k], sb_sum)
                nc.vector.tensor_add(a, a, bt)
                ctxT.append(a)
                ar = tp.tile([128, N], f32r, name=f"ctxTr{k}", tag="ctxTr",
                             bufs=4)
                nc.vector.tensor_copy(ar, a)
                ctxTr.append(ar)
            spkrTr = tp.tile([SPK, N], f32r, name="spkrTr", bufs=1)
            nc.vector.tensor_copy(spkrTr, spkrT_sb)

            if phase_limit < 2:
                return None
            # ---- prenet ----
            p1T = [None] * 8
            gemm2("pre1", "pre1T", OUT + SPK, 2 * DEC, [pinT_sb0, pinT_sb1],
                  act_evac(p1T, "p1T", 8, AF.Relu, pre1b_sb))
            preT = [None] * 4
            gemm2("pre2", "pre2T", 2 * DEC, DEC, p1T,
                  act_evac(preT, "preT", 4, AF.Relu, pre2b_sb))

            in_lstm_tiles = preT + ctxTr + [spkrTr]

            if phase_limit < 3:
                return None
            # ---- LSTM layer 0 ----
            l0 = [None] * 6
            gemm2("lstm0", "w0T", ENC + DEC + SPK, GS, in_lstm_tiles,
                  act_evac(l0, "l0", 6,
                           lambda m: AF.Sigmoid if (m < 2 or m >= 4) else AF.Tanh,
                           b0_sb, dt=f32))

            h1T = []
            for k in range(2):
                c = tp.tile([128, N], f32, name=f"c_{k}", tag="cc", bufs=2)
                nc.vector.tensor_mul(c, l0[k], l0[2 + k])
                tc_ = tp.tile([128, N], f32, name=f"tc_{k}", tag="cc", bufs=2)
                nc.scalar.activation(tc_, c, AF.Tanh)
                h = tp.tile([128, N], f32r, name=f"h1T{k}", tag="h1T", bufs=2)
                nc.vector.tensor_mul(h, l0[4 + k], tc_)
                h1T.append(h)

            if phase_limit < 4:
                return None
            # ---- LSTM layer 1 partial gates (batch-on-partition, no bias) ----
            g1p_sb = wp.tile([N, G3], f32, name="g1p_sb", bufs=1)

            def evac_l1_raw(name, c, ps):
                nc.scalar.activation(g1p_sb[:, c * CG:(c + 1) * CG], ps,
                                     AF.Identity)

            # inline gemm2 without transpose for lstm1
            for c in range(G3 // CG):
                w_sb = wp.tile([128, 2 * CG], f32r, name=f"l1_w{c}",
                               tag="w2", bufs=4, padded_shape=[128, 4 * CG])
                rows = dram["w1T"].ap()[c * HS:(c + 1) * HS, :]
                dma_rr(w_sb.rearrange("p (t j) -> p t j", j=CG),
                       rows.rearrange("(t p) j -> p t j", p=128))
                ps = pp.tile([N, CG], f32, name=f"l1_ps{c}", tag="ps_mm",
                             bufs=2, padded_shape=[N, CG])
                for ki in range(2):
                    nc.tensor.matmul(ps, h1T[ki],
                                     w_sb[:, ki * CG:(ki + 1) * CG],
                                     start=(ki == 0), stop=(ki == 1),
                                     skip_group_check=True)
                evac_l1_raw("lstm1", c, ps)

            if phase_limit < 5:
                return None
            # ---- AllReduce partial gates ----
            g1part = dr.tile([N, G3], f32, name="g1part")
            g1full = dr.tile([N, G3], f32, name="g1full", addr_space="Shared")
            nc.sync.dma_start(out=g1part, in_=g1p_sb)
            nc.gpsimd.collective_compute(
                "AllReduce", ALU.add,
                replica_groups=[list(range(NCORES))],
                ins=[g1part], outs=[g1full])
            g1f_sb = wp.tile([N, G3], f32, name="g1f_sb", bufs=1)
            nc.sync.dma_start(out=g1f_sb, in_=g1full)

            # transpose gates to feature-on-partition, add bias, h2
            W = 16 * N  # 512
            igo = []
            for gi in range(3):
                big = tp.tile([128, W], f32, name=f"igo{gi}", tag="hw", bufs=6)
                igo.append(big)
            for m in range(48):
                pst = pp.tile([128, N], f32, name=f"g1t{m}", tag="ps_tr2",
                              bufs=1, padded_shape=[128, 128])
                nc.tensor.transpose(pst, g1f_sb[:, m * 128:(m + 1) * 128],
                                    ident_sb[:N, :N])
                gi, t16 = divmod(m, 16)
                nc.scalar.activation(igo[gi][:, t16 * N:(t16 + 1) * N], pst,
                                     AF.Identity, bias=b1_sb[:, m:m + 1])
            sig_i = tp.tile([128, W], f32, name="sig_i", tag="hw", bufs=6)
            nc.scalar.activation(sig_i, igo[0], AF.Sigmoid)
            tan_g = tp.tile([128, W], f32, name="tan_g", tag="hw", bufs=6)
            nc.scalar.activation(tan_g, igo[1], AF.Tanh)
            nc.vector.tensor_mul(sig_i, sig_i, tan_g)           # c
            nc.scalar.activation(tan_g, sig_i, AF.Tanh)         # tanh(c)
            sig_o = tp.tile([128, W], f32, name="sig_o", tag="hw", bufs=6)
            nc.scalar.activation(sig_o, igo[2], AF.Sigmoid)
            h2_sb = wp.tile([128, W], f32r, name="h2_sb", bufs=1)
            nc.vector.tensor_mul(h2_sb, sig_o, tan_g)

            if phase_limit < 6:
                return None
            # ---- outl1 (in_lstm k-tiles first for AllReduce overlap) ----
            ol1_lhs = in_lstm_tiles + [h2_sb[:, t * N:(t + 1) * N]
                                       for t in range(16)]
            xT = [None] * 4
            gemm2("outl1", "ol1T", H + ENC + DEC + SPK, DEC, ol1_lhs,
                  act_evac(xT, "xT", 14, AF.Identity, ol1b_sb), split_k=9)

            if phase_limit < 7:
                return None

            # ---- transformer ----
            def layer_norm(x_tiles, g_sb, b_sb, nm):
                s_ps = pp.tile([1, N], f32, name=f"{nm}_s", tag="ps_misc",
                               bufs=3, padded_shape=[128, NA])
                for k in range(4):
                    nc.tensor.matmul(s_ps, ones_col_r, x_tiles[k],
                                     start=(k == 0), stop=(k == 3))
                s2_ps = pp.tile([1, N], f32, name=f"{nm}_s2", tag="ps_misc",
                                bufs=3, padded_shape=[128, NA])
                for k in range(4):
                    sq = tp.tile([128, N], f32r, name=f"{nm}_sq{k}", tag="sq",
                                 bufs=2)
                    nc.scalar.activation(sq, x_tiles[k], AF.Square)
                    nc.tensor.matmul(s2_ps, ones_col_r, sq, start=(k == 0),
                                     stop=(k == 3))
                mu = tp.tile([1, N], f32, name=f"{nm}_mu", tag="sm32", bufs=10)
                nc.scalar.activation(mu, s_ps, AF.Identity, scale=1.0 / DEC)
                ms = tp.tile([1, N], f32, name=f"{nm}_ms", tag="sm32", bufs=10)
                nc.scalar.activation(ms, s2_ps, AF.Identity, scale=1.0 / DEC)
                mu2 = tp.tile([1, N], f32, name=f"{nm}_mu2", tag="sm32",
                              bufs=10)
                nc.scalar.activation(mu2, mu, AF.Square)
                var = tp.tile([1, N], f32, name=f"{nm}_var", tag="sm32",
                              bufs=10)
                nc.vector.tensor_sub(var, ms, mu2)
                sd = tp.tile([1, N], f32, name=f"{nm}_sd", tag="sm32", bufs=10)
                nc.scalar.activation(sd, var, AF.Sqrt, bias=eps_sb)
                rstd = tp.tile([1, N], f32, name=f"{nm}_rstd", tag="sm32",
                               bufs=10)
                nc.vector.reciprocal(rstd, sd)
                mub_ps = pp.tile([128, N], f32, name=f"{nm}_mub",
                                 tag="ps_main", bufs=2, padded_shape=[128, N])
                nc.tensor.matmul(mub_ps, ones_row, mu, start=True, stop=True)
                rb_ps = pp.tile([128, N], f32, name=f"{nm}_rb", tag="ps_main",
                                bufs=2, padded_shape=[128, N])
                nc.tensor.matmul(rb_ps, ones_row, rstd, start=True, stop=True)
                out = []
                for k in range(4):
                    xc = tp.tile([128, N], f32, name=f"{nm}_xc{k}", tag="sq2",
                                 bufs=2)
                    nc.vector.tensor_sub(xc, x_tiles[k], mub_ps)
                    nc.vector.tensor_mul(xc, xc, rb_ps)
                    o = tp.tile([128, N], f32r, name=f"{nm}_o{k}", tag="xT",
                                bufs=14)
                    nc.scalar.activation(o, xc, AF.Identity,
                                         bias=b_sb[:, k:k + 1],
                                         scale=g_sb[:, k:k + 1])
                    out.append(o)
                return out

            for l in range(2):
                vT = [None] * 4
                gemm2(f"v{l}", f"wvT{l}", DEC, DEC, xT,
                      act_evac(vT, f"vT{l}", 12, AF.Identity, lb[f"bv{l}"]))

                yT = [None] * 4
                x_res = xT

                def evac_o(m, pst, l=l, x_res=x_res, yT=yT):
                    t = tp.tile([128, N], f32r, name=f"aT{l}_{m}", tag="xT",
                                bufs=14)
                    nc.scalar.activation(t, pst, AF.Identity,
                                         bias=lb[f"bo{l}"][:, m:m + 1])
                    nc.vector.tensor_add(t, t, x_res[m])
                    yT[m] = t

                gemm2(f"o{l}", f"woT{l}", DEC, DEC, vT, evac_o)
                xT = layer_norm(yT, lb[f"ln1g{l}"], lb[f"ln1b{l}"], f"ln1_{l}")

                fT = [None] * 8
                gemm2(f"f1{l}", f"wf1T{l}", DEC, FF, xT,
                      act_evac(fT, f"fT{l}", 8, AF.Relu, lb[f"bf1{l}"]))

                zT = [None] * 4
                x_res2 = xT

                def evac_f2(m, pst, l=l, x_res2=x_res2, zT=zT):
                    t = tp.tile([128, N], f32r, name=f"zT{l}_{m}", tag="xT",
                                bufs=14)
                    nc.scalar.activation(t, pst, AF.Identity,
                                         bias=lb[f"bf2{l}"][:, m:m + 1])
                    nc.vector.tensor_add(t, t, x_res2[m])
                    zT[m] = t

                gemm2(f"f2{l}", f"wf2T{l}", FF, DEC, fT, evac_f2)
                xT = layer_norm(zT, lb[f"ln2g{l}"], lb[f"ln2b{l}"], f"ln2_{l}")

            if phase_limit < 8:
                return None
            # ---- outl2: weights-moving, bias as extra ones-row k-tile ----
            ol2_lhs = xT + [ones_row_r]
            kt2, _ = _kt_of(ol2_lhs)
            ps_o2 = pp.tile([N, 2 * OUT], f32, name="o2_ps", tag="ps_mm",
                            bufs=2, padded_shape=[N, CG])
            for ki, (kk, ks) in enumerate(kt2):
                w_sb = wp.tile([ks, 2 * OUT], f32r, name=f"ol2_w{ki}",
                               tag="ol2_w", bufs=5, padded_shape=[128, 2 * OUT])
                dma_rr(w_sb, dram["ol2T"].ap()[kk:kk + ks, :])
                nc.tensor.matmul(ps_o2, ol2_lhs[ki], w_sb, start=(ki == 0),
                                 stop=(ki == 4), skip_group_check=True)
            out_sb = wp.tile([N, 2 * OUT], f32, name="out_sb", bufs=1)
            nc.scalar.activation(out_sb, ps_o2, AF.Identity)
            nc.sync.dma_start(out=out_d.ap(), in_=out_sb)

            if phase_limit < 9:
                return None
            # ---- ctx output: transpose to batch-major ----
            ctx_sb = wp.tile([N, ENC], f32, name="ctx_sb", bufs=1)
            for k in range(4):
                tpk = pp.tile([N, 128], f32, name=f"tpc{k}", tag="ps_tr2",
                              bufs=1, padded_shape=[128, 128])
                nc.tensor.transpose(tpk, ctxT[k], ident_sb)
                nc.vector.tensor_copy(ctx_sb[:, k * 128:(k + 1) * 128], tpk)
            nc.sync.dma_start(out=ctx_d.ap(), in_=ctx_sb)

    nc.compile()
    return nc


def _chunks_of(M):
    out = []
    g0 = 0
    while g0 < M:
        out.append(min(CG, M - g0))
        g0 += out[-1]
    return out


def _tcols(wT):
    """(K, M) fp32 -> vstacked CG-wide column chunks (remainder zero-padded):
    shape (nch*K, CG); chunk c's block is rows [c*K:(c+1)*K]."""
    K, M = wT.shape
    blocks = []
    g0 = 0
    while g0 < M:
        wc = min(CG, M - g0)
        b = wT[:, g0:g0 + wc]
        if wc < CG:
            b = np.pad(b, ((0, 0), (0, CG - wc)))
        blocks.append(b)
        g0 += wc
    return np.ascontiguousarray(np.vstack(blocks))


def _bias128(b):
    """(M,) -> (128, M//128): col m holds b[m*128:(m+1)*128]."""
    M = b.shape[0]
    return np.ascontiguousarray(b.reshape(M // 128, 128).T)


def prep_inputs(inputs):
    fz = np.float32

    def g(name):
        return np.asarray(inputs[name], fz)

    ie = g("input_enc")
    spkr = g("spkr_vec")[:, 0, :]
    encT = np.ascontiguousarray(
        ie[:, :AW, :].transpose(2, 0, 1).reshape(ENC, NA))
    spkrT = np.ascontiguousarray(spkr.T)
    speedT = np.ascontiguousarray(g("speed").reshape(1, N))
    pinT = np.ascontiguousarray(
        np.concatenate([g("input_dec"), spkr], axis=1).T)
    lens = np.asarray(inputs["lengths_enc"]).astype(np.int64)
    t = np.arange(AW)
    mask = (t[None, :] <= np.minimum(AW - 1, lens[:, None] - 1)).astype(fz)
    maskneg = np.ascontiguousarray(((mask - 1.0) * 1e4).reshape(1, NA))
    convT = np.ascontiguousarray(g("conv_w")[:, 0, 15 - t])

    ol1T_full = g("outl1_w").T  # (3136, 512); rows: [h2 (2048), in_lstm (1088)]
    ol1T = np.concatenate([ol1T_full[H:], ol1T_full[:H]], axis=0)

    base = {
        "encT": encT, "spkrT": spkrT, "speedT": speedT, "pinT": pinT,
        "maskneg": maskneg, "convT": convT,
        "enc_wT": np.ascontiguousarray(g("enc_w").T),
        "enc_b": _bias128(g("enc_b")),
        "spkr_wT": np.ascontiguousarray(g("spkr_w").T),
        "sattT": np.ascontiguousarray(g("speed_att_w").T),
        "apT": _bias128(g("attproj_w").reshape(ATT)),
        "apb": np.ascontiguousarray(g("attproj_b").reshape(1, 1)),
        "spd1T": np.ascontiguousarray(g("spd1_w").T),
        "spd1b": _bias128(g("spd1_b")),
        "spd2T": _tcols(g("spd2_w").T),
        "spd2b": _bias128(g("spd2_b")),
        "pre1T": _tcols(g("pre1_w").T),
        "pre1b": _bias128(g("pre1_b")),
        "pre2T": _tcols(g("pre2_w").T),
        "pre2b": _bias128(g("pre2_b")),
        "ol1T": _tcols(ol1T),
        "ol1b": _bias128(g("outl1_b")),
        "ol2T": np.ascontiguousarray(
            np.concatenate([g("outl2_w").T, g("outl2_b").reshape(1, 2 * OUT)],
                           axis=0)),
        "ident": np.eye(128, dtype=fz),
    }
    for l in range(2):
        base[f"wvT{l}"] = _tcols(g("tr_inproj_w")[l][1024:1536].T)
        base[f"bv{l}"] = _bias128(g("tr_inproj_b")[l][1024:1536])
        base[f"woT{l}"] = _tcols(g("tr_out_w")[l].T)
        base[f"bo{l}"] = _bias128(g("tr_out_b")[l])
        base[f"ln1g{l}"] = _bias128(g("tr_ln1_g")[l])
        base[f"ln1b{l}"] = _bias128(g("tr_ln1_b")[l])
        base[f"wf1T{l}"] = _tcols(g("tr_ff1_w")[l].T)
        base[f"bf1{l}"] = _bias128(g("tr_ff1_b")[l])
        base[f"wf2T{l}"] = _tcols(g("tr_ff2_w")[l].T)
        base[f"bf2{l}"] = _bias128(g("tr_ff2_b")[l])
        base[f"ln2g{l}"] = _bias128(g("tr_ln2_g")[l])
        base[f"ln2b{l}"] = _bias128(g("tr_ln2_b")[l])

    wih0 = g("lstm_wih0")
    b0full = g("lstm_bih0") + g("lstm_bhh0")
    wih1 = g("lstm_wih1")
    b1full = g("lstm_bih1") + g("lstm_bhh1")
    rows_igo = np.concatenate(
        [np.arange(H), 2 * H + np.arange(H), 3 * H + np.arange(H)])
    w1_igo = wih1[rows_igo]
    base["b1"] = _bias128(b1full[rows_igo])
    in_maps = []
    for k in range(NCORES):
        hs = np.arange(k * HS, (k + 1) * HS)
        rows0 = np.concatenate([hs, 2 * H + hs, 3 * H + hs])
        m = dict(base)
        m["w0T"] = _tcols(np.ascontiguousarray(wih0[rows0].T))
        m["b0"] = _bias128(b0full[rows0])
        m["w1T"] = _tcols(
            np.ascontiguousarray(w1_igo[:, k * HS:(k + 1) * HS].T))
        in_maps.append(m)
    return in_maps


_NC = None


def _get_nc():
    global _NC
    if _NC is None:
        _NC = build_program()
    return _NC


_LAST_RESULTS = None


def kernel(**inputs):
    global _LAST_RESULTS
    import os
    nc = _get_nc()
    in_maps = prep_inputs(inputs)
    kw = {}
    if os.environ.get("KERNEL_TRACE"):
        kw["trace"] = True
    res = run_bass_kernel_spmd(nc, in_maps, core_ids=list(range(NCORES)), **kw)
    _LAST_RESULTS = res
    r0 = res.results[0]
    out = np.asarray(r0["out"], np.float32).reshape(N, 2, OUT)
    ctx = np.asarray(r0["ctx"], np.float32).reshape(N, 1, ENC)
    return out, ctx


# revision 26
# speedup vs baseline: 1.5854x; 1.1533x over previous
"""Trainium2 Bass kernel for nn_AttnDecoderRNN3 (sparse_attention).

Strategy (8 NeuronCores):
- Only input_enc[:, :10, :] matters: the attention mask is a fixed 10-wide
  window at t=0 (aw0 is a one-hot at t=0 -> argmax 0) and the softmax
  max-subtraction cancels exactly, so the full-T encoder GEMM is skipped.
- LSTM h0=c0=0 -> whh*/f-gate weights unused. Transformer S=1 -> att == V.
- LSTM layer 0 sharded over the hidden dim (each core computes a 256-wide
  h1 slice for all 32 batch rows); layer 1 sharded over the contraction
  dim (each core's h1 slice x its wih1 column slice -> partial full
  gates), combined with ONE AllReduce; biases added after the AllReduce.
- Everything else is computed redundantly on all cores for all 32 batch
  rows (it is tiny); final outputs are read from core 0.
- Big GEMMs: weights-MOVING through the PE at fp32r (1 col/cycle), with
  the batch (32) as the stationary operand; outputs [32, chunk] are
  PE-transposed back to feature-on-partition with bias+activation fused
  into the ScalarE evacuation. Small GEMMs stay exact fp32 stationary.
"""

import numpy as np

import concourse.bacc as bacc
import concourse.mybir as mybir
import concourse.tile as tile
from concourse.bass_utils import run_bass_kernel_spmd

NCORES = 8
N = 32            # batch
ENC = 512
ATT = 256
DEC = 512
H = 2048          # lstm hidden
SPK = 64
OUT = 80
FF = 1024
AW = 10           # attention window (ATT_RANGE)
HS = H // NCORES  # hidden slice per core = 256
G3 = 3 * H        # i,g,o gates = 6144
GS = 3 * HS       # per-core layer0 gate rows = 768
NA = N * AW       # 320
CG = 512          # gemm2 column-group (chunk) width

f32 = mybir.dt.float32
f32r = mybir.dt.float32r
AF = mybir.ActivationFunctionType
ALU = mybir.AluOpType
AX = mybir.AxisListType


def _ceil_div(a, b):
    return (a + b - 1) // b


def _kt_of(tiles):
    kt = []
    k0 = 0
    for r in tiles:
        ks = r.partition_size()
        kt.append((k0, ks))
        k0 += ks
    return kt, k0


def build_program(phase_limit=99):
    import os
    phase_limit = int(os.environ.get("KPHASE", phase_limit))
    nc = bacc.Bacc("TRN2", target_bir_lowering=False, debug=False,
                   num_devices=NCORES)

    dram = {}

    def din(name, shape, dt=f32):
        dram[name] = nc.dram_tensor(name, list(shape), dt,
                                    kind="ExternalInput")
        return dram[name]

    # ---- inputs ----
    # v2 (tiled, fp32r) weight tensors have shape (M/CG * K, CG)
    def din2(name, K, M):
        din(name, (_ceil_div(M, CG) * K, CG), f32r)

    din("encT", (ENC, NA))
    din("spkrT", (SPK, N))
    din("speedT", (1, N))
    din("pinT", (OUT + SPK, N), f32r)
    din("maskneg", (1, NA))
    din("convT", (ATT, AW))
    din("enc_wT", (ENC, ATT)); din("enc_b", (128, (ATT) // 128))
    din("spkr_wT", (SPK, ATT))
    din("sattT", (1, ATT))
    din("apT", (128, 2)); din("apb", (1, 1))
    din("spd1T", (1, DEC)); din("spd1b", (128, (DEC) // 128))
    din2("spd2T", DEC, ENC); din("spd2b", (128, (ENC) // 128))
    din2("pre1T", OUT + SPK, 2 * DEC); din("pre1b", (128, (2 * DEC) // 128))
    din2("pre2T", 2 * DEC, DEC); din("pre2b", (128, (DEC) // 128))
    din2("w0T", ENC + DEC + SPK, GS); din("b0", (128, (GS) // 128))
    din2("w1T", HS, G3); din("b1", (128, (G3) // 128))
    din2("ol1T", H + ENC + DEC + SPK, DEC); din("ol1b", (128, (DEC) // 128))
    for l in range(2):
        din2(f"wvT{l}", DEC, DEC); din(f"bv{l}", (128, 4))
        din(f"ln1g{l}", (128, 4)); din(f"ln1b{l}", (128, 4))
        din2(f"wf1T{l}", DEC, FF); din(f"bf1{l}", (128, 8))
        din2(f"wf2T{l}", FF, DEC); din(f"bf2{l}", (128, 4))
        din(f"ln2g{l}", (128, 4)); din(f"ln2b{l}", (128, 4))
    din("ol2T", (DEC + 1, 2 * OUT), f32r)   # bias folded in as last row
    din("ident", (128, 128))
    out_d = nc.dram_tensor("out", [N, 2 * OUT], f32, kind="ExternalOutput")
    ctx_d = nc.dram_tensor("ctx", [N, ENC], f32, kind="ExternalOutput")

    with tile.TileContext(nc) as tc:
        with tc.tile_pool(name="wp", bufs=1) as wp, \
             tc.tile_pool(name="tp", bufs=2) as tp, \
             tc.tile_pool(name="pp", bufs=1, space="PSUM") as pp, \
             tc.tile_pool(name="dr", bufs=1, space="DRAM") as dr:

            _rr = [0]
            _dma_engs = [nc.sync, nc.gpsimd, nc.scalar]

            def dma_rr(out, in_):
                eng = _dma_engs[_rr[0] % len(_dma_engs)]
                _rr[0] += 1
                eng.dma_start(out=out, in_=in_)

            def sb_load(name, shape, tag=None, bufs=1, dt=f32):
                t = wp.tile(list(shape), dt, name=f"{name}_sb",
                            tag=tag or f"{name}_t", bufs=bufs)
                nc.sync.dma_start(out=t, in_=dram[name].ap())
                return t

            def load_bias(name, M):
                mt = _ceil_div(M, 128)
                assert M % 128 == 0
                t = wp.tile([128, mt], f32, name=f"{name}_sb",
                            tag=f"{name}_b", bufs=1)
                dma_rr(t, dram[name].ap())
                return t

            def softsign_from_psum(name, ps, bias_ap, w):
                val = tp.tile([128, w], f32, name=f"{name}_v",
                              tag=f"ss_v{w}", bufs=2)
                den = tp.tile([128, w], f32, name=f"{name}_d",
                              tag=f"ss_d{w}", bufs=2)
                if bias_ap is None:
                    nc.scalar.activation(val, ps, AF.Identity)
                    nc.scalar.activation(den, ps, AF.Abs)
                else:
                    nc.scalar.activation(val, ps, AF.Identity, bias=bias_ap)
                    nc.scalar.activation(den, ps, AF.Abs, bias=bias_ap)
                nc.vector.tensor_scalar_add(den, den, 1.0)
                nc.vector.reciprocal(den, den)
                nc.vector.tensor_mul(val, val, den)
                return val

            # ---- persistent SBUF loads ----
            encT_sb = []
            for k in range(4):
                t = wp.tile([128, NA], f32, name=f"encT{k}_sb", tag="encT",
                            bufs=4)
                dma_rr(t, dram["encT"].ap()[k * 128:(k + 1) * 128, :])
                encT_sb.append(t)
            spkrT_sb = sb_load("spkrT", (SPK, N))
            speedT_sb = sb_load("speedT", (1, N))
            pinT_sb0 = wp.tile([128, N], f32r, name="pinT0", bufs=1)
            nc.sync.dma_start(out=pinT_sb0, in_=dram["pinT"].ap()[0:128, :])
            pinT_sb1 = wp.tile([16, N], f32r, name="pinT1", bufs=1)
            nc.sync.dma_start(out=pinT_sb1, in_=dram["pinT"].ap()[128:144, :])
            maskneg_sb = sb_load("maskneg", (1, NA))
            conv_sb = []
            for k in range(2):
                t = wp.tile([128, AW], f32, name=f"conv{k}_sb", tag="convT",
                            bufs=2)
                nc.sync.dma_start(out=t,
                                  in_=dram["convT"].ap()[k * 128:(k + 1) * 128, :])
                conv_sb.append(t)
            ident_sb = sb_load("ident", (128, 128))
            apb_sb = sb_load("apb", (1, 1))

            enc_b_sb = load_bias("enc_b", ATT)
            ap_sb = load_bias("apT", ATT)
            spd1b_sb = load_bias("spd1b", DEC)
            spd2b_sb = load_bias("spd2b", ENC)
            pre1b_sb = load_bias("pre1b", 2 * DEC)
            pre2b_sb = load_bias("pre2b", DEC)
            b0_sb = load_bias("b0", GS)
            b1_sb = load_bias("b1", G3)
            ol1b_sb = load_bias("ol1b", DEC)
            lb = {}
            for l in range(2):
                for nm in (f"bv{l}", f"ln1g{l}", f"ln1b{l}",
                           f"bf1{l}", f"bf2{l}", f"ln2g{l}", f"ln2b{l}"):
                    M = FF if nm.startswith("bf1") else DEC
                    lb[nm] = load_bias(nm, M)

            ones_col = wp.tile([128, 1], f32, name="ones_col", bufs=1)
            nc.vector.memset(ones_col, 1.0)
            ones_col_r = wp.tile([128, 1], f32r, name="ones_col_r", bufs=1)
            nc.vector.tensor_copy(ones_col_r, ones_col)
            ones_row = wp.tile([1, 128], f32, name="ones_row", bufs=1)
            nc.vector.memset(ones_row, 1.0)
            ones_row_r = wp.tile([1, N], f32r, name="ones_row_r", bufs=1)
            nc.vector.tensor_copy(ones_row_r, ones_row[:, :N])
            eps_sb = wp.tile([1, 1], f32, name="eps_sb", bufs=1)
            nc.vector.memset(eps_sb, 1e-5)

            # ================= gemm v2: weights-moving fp32r =================
            def _kt_runs(kt):
                """Maximal runs (cap 4) of consecutive full-128 k-tiles;
                <128 tiles become singleton runs."""
                runs = []
                i = 0
                while i < len(kt):
                    if kt[i][1] == 128:
                        j = i
                        while j < len(kt) and kt[j][1] == 128 and j - i < 4:
                            j += 1
                        runs.append((i, j - i))
                        i = j
                    else:
                        runs.append((i, 1))
                        i += 1
                return runs

            def gemm2(name, w_name, K, M, lhsT_tiles, evac_t, wbufs=4,
                      split_k=None):
                """y[32, M] = x @ W^T streamed in <=CG-col chunks; each
                128-col output tile is PE-transposed back to [128, 32] and
                handed to evac_t(m, psum_t) for fused bias+activation
                evacuation. One weight DMA per (chunk, k-tile-run)."""
                w_dram = dram[w_name]
                kt, ksum = _kt_of(lhsT_tiles)
                assert ksum == K, (name, ksum, K)
                nkt = len(kt)
                widths = _chunks_of(M)
                runs = _kt_runs(kt)

                def load_run(c, wc, i0, cnt):
                    kk, ks0 = kt[i0]
                    w_sb = wp.tile([ks0, cnt * wc], f32r,
                                   name=f"{name}_w{i0}_{c}",
                                   tag="w2" if ks0 == 128 else "w2r",
                                   bufs=wbufs, padded_shape=[128, 4 * CG])
                    rows = w_dram.ap()[c * K + kk:c * K + kk + cnt * ks0, :wc]
                    if cnt == 1:
                        dma_rr(w_sb, rows)
                    else:
                        dma_rr(w_sb.rearrange("p (t j) -> p t j", j=wc),
                               rows.rearrange("(t p) j -> p t j", p=128))
                    return w_sb

                def mm_runs(ps, c, wc, sel_runs):
                    for i0, cnt in sel_runs:
                        w_sb = load_run(c, wc, i0, cnt)
                        for idx in range(cnt):
                            ki = i0 + idx
                            nc.tensor.matmul(
                                ps, lhsT_tiles[ki],
                                w_sb[:, idx * wc:(idx + 1) * wc],
                                start=(ki == 0), stop=(ki == nkt - 1),
                                skip_group_check=True)

                m = 0
                if split_k is None:
                    for c, wc in enumerate(widths):
                        ps = pp.tile([N, wc], f32, name=f"{name}_ps{c}",
                                     tag="ps_mm", bufs=2,
                                     padded_shape=[N, CG])
                        mm_runs(ps, c, wc, runs)
                        _gemm2_evac(name, c, wc, m, ps, evac_t)
                        m += wc // 128
                else:
                    runs_a = [r for r in runs if r[0] < split_k]
                    runs_b = [r for r in runs if r[0] >= split_k]
                    assert sum(cnt for _, cnt in runs_a) == split_k
                    pss = {}
                    for c, wc in enumerate(widths):
                        pss[c] = pp.tile([N, wc], f32, name=f"{name}_ps{c}",
                                         tag="ps_mm", bufs=2,
                                         padded_shape=[N, CG])
                    for c, wc in enumerate(widths):
                        mm_runs(pss[c], c, wc, runs_a)
                    for c, wc in enumerate(widths):
                        mm_runs(pss[c], c, wc, runs_b)
                    for c, wc in enumerate(widths):
                        _gemm2_evac(name, c, wc, m, pss[c], evac_t)
                        m += wc // 128

            def _gemm2_evac(name, c, wc, m0, ps, evac_t):
                y32 = tp.tile([N, wc], f32, name=f"{name}_y{c}", tag="y32",
                              bufs=3, padded_shape=[N, CG])
                nc.vector.tensor_copy(y32, ps)
                for j in range(wc // 128):
                    m = m0 + j
                    pst = pp.tile([128, N], f32, name=f"{name}_pt{m}",
                                  tag="ps_tr2", bufs=1,
                                  padded_shape=[128, 128])
                    nc.tensor.transpose(pst, y32[:, j * 128:(j + 1) * 128],
                                        ident_sb[:N, :N])
                    evac_t(m, pst)

            def act_evac(out_list, tag, bufs, func, bias_sb, dt=f32r):
                def _e(m, pst):
                    t = tp.tile([128, N], dt, name=f"{tag}_{m}", tag=tag,
                                bufs=bufs)
                    f = func(m) if callable(func) else func
                    nc.scalar.activation(t, pst, f, bias=bias_sb[:, m:m + 1])
                    out_list[m] = t
                return _e

            # =========== attention (exact fp32, weights-stationary) ==========
            spkr_w_sb = sb_load("spkr_wT", (SPK, ATT))
            satt_sb = sb_load("sattT", (1, ATT))
            c1 = []
            for m in range(2):
                ps = pp.tile([128, N], f32, name=f"spk_ps{m}", tag="ps_main",
                             bufs=2, padded_shape=[128, N])
                nc.tensor.matmul(ps, spkr_w_sb[:, m * 128:(m + 1) * 128],
                                 spkrT_sb, start=True, stop=True)
                s_spkr = softsign_from_psum(f"sspkr{m}", ps, None, N)
                ps2 = pp.tile([128, N], f32, name=f"sat_ps{m}", tag="ps_main",
                              bufs=2, padded_shape=[128, N])
                nc.tensor.matmul(ps2, satt_sb[:, m * 128:(m + 1) * 128],
                                 speedT_sb, start=True, stop=True)
                cm = tp.tile([128, N], f32, name=f"c1_{m}", tag="c1", bufs=2)
                nc.vector.tensor_add(cm, s_spkr, ps2)
                c1.append(cm)

            enc_w_sb = []
            for k in range(4):
                t = wp.tile([128, ATT], f32, name=f"encw{k}", tag="enc_wT",
                            bufs=4)
                dma_rr(t, dram["enc_wT"].ap()[k * 128:(k + 1) * 128, :])
                enc_w_sb.append(t)
            th = []
            for m in range(2):
                ps = pp.tile([128, NA], f32, name=f"eatt_ps{m}", tag="ps_misc",
                             bufs=3, padded_shape=[128, NA])
                for k in range(4):
                    nc.tensor.matmul(ps, enc_w_sb[k][:, m * 128:(m + 1) * 128],
                                     encT_sb[k], start=(k == 0), stop=(k == 3))
                e = softsign_from_psum(f"senc{m}", ps, enc_b_sb[:, m:m + 1], NA)
                ev = e.rearrange("p (n t) -> p n t", t=AW)
                nc.vector.tensor_add(
                    ev, ev, c1[m].unsqueeze(2).broadcast_to([128, N, AW]))
                nc.vector.tensor_add(
                    ev, ev, conv_sb[m].unsqueeze(1).broadcast_to([128, N, AW]))
                tm = tp.tile([128, NA], f32, name=f"th{m}", tag="th", bufs=2)
                nc.scalar.activation(tm, e, AF.Tanh)
                th.append(tm)

            ps_l = pp.tile([1, NA], f32, name="log_ps", tag="ps_misc", bufs=3,
                           padded_shape=[128, NA])
            for m in range(2):
                nc.tensor.matmul(ps_l, ap_sb[:, m:m + 1], th[m],
                                 start=(m == 0), stop=(m == 1))
            logit = tp.tile([1, NA], f32, name="logit", bufs=1)
            nc.scalar.activation(logit, ps_l, AF.Identity, bias=apb_sb)
            nc.vector.tensor_add(logit, logit, maskneg_sb)
            lv = logit.rearrange("p (n t) -> p n t", t=AW)
            mx = tp.tile([1, N], f32, name="mx", tag="sm32", bufs=10)
            nc.vector.tensor_reduce(mx, lv, axis=AX.X, op=ALU.max)
            nc.vector.tensor_sub(lv, lv,
                                 mx.unsqueeze(2).broadcast_to([1, N, AW]))
            wexp = tp.tile([1, NA], f32, name="wexp", bufs=1)
            nc.scalar.activation(wexp, logit, AF.Exp)
            wv = wexp.rearrange("p (n t) -> p n t", t=AW)
            ssum = tp.tile([1, N], f32, name="ssum", tag="sm32", bufs=10)
            nc.vector.tensor_reduce(ssum, wv, axis=AX.X, op=ALU.add)
            nc.vector.tensor_scalar_max(ssum, ssum, 1e-12)
            rsum = tp.tile([1, N], f32, name="rsum", tag="sm32", bufs=10)
            nc.vector.reciprocal(rsum, ssum)
            nc.vector.tensor_mul(wv, wv,
                                 rsum.unsqueeze(2).broadcast_to([1, N, AW]))

            wb_ps = pp.tile([128, NA], f32, name="wb_ps", tag="ps_misc",
                            bufs=3, padded_shape=[128, NA])
            nc.tensor.matmul(wb_ps, ones_row, wexp, start=True, stop=True)
            sb_sum = tp.tile([128, N], f32, name="sb_sum", bufs=1)
            nc.vector.tensor_reduce(sb_sum,
                                    wb_ps.rearrange("p (n t) -> p n t", t=AW),
                                    axis=AX.X, op=ALU.add)

            # ---- speed projection ----
            spd1_sb = sb_load("spd1T", (1, DEC))
            r1T = [None] * 4
            for m in range(4):
                ps = pp.tile([128, N], f32, name=f"sp1_ps{m}", tag="ps_main",
                             bufs=2, padded_shape=[128, N])
                nc.tensor.matmul(ps, spd1_sb[:, m * 128:(m + 1) * 128],
                                 speedT_sb, start=True, stop=True)
                t = tp.tile([128, N], f32r, name=f"r1T{m}", tag="r1T", bufs=4)
                nc.scalar.activation(t, ps, AF.Relu, bias=spd1b_sb[:, m:m + 1])
                r1T[m] = t
            spT = [None] * 4
            gemm2("spd2", "spd2T", DEC, ENC, r1T,
                  act_evac(spT, "spT", 4, AF.Tanh, spd2b_sb, dt=f32))

            # ---- context ----
            ctxT = []
            ctxTr = []
            for k in range(4):
                prod = tp.tile([128, NA], f32, name=f"cprod{k}", tag="cprod",
                               bufs=2)
                nc.vector.tensor_mul(prod, encT_sb[k], wb_ps)
                a = tp.tile([128, N], f32, name=f"ctxT{k}", tag="ctxT", bufs=4)
                nc.vector.tensor_reduce(
                    a, prod.rearrange("p (n t) -> p n t", t=AW),
                    axis=AX.X, op=ALU.add)
                bt = tp.tile([128, N], f32, name=f"cb{k}", tag="cb", bufs=2)
                nc.vector.tensor_mul(bt, spT[k], sb_sum)
                nc.vector.tensor_add(a, a, bt)
                ctxT.append(a)
                ar = tp.tile([128, N], f32r, name=f"ctxTr{k}", tag="ctxTr",
                             bufs=4)
                nc.vector.tensor_copy(ar, a)
                ctxTr.append(ar)
            spkrTr = tp.tile([SPK, N], f32r, name="spkrTr", bufs=1)
            nc.vector.tensor_copy(spkrTr, spkrT_sb)

            if phase_limit < 2:
                return None
            # ---- prenet ----
            p1T = [None] * 8
            gemm2("pre1", "pre1T", OUT + SPK, 2 * DEC, [pinT_sb0, pinT_sb1],
                  act_evac(p1T, "p1T", 8, AF.Relu, pre1b_sb))
            preT = [None] * 4
            gemm2("pre2", "pre2T", 2 * DEC, DEC, p1T,
                  act_evac(preT, "preT", 4, AF.Relu, pre2b_sb))

            in_lstm_tiles = preT + ctxTr + [spkrTr]

            if phase_limit < 3:
                return None
            # ---- LSTM layer 0 ----
            l0 = [None] * 6
            gemm2("lstm0", "w0T", ENC + DEC + SPK, GS, in_lstm_tiles,
                  act_evac(l0, "l0", 6,
                           lambda m: AF.Sigmoid if (m < 2 or m >= 4) else AF.Tanh,
                           b0_sb, dt=f32))

            h1T = []
            for k in range(2):
                c = tp.tile([128, N], f32, name=f"c_{k}", tag="cc", bufs=2)
                nc.vector.tensor_mul(c, l0[k], l0[2 + k])
                tc_ = tp.tile([128, N], f32, name=f"tc_{k}", tag="cc", bufs=2)
                nc.scalar.activation(tc_, c, AF.Tanh)
                h = tp.tile([128, N], f32r, name=f"h1T{k}", tag="h1T", bufs=2)
                nc.vector.tensor_mul(h, l0[4 + k], tc_)
                h1T.append(h)

            if phase_limit < 4:
                return None
            # ---- LSTM layer 1 partial gates (batch-on-partition, no bias) ----
            g1p_sb = wp.tile([N, G3], f32, name="g1p_sb", bufs=1)

            def evac_l1_raw(name, c, ps):
                nc.scalar.activation(g1p_sb[:, c * CG:(c + 1) * CG], ps,
                                     AF.Identity)

            # inline gemm2 without transpose for lstm1
            for c in range(G3 // CG):
                w_sb = wp.tile([128, 2 * CG], f32r, name=f"l1_w{c}",
                               tag="w2", bufs=4, padded_shape=[128, 4 * CG])
                rows = dram["w1T"].ap()[c * HS:(c + 1) * HS, :]
                dma_rr(w_sb.rearrange("p (t j) -> p t j", j=CG),
                       rows.rearrange("(t p) j -> p t j", p=128))
                ps = pp.tile([N, CG], f32, name=f"l1_ps{c}", tag="ps_mm",
                             bufs=2, padded_shape=[N, CG])
                for ki in range(2):
                    nc.tensor.matmul(ps, h1T[ki],
                                     w_sb[:, ki * CG:(ki + 1) * CG],
                                     start=(ki == 0), stop=(ki == 1),
                                     skip_group_check=True)
                evac_l1_raw("lstm1", c, ps)

            if phase_limit < 5:
                return None
            # ---- AllReduce partial gates ----
            g1part = dr.tile([N, G3], f32, name="g1part")
            g1full = dr.tile([N, G3], f32, name="g1full", addr_space="Shared")
            nc.sync.dma_start(out=g1part, in_=g1p_sb)
            nc.gpsimd.collective_compute(
                "AllReduce", ALU.add,
                replica_groups=[list(range(NCORES))],
                ins=[g1part], outs=[g1full])
            g1f_sb = wp.tile([N, G3], f32, name="g1f_sb", bufs=1)
            nc.sync.dma_start(out=g1f_sb, in_=g1full)

            # transpose gates to feature-on-partition, add bias, h2
            W = 16 * N  # 512
            igo = []
            for gi in range(3):
                big = tp.tile([128, W], f32, name=f"igo{gi}", tag="hw", bufs=6)
                igo.append(big)
            for m in range(48):
                pst = pp.tile([128, N], f32, name=f"g1t{m}", tag="ps_tr2",
                              bufs=1, padded_shape=[128, 128])
                nc.tensor.transpose(pst, g1f_sb[:, m * 128:(m + 1) * 128],
                                    ident_sb[:N, :N])
                gi, t16 = divmod(m, 16)
                nc.scalar.activation(igo[gi][:, t16 * N:(t16 + 1) * N], pst,
                                     AF.Identity, bias=b1_sb[:, m:m + 1])
            sig_i = tp.tile([128, W], f32, name="sig_i", tag="hw", bufs=6)
            nc.scalar.activation(sig_i, igo[0], AF.Sigmoid)
            tan_g = tp.tile([128, W], f32, name="tan_g", tag="hw", bufs=6)
            nc.scalar.activation(tan_g, igo[1], AF.Tanh)
            nc.vector.tensor_mul(sig_i, sig_i, tan_g)           # c
            nc.scalar.activation(tan_g, sig_i, AF.Tanh)         # tanh(c)
            sig_o = tp.tile([128, W], f32, name="sig_o", tag="hw", bufs=6)
            nc.scalar.activation(sig_o, igo[2], AF.Sigmoid)
            h2_sb = wp.tile([128, W], f32r, name="h2_sb", bufs=1)
            nc.vector.tensor_mul(h2_sb, sig_o, tan_g)

            if phase_limit < 6:
                return None
            # ---- outl1 (in_lstm k-tiles first for AllReduce overlap) ----
            ol1_lhs = in_lstm_tiles + [h2_sb[:, t * N:(t + 1) * N]
                                       for t in range(16)]
            xT = [None] * 4
            gemm2("outl1", "ol1T", H + ENC + DEC + SPK, DEC, ol1_lhs,
                  act_evac(xT, "xT", 14, AF.Identity, ol1b_sb), split_k=9)

            if phase_limit < 7:
                return None

            # ---- transformer ----
            def layer_norm(x_tiles, g_sb, b_sb, nm):
                s_ps = pp.tile([1, N], f32, name=f"{nm}_s", tag="ps_misc",
                               bufs=3, padded_shape=[128, NA])
                for k in range(4):
                    nc.tensor.matmul(s_ps, ones_col_r, x_tiles[k],
                                     start=(k == 0), stop=(k == 3))
                s2_ps = pp.tile([1, N], f32, name=f"{nm}_s2", tag="ps_misc",
                                bufs=3, padded_shape=[128, NA])
                for k in range(4):
                    sq = tp.tile([128, N], f32r, name=f"{nm}_sq{k}", tag="sq",
                                 bufs=2)
                    nc.scalar.activation(sq, x_tiles[k], AF.Square)
                    nc.tensor.matmul(s2_ps, ones_col_r, sq, start=(k == 0),
                                     stop=(k == 3))
                mu = tp.tile([1, N], f32, name=f"{nm}_mu", tag="sm32", bufs=10)
                nc.scalar.activation(mu, s_ps, AF.Identity, scale=1.0 / DEC)
                ms = tp.tile([1, N], f32, name=f"{nm}_ms", tag="sm32", bufs=10)
                nc.scalar.activation(ms, s2_ps, AF.Identity, scale=1.0 / DEC)
                mu2 = tp.tile([1, N], f32, name=f"{nm}_mu2", tag="sm32",
                              bufs=10)
                nc.scalar.activation(mu2, mu, AF.Square)
                var = tp.tile([1, N], f32, name=f"{nm}_var", tag="sm32",
                              bufs=10)
                nc.vector.tensor_sub(var, ms, mu2)
                sd = tp.tile([1, N], f32, name=f"{nm}_sd", tag="sm32", bufs=10)
                nc.scalar.activation(sd, var, AF.Sqrt, bias=eps_sb)
                rstd = tp.tile([1, N], f32, name=f"{nm}_rstd", tag="sm32",
                               bufs=10)
                nc.vector.reciprocal(rstd, sd)
                mub_ps = pp.tile([128, N], f32, name=f"{nm}_mub",
                                 tag="ps_main", bufs=2, padded_shape=[128, N])
                nc.tensor.matmul(mub_ps, ones_row, mu, start=True, stop=True)
                rb_ps = pp.tile([128, N], f32, name=f"{nm}_rb", tag="ps_main",
                                bufs=2, padded_shape=[128, N])
                nc.tensor.matmul(rb_ps, ones_row, rstd, start=True, stop=True)
                out = []
                for k in range(4):
                    xc = tp.tile([128, N], f32, name=f"{nm}_xc{k}", tag="sq2",
                                 bufs=2)
                    nc.vector.tensor_sub(xc, x_tiles[k], mub_ps)
                    nc.vector.tensor_mul(xc, xc, rb_ps)
                    o = tp.tile([128, N], f32r, name=f"{nm}_o{k}", tag="xT",
                                bufs=14)
                    nc.scalar.activation(o, xc, AF.Identity,
                                         bias=b_sb[:, k:k + 1],
                                         scale=g_sb[:, k:k + 1])
                    out.append(o)
                return out

            for l in range(2):
                yT = [None] * 4
                x_res = xT

                def evac_vo(m, pst, l=l, x_res=x_res, yT=yT):
                    t = tp.tile([128, N], f32r, name=f"aT{l}_{m}", tag="xT",
                                bufs=14)
                    nc.scalar.activation(t, pst, AF.Identity,
                                         bias=lb[f"bv{l}"][:, m:m + 1])
                    nc.vector.tensor_add(t, t, x_res[m])
                    yT[m] = t

                gemm2(f"vo{l}", f"wvT{l}", DEC, DEC, xT, evac_vo)

                xT = layer_norm(yT, lb[f"ln1g{l}"], lb[f"ln1b{l}"], f"ln1_{l}")

                fT = [None] * 8
                gemm2(f"f1{l}", f"wf1T{l}", DEC, FF, xT,
                      act_evac(fT, f"fT{l}", 8, AF.Relu, lb[f"bf1{l}"]))

                zT = [None] * 4
                x_res2 = xT

                def evac_f2(m, pst, l=l, x_res2=x_res2, zT=zT):
                    t = tp.tile([128, N], f32r, name=f"zT{l}_{m}", tag="xT",
                                bufs=14)
                    nc.scalar.activation(t, pst, AF.Identity,
                                         bias=lb[f"bf2{l}"][:, m:m + 1])
                    nc.vector.tensor_add(t, t, x_res2[m])
                    zT[m] = t

                gemm2(f"f2{l}", f"wf2T{l}", FF, DEC, fT, evac_f2)
                xT = layer_norm(zT, lb[f"ln2g{l}"], lb[f"ln2b{l}"], f"ln2_{l}")

            if phase_limit < 8:
                return None
            # ---- outl2: weights-moving, bias as extra ones-row k-tile ----
            ol2_lhs = xT + [ones_row_r]
            kt2, _ = _kt_of(ol2_lhs)
            ps_o2 = pp.tile([N, 2 * OUT], f32, name="o2_ps", tag="ps_mm",
                            bufs=2, padded_shape=[N, CG])
            for ki, (kk, ks) in enumerate(kt2):
                w_sb = wp.tile([ks, 2 * OUT], f32r, name=f"ol2_w{ki}",
                               tag="ol2_w", bufs=5, padded_shape=[128, 2 * OUT])
                dma_rr(w_sb, dram["ol2T"].ap()[kk:kk + ks, :])
                nc.tensor.matmul(ps_o2, ol2_lhs[ki], w_sb, start=(ki == 0),
                                 stop=(ki == 4), skip_group_check=True)
            out_sb = wp.tile([N, 2 * OUT], f32, name="out_sb", bufs=1)
            nc.scalar.activation(out_sb, ps_o2, AF.Identity)
            nc.sync.dma_start(out=out_d.ap(), in_=out_sb)

            if phase_limit < 9:
                return None
            # ---- ctx output: transpose to batch-major ----
            ctx_sb = wp.tile([N, ENC], f32, name="ctx_sb", bufs=1)
            for k in range(4):
                tpk = pp.tile([N, 128], f32, name=f"tpc{k}", tag="ps_tr2",
                              bufs=1, padded_shape=[128, 128])
                nc.tensor.transpose(tpk, ctxT[k], ident_sb)
                nc.vector.tensor_copy(ctx_sb[:, k * 128:(k + 1) * 128], tpk)
            nc.sync.dma_start(out=ctx_d.ap(), in_=ctx_sb)

    nc.compile()
    return nc


def _chunks_of(M):
    out = []
    g0 = 0
    while g0 < M:
        out.append(min(CG, M - g0))
        g0 += out[-1]
    return out


def _tcols(wT):
    """(K, M) fp32 -> vstacked CG-wide column chunks (remainder zero-padded):
    shape (nch*K, CG); chunk c's block is rows [c*K:(c+1)*K]."""
    K, M = wT.shape
    blocks = []
    g0 = 0
    while g0 < M:
        wc = min(CG, M - g0)
        b = wT[:, g0:g0 + wc]
        if wc < CG:
            b = np.pad(b, ((0, 0), (0, CG - wc)))
        blocks.append(b)
        g0 += wc
    return np.ascontiguousarray(np.vstack(blocks))


def _bias128(b):
    """(M,) -> (128, M//128): col m holds b[m*128:(m+1)*128]."""
    M = b.shape[0]
    return np.ascontiguousarray(b.reshape(M // 128, 128).T)


def prep_inputs(inputs):
    fz = np.float32

    def g(name):
        return np.asarray(inputs[name], fz)

    ie = g("input_enc")
    spkr = g("spkr_vec")[:, 0, :]
    encT = np.ascontiguousarray(
        ie[:, :AW, :].transpose(2, 0, 1).reshape(ENC, NA))
    spkrT = np.ascontiguousarray(spkr.T)
    speedT = np.ascontiguousarray(g("speed").reshape(1, N))
    pinT = np.ascontiguousarray(
        np.concatenate([g("input_dec"), spkr], axis=1).T)
    lens = np.asarray(inputs["lengths_enc"]).astype(np.int64)
    t = np.arange(AW)
    mask = (t[None, :] <= np.minimum(AW - 1, lens[:, None] - 1)).astype(fz)
    maskneg = np.ascontiguousarray(((mask - 1.0) * 1e4).reshape(1, NA))
    convT = np.ascontiguousarray(g("conv_w")[:, 0, 15 - t])

    ol1T_full = g("outl1_w").T  # (3136, 512); rows: [h2 (2048), in_lstm (1088)]
    ol1T = np.concatenate([ol1T_full[H:], ol1T_full[:H]], axis=0)

    base = {
        "encT": encT, "spkrT": spkrT, "speedT": speedT, "pinT": pinT,
        "maskneg": maskneg, "convT": convT,
        "enc_wT": np.ascontiguousarray(g("enc_w").T),
        "enc_b": _bias128(g("enc_b")),
        "spkr_wT": np.ascontiguousarray(g("spkr_w").T),
        "sattT": np.ascontiguousarray(g("speed_att_w").T),
        "apT": _bias128(g("attproj_w").reshape(ATT)),
        "apb": np.ascontiguousarray(g("attproj_b").reshape(1, 1)),
        "spd1T": np.ascontiguousarray(g("spd1_w").T),
        "spd1b": _bias128(g("spd1_b")),
        "spd2T": _tcols(g("spd2_w").T),
        "spd2b": _bias128(g("spd2_b")),
        "pre1T": _tcols(g("pre1_w").T),
        "pre1b": _bias128(g("pre1_b")),
        "pre2T": _tcols(g("pre2_w").T),
        "pre2b": _bias128(g("pre2_b")),
        "ol1T": _tcols(ol1T),
        "ol1b": _bias128(g("outl1_b")),
        "ol2T": np.ascontiguousarray(
            np.concatenate([g("outl2_w").T, g("outl2_b").reshape(1, 2 * OUT)],
                           axis=0)),
        "ident": np.eye(128, dtype=fz),
    }
    for l in range(2):
        wv_ = g("tr_inproj_w")[l][1024:1536]
        bv_ = g("tr_inproj_b")[l][1024:1536]
        wo_ = g("tr_out_w")[l]
        bo_ = g("tr_out_b")[l]
        base[f"wvT{l}"] = _tcols(np.ascontiguousarray((wo_ @ wv_).T))
        base[f"bv{l}"] = _bias128(bo_ + wo_ @ bv_)
        base[f"ln1g{l}"] = _bias128(g("tr_ln1_g")[l])
        base[f"ln1b{l}"] = _bias128(g("tr_ln1_b")[l])
        base[f"wf1T{l}"] = _tcols(g("tr_ff1_w")[l].T)
        base[f"bf1{l}"] = _bias128(g("tr_ff1_b")[l])
        base[f"wf2T{l}"] = _tcols(g("tr_ff2_w")[l].T)
        base[f"bf2{l}"] = _bias128(g("tr_ff2_b")[l])
        base[f"ln2g{l}"] = _bias128(g("tr_ln2_g")[l])
        base[f"ln2b{l}"] = _bias128(g("tr_ln2_b")[l])

    wih0 = g("lstm_wih0")
    b0full = g("lstm_bih0") + g("lstm_bhh0")
    wih1 = g("lstm_wih1")
    b1full = g("lstm_bih1") + g("lstm_bhh1")
    rows_igo = np.concatenate(
        [np.arange(H), 2 * H + np.arange(H), 3 * H + np.arange(H)])
    w1_igo = wih1[rows_igo]
    base["b1"] = _bias128(b1full[rows_igo])
    in_maps = []
    for k in range(NCORES):
        hs = np.arange(k * HS, (k + 1) * HS)
        rows0 = np.concatenate([hs, 2 * H + hs, 3 * H + hs])
        m = dict(base)
        m["w0T"] = _tcols(np.ascontiguousarray(wih0[rows0].T))
        m["b0"] = _bias128(b0full[rows0])
        m["w1T"] = _tcols(
            np.ascontiguousarray(w1_igo[:, k * HS:(k + 1) * HS].T))
        in_maps.append(m)
    return in_maps


_NC = None


def _get_nc():
    global _NC
    if _NC is None:
        _NC = build_program()
    return _NC


_LAST_RESULTS = None


def kernel(**inputs):
    global _LAST_RESULTS
    import os
    nc = _get_nc()
    in_maps = prep_inputs(inputs)
    kw = {}
    if os.environ.get("KERNEL_TRACE"):
        kw["trace"] = True
    res = run_bass_kernel_spmd(nc, in_maps, core_ids=list(range(NCORES)), **kw)
    _LAST_RESULTS = res
    r0 = res.results[0]
    out = np.asarray(r0["out"], np.float32).reshape(N, 2, OUT)
    ctx = np.asarray(r0["ctx"], np.float32).reshape(N, 1, ENC)
    return out, ctx


# revision 28
# speedup vs baseline: 1.6410x; 1.0351x over previous
"""Trainium2 Bass kernel for nn_AttnDecoderRNN3 (sparse_attention).

Strategy (8 NeuronCores):
- Only input_enc[:, :10, :] matters: the attention mask is a fixed 10-wide
  window at t=0 (aw0 is a one-hot at t=0 -> argmax 0) and the softmax
  max-subtraction cancels exactly, so the full-T encoder GEMM is skipped.
- LSTM h0=c0=0 -> whh*/f-gate weights unused. Transformer S=1 -> att == V.
- LSTM layer 0 sharded over the hidden dim (each core computes a 256-wide
  h1 slice for all 32 batch rows); layer 1 sharded over the contraction
  dim (each core's h1 slice x its wih1 column slice -> partial full
  gates), combined with ONE AllReduce; biases added after the AllReduce.
- Everything else is computed redundantly on all cores for all 32 batch
  rows (it is tiny); final outputs are read from core 0.
- Big GEMMs: weights-MOVING through the PE at fp32r (1 col/cycle), with
  the batch (32) as the stationary operand; outputs [32, chunk] are
  PE-transposed back to feature-on-partition with bias+activation fused
  into the ScalarE evacuation. Small GEMMs stay exact fp32 stationary.
"""

import numpy as np

import concourse.bacc as bacc
import concourse.mybir as mybir
import concourse.tile as tile
from concourse.bass_utils import run_bass_kernel_spmd

NCORES = 8
N = 32            # batch
ENC = 512
ATT = 256
DEC = 512
H = 2048          # lstm hidden
SPK = 64
OUT = 80
FF = 1024
AW = 10           # attention window (ATT_RANGE)
HS = H // NCORES  # hidden slice per core = 256
G3 = 3 * H        # i,g,o gates = 6144
GS = 3 * HS       # per-core layer0 gate rows = 768
NA = N * AW       # 320
CG = 512          # gemm2 column-group (chunk) width

f32 = mybir.dt.float32
f32r = mybir.dt.float32r
AF = mybir.ActivationFunctionType
ALU = mybir.AluOpType
AX = mybir.AxisListType


def _ceil_div(a, b):
    return (a + b - 1) // b


def _kt_of(tiles):
    kt = []
    k0 = 0
    for r in tiles:
        ks = r.partition_size()
        kt.append((k0, ks))
        k0 += ks
    return kt, k0


def build_program(phase_limit=99):
    import os
    phase_limit = int(os.environ.get("KPHASE", phase_limit))
    nc = bacc.Bacc("TRN2", target_bir_lowering=False, debug=False,
                   num_devices=NCORES)

    dram = {}

    def din(name, shape, dt=f32):
        dram[name] = nc.dram_tensor(name, list(shape), dt,
                                    kind="ExternalInput")
        return dram[name]

    # ---- inputs ----
    # v2 (tiled, fp32r) weight tensors have shape (M/CG * K, CG)
    def din2(name, K, M):
        din(name, (_ceil_div(M, CG) * K, CG), f32r)

    din("encT", (ENC, NA))
    din("spkrT", (SPK, N))
    din("speedT", (1, N))
    din("pinT", (OUT + SPK, N), f32r)
    din("maskneg", (1, NA))
    din("convT", (ATT, AW))
    din("enc_wT", (ENC, ATT), f32r); din("enc_b", (128, (ATT) // 128))
    din("spkr_wT", (SPK, ATT))
    din("sattT", (1, ATT))
    din("apT", (128, 2)); din("apb", (1, 1))
    din("spd1T", (1, DEC)); din("spd1b", (128, (DEC) // 128))
    din2("spd2T", DEC, ENC); din("spd2b", (128, (ENC) // 128))
    din2("pre1T", OUT + SPK, 2 * DEC); din("pre1b", (128, (2 * DEC) // 128))
    din2("pre2T", 2 * DEC, DEC); din("pre2b", (128, (DEC) // 128))
    din2("w0T", ENC + DEC + SPK, GS); din("b0", (128, (GS) // 128))
    din2("w1T", HS, G3); din("b1", (128, (G3) // 128))
    din2("ol1T", H + ENC + DEC + SPK, DEC); din("ol1b", (128, (DEC) // 128))
    for l in range(2):
        din2(f"wvT{l}", DEC, DEC); din(f"bv{l}", (128, 4))
        din(f"ln1g{l}", (128, 4)); din(f"ln1b{l}", (128, 4))
        din2(f"wf1T{l}", DEC, FF); din(f"bf1{l}", (128, 8))
        din2(f"wf2T{l}", FF, DEC); din(f"bf2{l}", (128, 4))
        din(f"ln2g{l}", (128, 4)); din(f"ln2b{l}", (128, 4))
    din("ol2T", (DEC + 1, 2 * OUT), f32r)   # bias folded in as last row
    din("ident", (128, 128))
    out_d = nc.dram_tensor("out", [N, 2 * OUT], f32, kind="ExternalOutput")
    ctx_d = nc.dram_tensor("ctx", [N, ENC], f32, kind="ExternalOutput")

    with tile.TileContext(nc) as tc:
        with tc.tile_pool(name="wp", bufs=1) as wp, \
             tc.tile_pool(name="tp", bufs=2) as tp, \
             tc.tile_pool(name="pp", bufs=1, space="PSUM") as pp, \
             tc.tile_pool(name="dr", bufs=1, space="DRAM") as dr:

            _rr = [0]
            _dma_engs = [nc.sync, nc.gpsimd, nc.scalar]

            def dma_rr(out, in_):
                eng = _dma_engs[_rr[0] % len(_dma_engs)]
                _rr[0] += 1
                eng.dma_start(out=out, in_=in_)

            def sb_load(name, shape, tag=None, bufs=1, dt=f32):
                t = wp.tile(list(shape), dt, name=f"{name}_sb",
                            tag=tag or f"{name}_t", bufs=bufs)
                nc.sync.dma_start(out=t, in_=dram[name].ap())
                return t

            def load_bias(name, M):
                mt = _ceil_div(M, 128)
                assert M % 128 == 0
                t = wp.tile([128, mt], f32, name=f"{name}_sb",
                            tag=f"{name}_b", bufs=1)
                dma_rr(t, dram[name].ap())
                return t

            def softsign_from_psum(name, ps, bias_ap, w):
                val = tp.tile([128, w], f32, name=f"{name}_v",
                              tag=f"ss_v{w}", bufs=2)
                den = tp.tile([128, w], f32, name=f"{name}_d",
                              tag=f"ss_d{w}", bufs=2)
                if bias_ap is None:
                    nc.scalar.activation(val, ps, AF.Identity)
                    nc.scalar.activation(den, ps, AF.Abs)
                else:
                    nc.scalar.activation(val, ps, AF.Identity, bias=bias_ap)
                    nc.scalar.activation(den, ps, AF.Abs, bias=bias_ap)
                nc.vector.tensor_scalar_add(den, den, 1.0)
                nc.vector.reciprocal(den, den)
                nc.vector.tensor_mul(val, val, den)
                return val

            # ---- persistent SBUF loads ----
            encT_sb = []
            for k in range(4):
                t = wp.tile([128, NA], f32, name=f"encT{k}_sb", tag="encT",
                            bufs=4)
                dma_rr(t, dram["encT"].ap()[k * 128:(k + 1) * 128, :])
                encT_sb.append(t)
            encTr_sb = []
            for k in range(4):
                t = tp.tile([128, NA], f32r, name=f"encTr{k}", tag="encTr",
                            bufs=4)
                nc.vector.tensor_copy(t, encT_sb[k])
                encTr_sb.append(t)
            spkrT_sb = sb_load("spkrT", (SPK, N))
            speedT_sb = sb_load("speedT", (1, N))
            pinT_sb0 = wp.tile([128, N], f32r, name="pinT0", bufs=1)
            nc.sync.dma_start(out=pinT_sb0, in_=dram["pinT"].ap()[0:128, :])
            pinT_sb1 = wp.tile([16, N], f32r, name="pinT1", bufs=1)
            nc.sync.dma_start(out=pinT_sb1, in_=dram["pinT"].ap()[128:144, :])
            maskneg_sb = sb_load("maskneg", (1, NA))
            conv_sb = []
            for k in range(2):
                t = wp.tile([128, AW], f32, name=f"conv{k}_sb", tag="convT",
                            bufs=2)
                nc.sync.dma_start(out=t,
                                  in_=dram["convT"].ap()[k * 128:(k + 1) * 128, :])
                conv_sb.append(t)
            ident_sb = sb_load("ident", (128, 128))
            apb_sb = sb_load("apb", (1, 1))

            enc_b_sb = load_bias("enc_b", ATT)
            ap_sb = load_bias("apT", ATT)
            spd1b_sb = load_bias("spd1b", DEC)
            spd2b_sb = load_bias("spd2b", ENC)
            pre1b_sb = load_bias("pre1b", 2 * DEC)
            pre2b_sb = load_bias("pre2b", DEC)
            b0_sb = load_bias("b0", GS)
            b1_sb = load_bias("b1", G3)
            ol1b_sb = load_bias("ol1b", DEC)
            lb = {}
            for l in range(2):
                for nm in (f"bv{l}", f"ln1g{l}", f"ln1b{l}",
                           f"bf1{l}", f"bf2{l}", f"ln2g{l}", f"ln2b{l}"):
                    M = FF if nm.startswith("bf1") else DEC
                    lb[nm] = load_bias(nm, M)

            ones_col = wp.tile([128, 1], f32, name="ones_col", bufs=1)
            nc.vector.memset(ones_col, 1.0)
            ones_col_r = wp.tile([128, 1], f32r, name="ones_col_r", bufs=1)
            nc.vector.tensor_copy(ones_col_r, ones_col)
            ones_row = wp.tile([1, 128], f32, name="ones_row", bufs=1)
            nc.vector.memset(ones_row, 1.0)
            ones_row_r = wp.tile([1, N], f32r, name="ones_row_r", bufs=1)
            nc.vector.tensor_copy(ones_row_r, ones_row[:, :N])
            eps_sb = wp.tile([1, 1], f32, name="eps_sb", bufs=1)
            nc.vector.memset(eps_sb, 1e-5)

            # ================= gemm v2: weights-moving fp32r =================
            def _kt_runs(kt):
                """Maximal runs (cap 4) of consecutive full-128 k-tiles;
                <128 tiles become singleton runs."""
                runs = []
                i = 0
                while i < len(kt):
                    if kt[i][1] == 128:
                        j = i
                        while j < len(kt) and kt[j][1] == 128 and j - i < 4:
                            j += 1
                        runs.append((i, j - i))
                        i = j
                    else:
                        runs.append((i, 1))
                        i += 1
                return runs

            def gemm2(name, w_name, K, M, lhsT_tiles, evac_t, wbufs=4,
                      split_k=None):
                """y[32, M] = x @ W^T streamed in <=CG-col chunks; each
                128-col output tile is PE-transposed back to [128, 32] and
                handed to evac_t(m, psum_t) for fused bias+activation
                evacuation. One weight DMA per (chunk, k-tile-run)."""
                w_dram = dram[w_name]
                kt, ksum = _kt_of(lhsT_tiles)
                assert ksum == K, (name, ksum, K)
                nkt = len(kt)
                widths = _chunks_of(M)
                runs = _kt_runs(kt)

                def load_run(c, wc, i0, cnt):
                    kk, ks0 = kt[i0]
                    w_sb = wp.tile([ks0, cnt * wc], f32r,
                                   name=f"{name}_w{i0}_{c}",
                                   tag="w2" if ks0 == 128 else "w2r",
                                   bufs=wbufs, padded_shape=[128, 4 * CG])
                    rows = w_dram.ap()[c * K + kk:c * K + kk + cnt * ks0, :wc]
                    if cnt == 1:
                        dma_rr(w_sb, rows)
                    else:
                        dma_rr(w_sb.rearrange("p (t j) -> p t j", j=wc),
                               rows.rearrange("(t p) j -> p t j", p=128))
                    return w_sb

                def mm_runs(ps, c, wc, sel_runs):
                    for i0, cnt in sel_runs:
                        w_sb = load_run(c, wc, i0, cnt)
                        for idx in range(cnt):
                            ki = i0 + idx
                            nc.tensor.matmul(
                                ps, lhsT_tiles[ki],
                                w_sb[:, idx * wc:(idx + 1) * wc],
                                start=(ki == 0), stop=(ki == nkt - 1),
                                skip_group_check=True)

                m = 0
                if split_k is None:
                    for c, wc in enumerate(widths):
                        ps = pp.tile([N, wc], f32, name=f"{name}_ps{c}",
                                     tag="ps_mm", bufs=2,
                                     padded_shape=[N, CG])
                        mm_runs(ps, c, wc, runs)
                        _gemm2_evac(name, c, wc, m, ps, evac_t)
                        m += wc // 128
                else:
                    runs_a = [r for r in runs if r[0] < split_k]
                    runs_b = [r for r in runs if r[0] >= split_k]
                    assert sum(cnt for _, cnt in runs_a) == split_k
                    pss = {}
                    for c, wc in enumerate(widths):
                        pss[c] = pp.tile([N, wc], f32, name=f"{name}_ps{c}",
                                         tag="ps_mm", bufs=2,
                                         padded_shape=[N, CG])
                    for c, wc in enumerate(widths):
                        mm_runs(pss[c], c, wc, runs_a)
                    for c, wc in enumerate(widths):
                        mm_runs(pss[c], c, wc, runs_b)
                    for c, wc in enumerate(widths):
                        _gemm2_evac(name, c, wc, m, pss[c], evac_t)
                        m += wc // 128

            def _gemm2_evac(name, c, wc, m0, ps, evac_t):
                y32 = tp.tile([N, wc], f32, name=f"{name}_y{c}", tag="y32",
                              bufs=3, padded_shape=[N, CG])
                nc.vector.tensor_copy(y32, ps)
                for j in range(wc // 128):
                    m = m0 + j
                    pst = pp.tile([128, N], f32, name=f"{name}_pt{m}",
                                  tag="ps_tr2", bufs=2,
                                  padded_shape=[128, 128])
                    nc.tensor.transpose(pst, y32[:, j * 128:(j + 1) * 128],
                                        ident_sb[:N, :N])
                    evac_t(m, pst)

            def act_evac(out_list, tag, bufs, func, bias_sb, dt=f32r):
                def _e(m, pst):
                    t = tp.tile([128, N], dt, name=f"{tag}_{m}", tag=tag,
                                bufs=bufs)
                    f = func(m) if callable(func) else func
                    nc.scalar.activation(t, pst, f, bias=bias_sb[:, m:m + 1])
                    out_list[m] = t
                return _e

            # =========== attention (exact fp32, weights-stationary) ==========
            spkr_w_sb = sb_load("spkr_wT", (SPK, ATT))
            satt_sb = sb_load("sattT", (1, ATT))
            c1 = []
            for m in range(2):
                ps = pp.tile([128, N], f32, name=f"spk_ps{m}", tag="ps_main",
                             bufs=2, padded_shape=[128, N])
                nc.tensor.matmul(ps, spkr_w_sb[:, m * 128:(m + 1) * 128],
                                 spkrT_sb, start=True, stop=True)
                s_spkr = softsign_from_psum(f"sspkr{m}", ps, None, N)
                ps2 = pp.tile([128, N], f32, name=f"sat_ps{m}", tag="ps_main",
                              bufs=2, padded_shape=[128, N])
                nc.tensor.matmul(ps2, satt_sb[:, m * 128:(m + 1) * 128],
                                 speedT_sb, start=True, stop=True)
                cm = tp.tile([128, N], f32, name=f"c1_{m}", tag="c1", bufs=2)
                nc.vector.tensor_add(cm, s_spkr, ps2)
                c1.append(cm)

            enc_w_sb = []
            for k in range(4):
                t = wp.tile([128, ATT], f32r, name=f"encw{k}", tag="enc_wT",
                            bufs=4)
                dma_rr(t, dram["enc_wT"].ap()[k * 128:(k + 1) * 128, :])
                enc_w_sb.append(t)
            th = []
            for m in range(2):
                ps = pp.tile([128, NA], f32, name=f"eatt_ps{m}", tag="ps_misc",
                             bufs=2, padded_shape=[128, NA])
                for k in range(4):
                    nc.tensor.matmul(ps, enc_w_sb[k][:, m * 128:(m + 1) * 128],
                                     encTr_sb[k], start=(k == 0), stop=(k == 3))
                e = softsign_from_psum(f"senc{m}", ps, enc_b_sb[:, m:m + 1], NA)
                ev = e.rearrange("p (n t) -> p n t", t=AW)
                nc.vector.tensor_add(
                    ev, ev, c1[m].unsqueeze(2).broadcast_to([128, N, AW]))
                nc.vector.tensor_add(
                    ev, ev, conv_sb[m].unsqueeze(1).broadcast_to([128, N, AW]))
                tm = tp.tile([128, NA], f32, name=f"th{m}", tag="th", bufs=2)
                nc.scalar.activation(tm, e, AF.Tanh)
                th.append(tm)

            ps_l = pp.tile([1, NA], f32, name="log_ps", tag="ps_misc", bufs=2,
                           padded_shape=[128, NA])
            for m in range(2):
                nc.tensor.matmul(ps_l, ap_sb[:, m:m + 1], th[m],
                                 start=(m == 0), stop=(m == 1))
            logit = tp.tile([1, NA], f32, name="logit", bufs=1)
            nc.scalar.activation(logit, ps_l, AF.Identity, bias=apb_sb)
            nc.vector.tensor_add(logit, logit, maskneg_sb)
            lv = logit.rearrange("p (n t) -> p n t", t=AW)
            mx = tp.tile([1, N], f32, name="mx", tag="sm32", bufs=10)
            nc.vector.tensor_reduce(mx, lv, axis=AX.X, op=ALU.max)
            nc.vector.tensor_sub(lv, lv,
                                 mx.unsqueeze(2).broadcast_to([1, N, AW]))
            wexp = tp.tile([1, NA], f32, name="wexp", bufs=1)
            nc.scalar.activation(wexp, logit, AF.Exp)
            wv = wexp.rearrange("p (n t) -> p n t", t=AW)
            ssum = tp.tile([1, N], f32, name="ssum", tag="sm32", bufs=10)
            nc.vector.tensor_reduce(ssum, wv, axis=AX.X, op=ALU.add)
            nc.vector.tensor_scalar_max(ssum, ssum, 1e-12)
            rsum = tp.tile([1, N], f32, name="rsum", tag="sm32", bufs=10)
            nc.vector.reciprocal(rsum, ssum)
            nc.vector.tensor_mul(wv, wv,
                                 rsum.unsqueeze(2).broadcast_to([1, N, AW]))

            wb_ps = pp.tile([128, NA], f32, name="wb_ps", tag="ps_misc",
                            bufs=2, padded_shape=[128, NA])
            nc.tensor.matmul(wb_ps, ones_row, wexp, start=True, stop=True)
            sb_sum = tp.tile([128, N], f32, name="sb_sum", bufs=1)
            nc.vector.tensor_reduce(sb_sum,
                                    wb_ps.rearrange("p (n t) -> p n t", t=AW),
                                    axis=AX.X, op=ALU.add)

            # ---- speed projection ----
            spd1_sb = sb_load("spd1T", (1, DEC))
            r1T = [None] * 4
            for m in range(4):
                ps = pp.tile([128, N], f32, name=f"sp1_ps{m}", tag="ps_main",
                             bufs=2, padded_shape=[128, N])
                nc.tensor.matmul(ps, spd1_sb[:, m * 128:(m + 1) * 128],
                                 speedT_sb, start=True, stop=True)
                t = tp.tile([128, N], f32r, name=f"r1T{m}", tag="r1T", bufs=4)
                nc.scalar.activation(t, ps, AF.Relu, bias=spd1b_sb[:, m:m + 1])
                r1T[m] = t
            spT = [None] * 4
            gemm2("spd2", "spd2T", DEC, ENC, r1T,
                  act_evac(spT, "spT", 4, AF.Tanh, spd2b_sb, dt=f32))

            # ---- context ----
            ctxT = []
            ctxTr = []
            for k in range(4):
                prod = tp.tile([128, NA], f32, name=f"cprod{k}", tag="cprod",
                               bufs=2)
                nc.vector.tensor_mul(prod, encT_sb[k], wb_ps)
                a = tp.tile([128, N], f32, name=f"ctxT{k}", tag="ctxT", bufs=4)
                nc.vector.tensor_reduce(
                    a, prod.rearrange("p (n t) -> p n t", t=AW),
                    axis=AX.X, op=ALU.add)
                bt = tp.tile([128, N], f32, name=f"cb{k}", tag="cb", bufs=2)
                nc.vector.tensor_mul(bt, spT[k], sb_sum)
                nc.vector.tensor_add(a, a, bt)
                ctxT.append(a)
                ar = tp.tile([128, N], f32r, name=f"ctxTr{k}", tag="ctxTr",
                             bufs=4)
                nc.vector.tensor_copy(ar, a)
                ctxTr.append(ar)
            spkrTr = tp.tile([SPK, N], f32r, name="spkrTr", bufs=1)
            nc.vector.tensor_copy(spkrTr, spkrT_sb)

            if phase_limit < 2:
                return None
            # ---- prenet ----
            p1T = [None] * 8
            gemm2("pre1", "pre1T", OUT + SPK, 2 * DEC, [pinT_sb0, pinT_sb1],
                  act_evac(p1T, "p1T", 8, AF.Relu, pre1b_sb))
            preT = [None] * 4
            gemm2("pre2", "pre2T", 2 * DEC, DEC, p1T,
                  act_evac(preT, "preT", 4, AF.Relu, pre2b_sb))

            in_lstm_tiles = preT + ctxTr + [spkrTr]

            if phase_limit < 3:
                return None
            # ---- LSTM layer 0 ----
            l0 = [None] * 6
            gemm2("lstm0", "w0T", ENC + DEC + SPK, GS, in_lstm_tiles,
                  act_evac(l0, "l0", 6,
                           lambda m: AF.Sigmoid if (m < 2 or m >= 4) else AF.Tanh,
                           b0_sb, dt=f32))

            h1T = []
            for k in range(2):
                c = tp.tile([128, N], f32, name=f"c_{k}", tag="cc", bufs=2)
                nc.vector.tensor_mul(c, l0[k], l0[2 + k])
                tc_ = tp.tile([128, N], f32, name=f"tc_{k}", tag="cc", bufs=2)
                nc.scalar.activation(tc_, c, AF.Tanh)
                h = tp.tile([128, N], f32r, name=f"h1T{k}", tag="h1T", bufs=2)
                nc.vector.tensor_mul(h, l0[4 + k], tc_)
                h1T.append(h)

            if phase_limit < 4:
                return None
            # ---- LSTM layer 1 partial gates (batch-on-partition, no bias) ----
            g1p_sb = wp.tile([N, G3], f32, name="g1p_sb", bufs=1)

            def evac_l1_raw(name, c, ps):
                nc.scalar.activation(g1p_sb[:, c * CG:(c + 1) * CG], ps,
                                     AF.Identity)

            # inline gemm2 without transpose for lstm1
            for c in range(G3 // CG):
                w_sb = wp.tile([128, 2 * CG], f32r, name=f"l1_w{c}",
                               tag="w2", bufs=4, padded_shape=[128, 4 * CG])
                rows = dram["w1T"].ap()[c * HS:(c + 1) * HS, :]
                dma_rr(w_sb.rearrange("p (t j) -> p t j", j=CG),
                       rows.rearrange("(t p) j -> p t j", p=128))
                ps = pp.tile([N, CG], f32, name=f"l1_ps{c}", tag="ps_mm",
                             bufs=2, padded_shape=[N, CG])
                for ki in range(2):
                    nc.tensor.matmul(ps, h1T[ki],
                                     w_sb[:, ki * CG:(ki + 1) * CG],
                                     start=(ki == 0), stop=(ki == 1),
                                     skip_group_check=True)
                evac_l1_raw("lstm1", c, ps)

            if phase_limit < 5:
                return None
            # ---- AllReduce partial gates ----
            g1part = dr.tile([N, G3], f32, name="g1part")
            g1full = dr.tile([N, G3], f32, name="g1full", addr_space="Shared")
            nc.sync.dma_start(out=g1part, in_=g1p_sb)
            nc.gpsimd.collective_compute(
                "AllReduce", ALU.add,
                replica_groups=[list(range(NCORES))],
                ins=[g1part], outs=[g1full])
            g1f_sb = wp.tile([N, G3], f32, name="g1f_sb", bufs=1)
            nc.sync.dma_start(out=g1f_sb, in_=g1full)

            # transpose gates to feature-on-partition, add bias, h2
            W = 16 * N  # 512
            igo = []
            for gi in range(3):
                big = tp.tile([128, W], f32, name=f"igo{gi}", tag="hw", bufs=6)
                igo.append(big)
            for m in range(48):
                pst = pp.tile([128, N], f32, name=f"g1t{m}", tag="ps_tr2",
                              bufs=2, padded_shape=[128, 128])
                nc.tensor.transpose(pst, g1f_sb[:, m * 128:(m + 1) * 128],
                                    ident_sb[:N, :N])
                gi, t16 = divmod(m, 16)
                nc.scalar.activation(igo[gi][:, t16 * N:(t16 + 1) * N], pst,
                                     AF.Identity, bias=b1_sb[:, m:m + 1])
            sig_i = tp.tile([128, W], f32, name="sig_i", tag="hw", bufs=6)
            nc.scalar.activation(sig_i, igo[0], AF.Sigmoid)
            tan_g = tp.tile([128, W], f32, name="tan_g", tag="hw", bufs=6)
            nc.scalar.activation(tan_g, igo[1], AF.Tanh)
            nc.vector.tensor_mul(sig_i, sig_i, tan_g)           # c
            nc.scalar.activation(tan_g, sig_i, AF.Tanh)         # tanh(c)
            sig_o = tp.tile([128, W], f32, name="sig_o", tag="hw", bufs=6)
            nc.scalar.activation(sig_o, igo[2], AF.Sigmoid)
            h2_sb = wp.tile([128, W], f32r, name="h2_sb", bufs=1)
            nc.vector.tensor_mul(h2_sb, sig_o, tan_g)

            if phase_limit < 6:
                return None
            # ---- outl1 (in_lstm k-tiles first for AllReduce overlap) ----
            ol1_lhs = in_lstm_tiles + [h2_sb[:, t * N:(t + 1) * N]
                                       for t in range(16)]
            xT = [None] * 4
            gemm2("outl1", "ol1T", H + ENC + DEC + SPK, DEC, ol1_lhs,
                  act_evac(xT, "xT", 14, AF.Identity, ol1b_sb), split_k=9)

            if phase_limit < 7:
                return None

            # ---- transformer ----
            def layer_norm(x_tiles, g_sb, b_sb, nm):
                s_ps = pp.tile([1, N], f32, name=f"{nm}_s", tag="ps_misc",
                               bufs=2, padded_shape=[128, NA])
                for k in range(4):
                    nc.tensor.matmul(s_ps, ones_col_r, x_tiles[k],
                                     start=(k == 0), stop=(k == 3))
                s2_ps = pp.tile([1, N], f32, name=f"{nm}_s2", tag="ps_misc",
                                bufs=2, padded_shape=[128, NA])
                for k in range(4):
                    sq = tp.tile([128, N], f32r, name=f"{nm}_sq{k}", tag="sq",
                                 bufs=2)
                    nc.scalar.activation(sq, x_tiles[k], AF.Square)
                    nc.tensor.matmul(s2_ps, ones_col_r, sq, start=(k == 0),
                                     stop=(k == 3))
                mu = tp.tile([1, N], f32, name=f"{nm}_mu", tag="sm32", bufs=10)
                nc.scalar.activation(mu, s_ps, AF.Identity, scale=1.0 / DEC)
                ms = tp.tile([1, N], f32, name=f"{nm}_ms", tag="sm32", bufs=10)
                nc.scalar.activation(ms, s2_ps, AF.Identity, scale=1.0 / DEC)
                mu2 = tp.tile([1, N], f32, name=f"{nm}_mu2", tag="sm32",
                              bufs=10)
                nc.scalar.activation(mu2, mu, AF.Square)
                var = tp.tile([1, N], f32, name=f"{nm}_var", tag="sm32",
                              bufs=10)
                nc.vector.tensor_sub(var, ms, mu2)
                sd = tp.tile([1, N], f32, name=f"{nm}_sd", tag="sm32", bufs=10)
                nc.scalar.activation(sd, var, AF.Sqrt, bias=eps_sb)
                rstd = tp.tile([1, N], f32, name=f"{nm}_rstd", tag="sm32",
                               bufs=10)
                nc.vector.reciprocal(rstd, sd)
                mub_ps = pp.tile([128, N], f32, name=f"{nm}_mub",
                                 tag="ps_main", bufs=2, padded_shape=[128, N])
                nc.tensor.matmul(mub_ps, ones_row, mu, start=True, stop=True)
                rb_ps = pp.tile([128, N], f32, name=f"{nm}_rb", tag="ps_main",
                                bufs=2, padded_shape=[128, N])
                nc.tensor.matmul(rb_ps, ones_row, rstd, start=True, stop=True)
                out = []
                for k in range(4):
                    xc = tp.tile([128, N], f32, name=f"{nm}_xc{k}", tag="sq2",
                                 bufs=2)
                    nc.vector.tensor_sub(xc, x_tiles[k], mub_ps)
                    nc.vector.tensor_mul(xc, xc, rb_ps)
                    o = tp.tile([128, N], f32r, name=f"{nm}_o{k}", tag="xT",
                                bufs=14)
                    nc.scalar.activation(o, xc, AF.Identity,
                                         bias=b_sb[:, k:k + 1],
                                         scale=g_sb[:, k:k + 1])
                    out.append(o)
                return out

            for l in range(2):
                yT = [None] * 4
                x_res = xT

                def evac_vo(m, pst, l=l, x_res=x_res, yT=yT):
                    t = tp.tile([128, N], f32r, name=f"aT{l}_{m}", tag="xT",
                                bufs=14)
                    nc.scalar.activation(t, pst, AF.Identity,
                                         bias=lb[f"bv{l}"][:, m:m + 1])
                    nc.vector.tensor_add(t, t, x_res[m])
                    yT[m] = t

                gemm2(f"vo{l}", f"wvT{l}", DEC, DEC, xT, evac_vo)

                xT = layer_norm(yT, lb[f"ln1g{l}"], lb[f"ln1b{l}"], f"ln1_{l}")

                fT = [None] * 8
                gemm2(f"f1{l}", f"wf1T{l}", DEC, FF, xT,
                      act_evac(fT, f"fT{l}", 8, AF.Relu, lb[f"bf1{l}"]))

                zT = [None] * 4
                x_res2 = xT

                def evac_f2(m, pst, l=l, x_res2=x_res2, zT=zT):
                    t = tp.tile([128, N], f32r, name=f"zT{l}_{m}", tag="xT",
                                bufs=14)
                    nc.scalar.activation(t, pst, AF.Identity,
                                         bias=lb[f"bf2{l}"][:, m:m + 1])
                    nc.vector.tensor_add(t, t, x_res2[m])
                    zT[m] = t

                gemm2(f"f2{l}", f"wf2T{l}", FF, DEC, fT, evac_f2)
                xT = layer_norm(zT, lb[f"ln2g{l}"], lb[f"ln2b{l}"], f"ln2_{l}")

            if phase_limit < 8:
                return None
            # ---- outl2: weights-moving, bias as extra ones-row k-tile ----
            ol2_lhs = xT + [ones_row_r]
            kt2, _ = _kt_of(ol2_lhs)
            ps_o2 = pp.tile([N, 2 * OUT], f32, name="o2_ps", tag="ps_mm",
                            bufs=2, padded_shape=[N, CG])
            for ki, (kk, ks) in enumerate(kt2):
                w_sb = wp.tile([ks, 2 * OUT], f32r, name=f"ol2_w{ki}",
                               tag="ol2_w", bufs=5, padded_shape=[128, 2 * OUT])
                dma_rr(w_sb, dram["ol2T"].ap()[kk:kk + ks, :])
                nc.tensor.matmul(ps_o2, ol2_lhs[ki], w_sb, start=(ki == 0),
                                 stop=(ki == 4), skip_group_check=True)
            out_sb = wp.tile([N, 2 * OUT], f32, name="out_sb", bufs=1)
            nc.scalar.activation(out_sb, ps_o2, AF.Identity)
            nc.sync.dma_start(out=out_d.ap(), in_=out_sb)

            if phase_limit < 9:
                return None
            # ---- ctx output: transpose to batch-major ----
            ctx_sb = wp.tile([N, ENC], f32, name="ctx_sb", bufs=1)
            for k in range(4):
                tpk = pp.tile([N, 128], f32, name=f"tpc{k}", tag="ps_tr2",
                              bufs=2, padded_shape=[128, 128])
                nc.tensor.transpose(tpk, ctxT[k], ident_sb)
                nc.vector.tensor_copy(ctx_sb[:, k * 128:(k + 1) * 128], tpk)
            nc.sync.dma_start(out=ctx_d.ap(), in_=ctx_sb)

    nc.compile()
    return nc


def _chunks_of(M):
    out = []
    g0 = 0
    while g0 < M:
        out.append(min(CG, M - g0))
        g0 += out[-1]
    return out


def _tcols(wT):
    """(K, M) fp32 -> vstacked CG-wide column chunks (remainder zero-padded):
    shape (nch*K, CG); chunk c's block is rows [c*K:(c+1)*K]."""
    K, M = wT.shape
    blocks = []
    g0 = 0
    while g0 < M:
        wc = min(CG, M - g0)
        b = wT[:, g0:g0 + wc]
        if wc < CG:
            b = np.pad(b, ((0, 0), (0, CG - wc)))
        blocks.append(b)
        g0 += wc
    return np.ascontiguousarray(np.vstack(blocks))


def _bias128(b):
    """(M,) -> (128, M//128): col m holds b[m*128:(m+1)*128]."""
    M = b.shape[0]
    return np.ascontiguousarray(b.reshape(M // 128, 128).T)


def prep_inputs(inputs):
    fz = np.float32

    def g(name):
        return np.asarray(inputs[name], fz)

    ie = g("input_enc")
    spkr = g("spkr_vec")[:, 0, :]
    encT = np.ascontiguousarray(
        ie[:, :AW, :].transpose(2, 0, 1).reshape(ENC, NA))
    spkrT = np.ascontiguousarray(spkr.T)
    speedT = np.ascontiguousarray(g("speed").reshape(1, N))
    pinT = np.ascontiguousarray(
        np.concatenate([g("input_dec"), spkr], axis=1).T)
    lens = np.asarray(inputs["lengths_enc"]).astype(np.int64)
    t = np.arange(AW)
    mask = (t[None, :] <= np.minimum(AW - 1, lens[:, None] - 1)).astype(fz)
    maskneg = np.ascontiguousarray(((mask - 1.0) * 1e4).reshape(1, NA))
    convT = np.ascontiguousarray(g("conv_w")[:, 0, 15 - t])

    ol1T_full = g("outl1_w").T  # (3136, 512); rows: [h2 (2048), in_lstm (1088)]
    ol1T = np.concatenate([ol1T_full[H:], ol1T_full[:H]], axis=0)

    base = {
        "encT": encT, "spkrT": spkrT, "speedT": speedT, "pinT": pinT,
        "maskneg": maskneg, "convT": convT,
        "enc_wT": np.ascontiguousarray(g("enc_w").T),
        "enc_b": _bias128(g("enc_b")),
        "spkr_wT": np.ascontiguousarray(g("spkr_w").T),
        "sattT": np.ascontiguousarray(g("speed_att_w").T),
        "apT": _bias128(g("attproj_w").reshape(ATT)),
        "apb": np.ascontiguousarray(g("attproj_b").reshape(1, 1)),
        "spd1T": np.ascontiguousarray(g("spd1_w").T),
        "spd1b": _bias128(g("spd1_b")),
        "spd2T": _tcols(g("spd2_w").T),
        "spd2b": _bias128(g("spd2_b")),
        "pre1T": _tcols(g("pre1_w").T),
        "pre1b": _bias128(g("pre1_b")),
        "pre2T": _tcols(g("pre2_w").T),
        "pre2b": _bias128(g("pre2_b")),
        "ol1T": _tcols(ol1T),
        "ol1b": _bias128(g("outl1_b")),
        "ol2T": np.ascontiguousarray(
            np.concatenate([g("outl2_w").T, g("outl2_b").reshape(1, 2 * OUT)],
                           axis=0)),
        "ident": np.eye(128, dtype=fz),
    }
    for l in range(2):
        wv_ = g("tr_inproj_w")[l][1024:1536]
        bv_ = g("tr_inproj_b")[l][1024:1536]
        wo_ = g("tr_out_w")[l]
        bo_ = g("tr_out_b")[l]
        base[f"wvT{l}"] = _tcols(np.ascontiguousarray((wo_ @ wv_).T))
        base[f"bv{l}"] = _bias128(bo_ + wo_ @ bv_)
        base[f"ln1g{l}"] = _bias128(g("tr_ln1_g")[l])
        base[f"ln1b{l}"] = _bias128(g("tr_ln1_b")[l])
        base[f"wf1T{l}"] = _tcols(g("tr_ff1_w")[l].T)
        base[f"bf1{l}"] = _bias128(g("tr_ff1_b")[l])
        base[f"wf2T{l}"] = _tcols(g("tr_ff2_w")[l].T)
        base[f"bf2{l}"] = _bias128(g("tr_ff2_b")[l])
        base[f"ln2g{l}"] = _bias128(g("tr_ln2_g")[l])
        base[f"ln2b{l}"] = _bias128(g("tr_ln2_b")[l])

    wih0 = g("lstm_wih0")
    b0full = g("lstm_bih0") + g("lstm_bhh0")
    wih1 = g("lstm_wih1")
    b1full = g("lstm_bih1") + g("lstm_bhh1")
    rows_igo = np.concatenate(
        [np.arange(H), 2 * H + np.arange(H), 3 * H + np.arange(H)])
    w1_igo = wih1[rows_igo]
    base["b1"] = _bias128(b1full[rows_igo])
    in_maps = []
    for k in range(NCORES):
        hs = np.arange(k * HS, (k + 1) * HS)
        rows0 = np.concatenate([hs, 2 * H + hs, 3 * H + hs])
        m = dict(base)
        m["w0T"] = _tcols(np.ascontiguousarray(wih0[rows0].T))
        m["b0"] = _bias128(b0full[rows0])
        m["w1T"] = _tcols(
            np.ascontiguousarray(w1_igo[:, k * HS:(k + 1) * HS].T))
        in_maps.append(m)
    return in_maps


_NC = None


def _get_nc():
    global _NC
    if _NC is None:
        _NC = build_program()
    return _NC


_LAST_RESULTS = None


def kernel(**inputs):
    global _LAST_RESULTS
    import os
    nc = _get_nc()
    in_maps = prep_inputs(inputs)
    kw = {}
    if os.environ.get("KERNEL_TRACE"):
        kw["trace"] = True
    res = run_bass_kernel_spmd(nc, in_maps, core_ids=list(range(NCORES)), **kw)
    _LAST_RESULTS = res
    r0 = res.results[0]
    out = np.asarray(r0["out"], np.float32).reshape(N, 2, OUT)
    ctx = np.asarray(r0["ctx"], np.float32).reshape(N, 1, ENC)
    return out, ctx
